# revision 1
# baseline (speedup 1.0000x reference)
"""GCN (4x SAGEConv mean-agg + PReLU + BatchNorm, graph mean-pool) on 8 TRN2 NeuronCores.

Contract: kernel(**inputs) takes FULL inputs (as produced by setup_inputs) and
returns the FULL [G, 4H] output. Self-contained: all shapes/sharding hardcoded.

Sharding: nodes (and their in-edges, i.e. edges bucketed by dst) are
partitioned contiguously across 8 cores. Weights replicated. h is replicated
in HBM per layer via AllGather. BatchNorm stats and the final pooled output
use small AllReduces.

Aggregation: edges sorted by (dst-range, dst) and packed into 128-edge tiles
of <=32 whole dst runs. Per tile one indirect-DMA gather pulls h[src] rows
(128 x 512B descriptors); a (1/deg-weighted) indicator matmul reduces the tile
to its dst slots in PSUM (3 tiles per PSUM tile at base partitions 0/32/64);
all 1024 slots of a 24-tile call are flushed with ONE direct HWDGE DMA into a
slot-space DRAM buffer, and the main pass reads each 128-node block back with
a single 128-row indirect gather through the host-built inverse slot map
(deg-0 nodes point at a zeroed trailing block). Layer 1 needs no gather or
DRAM roundtrip at all: agg0 is a count-matrix matmul against the 257-row
embedding table, and the self term gathers from the tiny table directly.

The device-time floor on this hardware is the Pool engine's SWDGE fixed cost
(~1us per indirect DMA instruction, max 128 descriptors each); the batched
GPSIMD dma_gather/dma_scatter_add ucode that would lift it is not present in
this (bedrock) image.
"""

import numpy as np

import concourse.bass as bass
import concourse.tile as tile
from concourse import bacc, mybir
from concourse.masks import make_identity

FP = mybir.dt.float32
I16 = mybir.dt.int16
I32 = mybir.dt.int32

N_CORES = 8
P = 128          # partitions
J = 32           # dst slots (runs) per edge-tile
TPP = 3          # edge-tiles per PSUM tile (matmul out base partition 0/32/64)
PPC = 8          # PSUM tiles per call
TPC = TPP * PPC  # 24 edge-tiles per gather/scatter call
NIDX = TPC * P   # 3072 gather indices per call
NSLOT = PPC * P  # 1024 scatter slots per call
L = 4
EPS = 1e-5

# SEG = dst rows per agg range (per core, multiple of 128). Each range gets
# its own agg tensor so Tile can overlap main-pass blocks of completed ranges
# with the remaining ranges' gathers/scatters.
CFG_FULL = dict(N=100_000, G=128, H=128, NV=257, SEG=3_200)


def _mkcfg(N, G, H, NV, SEG):
    assert N % N_CORES == 0
    npc = N // N_CORES
    nblk = (npc + P - 1) // P
    last = npc - (nblk - 1) * P
    assert SEG % P == 0
    return dict(
        N=N, G=G, H=H, NV=NV, SEG=SEG, NPC=npc, NBLK=nblk, LAST=last,
        NSEG=(npc + SEG - 1) // SEG,
        NVC=(NV + P - 1) // P,
        AGG_ROWS=nblk * P,
    )


# ---------------------------------------------------------------------------
# host-side preprocessing
# ---------------------------------------------------------------------------

def _pack_tiles(run_len):
    """Pack runs (all of one src-segment, dst-sorted) into tiles of <=128
    edges and <=J runs, runs kept whole."""
    K = len(run_len)
    tile_of_run = np.empty(K, np.int64)
    slot_of_run = np.empty(K, np.int64)
    t = 0
    edges = 0
    runs = 0
    for k in range(K):
        r = int(run_len[k])
        if r > P:
            raise ValueError(f"in-degree run {r} exceeds {P}")
        if edges + r > P or runs >= J:
            t += 1
            edges = 0
            runs = 0
        tile_of_run[k] = t
        slot_of_run[k] = runs
        edges += r
        runs += 1
    return tile_of_run, slot_of_run, t + 1


def _prep_core(cfg, cc, src, dst, in_feat, invdeg):
    npc, nblk = cfg["NPC"], cfg["NBLK"]
    SEG, NSEG = cfg["SEG"], cfg["NSEG"]
    lo = cc * npc
    sel = (dst >= lo) & (dst < lo + npc)
    e_src = src[sel]
    e_dstl = (dst[sel] - lo).astype(np.int64)
    e_seg = e_dstl // SEG
    order = np.lexsort((e_dstl, e_seg))
    e_src = e_src[order].astype(np.int64)
    e_dstl = e_dstl[order]
    e_seg = e_seg[order]
    Ec = len(e_src)

    # runs of equal (seg, dst)
    key = e_seg * npc + e_dstl
    change = np.empty(Ec, bool)
    if Ec:
        change[0] = True
        change[1:] = key[1:] != key[:-1]
    run_starts = np.nonzero(change)[0]
    run_len = np.diff(np.concatenate([run_starts, [Ec]]))
    run_dst = e_dstl[run_starts]
    run_seg = e_seg[run_starts]

    # pack per segment
    seg_tiles = []          # per segment: (tile_of_run idx arrays etc.)
    tiles_per_seg = np.zeros(NSEG, np.int64)
    seg_run_sel = [np.nonzero(run_seg == s)[0] for s in range(NSEG)]
    packs = []
    for s in range(NSEG):
        rl = run_len[seg_run_sel[s]]
        if len(rl):
            tr, sr, T = _pack_tiles(rl)
        else:
            tr = np.empty(0, np.int64)
            sr = np.empty(0, np.int64)
            T = 0
        packs.append((tr, sr))
        tiles_per_seg[s] = T
    return dict(
        Ec=Ec, e_src=e_src, e_dstl=e_dstl, run_starts=run_starts,
        run_len=run_len, run_dst=run_dst, run_seg=run_seg,
        seg_run_sel=seg_run_sel, packs=packs, tiles_per_seg=tiles_per_seg,
        in_feat=in_feat, invdeg=invdeg, lo=lo,
    )


def _finish_core(cfg, core, calls_per_seg):
    npc, nblk = cfg["NPC"], cfg["NBLK"]
    SEG, NSEG, NVC = cfg["SEG"], cfg["NSEG"], cfg["NVC"]
    ncalls = int(calls_per_seg.sum())
    T_total = ncalls * TPC
    SCR = cfg["SEG"]  # scratch row, local to each range's agg tensor

    src_idx = np.zeros((T_total, P), np.int64)   # within-segment row idx
    ind = np.zeros((T_total, P, J), np.float32)
    slotrow = np.full((T_total, J), SCR, np.int64)

    call_base_of_seg = np.concatenate([[0], np.cumsum(calls_per_seg)])[:-1]
    invdeg = core["invdeg"]
    for s in range(NSEG):
        rsel = core["seg_run_sel"][s]
        if not len(rsel):
            continue
        tr, sr = core["packs"][s]
        tile_base = call_base_of_seg[s] * TPC
        run_starts = core["run_starts"][rsel]
        run_len = core["run_len"][rsel]
        run_dst = core["run_dst"][rsel]
        # per-edge position info (vectorized)
        n_e = int(run_len.sum())
        roe = np.repeat(np.arange(len(rsel)), run_len)     # run-of-edge (local)
        t_e = tr[roe] + tile_base
        # first edge index (global, within e_src) of each tile
        e_idx = np.repeat(run_starts, run_len) + (
            np.arange(n_e) - np.repeat(np.cumsum(run_len) - run_len, run_len))
        tfe = np.full(tr.max() + 1 + tile_base, 1 << 60, np.int64)
        np.minimum.at(tfe, t_e, e_idx)
        pos_e = e_idx - tfe[t_e]
        j_e = sr[roe]
        src_idx[t_e, pos_e] = core["e_src"][e_idx]
        dst_e = core["e_dstl"][e_idx]
        ind[t_e, pos_e, j_e] = invdeg[core["lo"] + dst_e]
        slotrow[tr + tile_base, sr] = run_dst - s * SEG

    # ---- device layouts (walrus indirect DMA: one int32 offset column per
    # instruction; gather per tile, scatter per 128-slot PSUM chunk) ----
    src32 = np.ascontiguousarray(src_idx.T.astype(np.int32))  # [128, T_total]
    # scatter chunk (call c, psum q): row p holds slot (tile c*TPC+q*TPP+p//J,
    # run p%J) for p < TPP*J, scratch otherwise
    scat_tok = np.full((ncalls, NSLOT), SCR, np.int64)
    sr_view = slotrow.reshape(ncalls, TPC, J)
    for q in range(PPC):
        blk = sr_view[:, q * TPP:(q + 1) * TPP, :].reshape(ncalls, TPP * J)
        scat_tok[:, q * P:q * P + TPP * J] = blk
    scat32 = np.ascontiguousarray(
        scat_tok.reshape(ncalls * PPC, P).T.astype(np.int32))  # [128, ncalls*PPC]

    ind_dev = np.ascontiguousarray(
        ind.transpose(1, 0, 2).reshape(P, T_total * J))

    # inverse slot map: own node row -> slot-space index (or the zero row)
    seg_of_call = np.repeat(np.arange(NSEG), calls_per_seg)
    ZROW = ncalls * NSLOT
    d2s = np.full(nblk * P, ZROW, np.int64)
    tt, jj = np.nonzero(slotrow != SCR)
    loc = slotrow[tt, jj] + seg_of_call[tt // TPC] * SEG
    q_ = (tt % TPC) // TPP
    p_ = ((tt % TPC) % TPP) * J + jj
    d2s[loc] = (tt // TPC) * NSLOT + q_ * P + p_
    d2s_dev = np.ascontiguousarray(d2s.reshape(nblk, P).T.astype(np.int32))

    # layer-1 count matrix (1/deg folded)
    NV = cfg["NV"]
    v_e = core["in_feat"][core["e_src"]]
    cntm = np.zeros(npc * NVC * P, np.float32)
    np.add.at(cntm, core["e_dstl"] * (NVC * P) + v_e, 1.0)
    cntm = cntm.reshape(npc, NVC * P) * invdeg[core["lo"]:core["lo"] + npc,
                                               None].astype(np.float32)
    cnt_pad = np.zeros((nblk * P, NVC * P), np.float32)
    cnt_pad[:npc] = cntm
    cb = cnt_pad.reshape(nblk, P, NVC, P)
    cnt_dev = np.ascontiguousarray(
        cb.transpose(0, 3, 2, 1).reshape(nblk, P, NVC * P))

    # layer-1 self gather indices (emb rows per own node) [128, nblk]
    feat = np.zeros(nblk * P, np.int64)
    feat[:npc] = core["in_feat"][core["lo"]:core["lo"] + npc]
    feat32 = np.ascontiguousarray(feat.reshape(nblk, P).T.astype(np.int32))

    return dict(src32=src32, scat32=scat32, ind=ind_dev, cnt=cnt_dev,
                feat32=feat32, d2s=d2s_dev, ncalls0=0)


def _prep(cfg, in_feat, src, dst, graph_ids, emb, W_self, W_neigh, b,
          gamma, beta, prelu_w):
    N, G, H = cfg["N"], cfg["G"], cfg["H"]
    npc, nblk = cfg["NPC"], cfg["NBLK"]
    NV, NVC, NSEG = cfg["NV"], cfg["NVC"], cfg["NSEG"]
    in_feat = np.asarray(in_feat).astype(np.int64)
    src = np.asarray(src).astype(np.int64)
    dst = np.asarray(dst).astype(np.int64)
    graph_ids = np.asarray(graph_ids).astype(np.int64)

    deg = np.bincount(dst, minlength=N)
    invdeg = (1.0 / np.clip(deg, 1, None)).astype(np.float64)

    cores = [_prep_core(cfg, cc, src, dst, in_feat, invdeg)
             for cc in range(N_CORES)]
    calls_per_seg = np.zeros(NSEG, np.int64)
    for s in range(NSEG):
        mx = max(int(c["tiles_per_seg"][s]) for c in cores)
        calls_per_seg[s] = (mx + TPC - 1) // TPC
    ncalls = int(calls_per_seg.sum())

    fins = [_finish_core(cfg, c, calls_per_seg) for c in cores]

    cnt_g = np.clip(np.bincount(graph_ids, minlength=G), 1, None)
    emb_pad = np.zeros((NVC * P, H), np.float32)
    emb_pad[:NV] = np.asarray(emb, np.float32)

    # per-call dst-range index (same on every core)
    seg_bounds = []
    for s in range(NSEG):
        seg_bounds += [s] * int(calls_per_seg[s])

    in_maps = []
    for cc, fin in enumerate(fins):
        lo = cc * npc
        gown = np.zeros(nblk * P, np.int64)
        gown[:npc] = graph_ids[lo:lo + npc]
        gind = np.zeros((nblk * P, G), np.float32)
        gind[np.arange(npc), gown[:npc]] = 1.0 / cnt_g[gown[:npc]]
        gind = np.ascontiguousarray(gind.reshape(nblk, P, G))

        in_maps.append(dict(
            src32=fin["src32"], scat32=fin["scat32"], ind=fin["ind"],
            cnt=fin["cnt"], feat32=fin["feat32"], d2s=fin["d2s"],
            gind=gind, emb=emb_pad,
            zeros=np.zeros((cfg["SEG"] + P, H), np.float32),
            W_self=np.ascontiguousarray(np.asarray(W_self, np.float32)),
            W_neigh=np.ascontiguousarray(np.asarray(W_neigh, np.float32)),
            b_cols=np.ascontiguousarray(np.asarray(b, np.float32).T),
            gam_cols=np.ascontiguousarray(np.asarray(gamma, np.float32).T),
            bet_cols=np.ascontiguousarray(np.asarray(beta, np.float32).T),
            alp_cols=np.ascontiguousarray(np.asarray(prelu_w, np.float32).T),
        ))
    return in_maps, ncalls, seg_bounds, fins[0]["ncalls0"]


# ---------------------------------------------------------------------------
# device program
# ---------------------------------------------------------------------------

def build_program(cfg, ncalls, seg_bounds, ncalls0, ablate=()):
    N, G, H = cfg["N"], cfg["G"], cfg["H"]
    npc, nblk, last = cfg["NPC"], cfg["NBLK"], cfg["LAST"]
    NVC = cfg["NVC"]
    agg_rows = cfg["AGG_ROWS"]
    T_total = ncalls * TPC

    nc = bacc.Bacc("TRN2", target_bir_lowering=False, debug=False,
                   num_devices=N_CORES)

    src32_d = nc.declare_dram_parameter("src32", [P, T_total], I32,
                                        isOutput=False)
    scat32_d = nc.declare_dram_parameter("scat32", [P, ncalls * PPC], I32,
                                         isOutput=False)
    ind_d = nc.declare_dram_parameter("ind", [P, T_total * J], FP, isOutput=False)
    cnt_d = nc.declare_dram_parameter("cnt", [nblk, P, NVC * P], FP, isOutput=False)
    feat32_d = nc.declare_dram_parameter("feat32", [P, nblk], I32,
                                         isOutput=False)
    d2s_d = nc.declare_dram_parameter("d2s", [P, nblk], I32, isOutput=False)
    gind_d = nc.declare_dram_parameter("gind", [nblk, P, G], FP, isOutput=False)
    emb_d = nc.declare_dram_parameter("emb", [NVC * P, H], FP, isOutput=False)
    zeros_d = nc.declare_dram_parameter("zeros", [cfg["SEG"] + P, H], FP,
                                        isOutput=False)
    ws_d = nc.declare_dram_parameter("W_self", [L, H, H], FP, isOutput=False)
    wn_d = nc.declare_dram_parameter("W_neigh", [L, H, H], FP, isOutput=False)
    bcol_d = nc.declare_dram_parameter("b_cols", [H, L], FP, isOutput=False)
    gcol_d = nc.declare_dram_parameter("gam_cols", [H, L], FP, isOutput=False)
    becol_d = nc.declare_dram_parameter("bet_cols", [H, L], FP, isOutput=False)
    acol_d = nc.declare_dram_parameter("alp_cols", [H, L], FP, isOutput=False)
    out_d = nc.declare_dram_parameter("out", [G, L * H], FP, isOutput=True)

    NSEG = cfg["NSEG"]
    SEG = cfg["SEG"]
    h_shard = nc.dram_tensor("h_shard", [npc, H], FP)
    h_full = nc.dram_tensor("h_full", [N, H], FP, addr_space="Shared")
    # slot-space aggregation buffer + one trailing zero block for deg-0 rows
    slotbuf = nc.dram_tensor("slotbuf", [ncalls * NSLOT + P, H], FP)
    stats_loc = nc.dram_tensor("stats_loc", [2, H], FP)
    stats_red = nc.dram_tensor("stats_red", [2, H], FP, addr_space="Shared")
    pool_loc = nc.dram_tensor("pool_loc", [L, G, H], FP)
    pool_red = nc.dram_tensor("pool_red", [L, G, H], FP, addr_space="Shared")

    groups = [list(range(N_CORES))]

    with tile.TileContext(nc) as tc:
        with (
            tc.tile_pool(name="res", bufs=1) as res,
            tc.tile_pool(name="wrk", bufs=3) as wrk,
            tc.tile_pool(name="gat", bufs=3) as gat,
            tc.tile_pool(name="ps_slot", bufs=3, space="PSUM") as ps_slot,
            tc.tile_pool(name="ps_tp", bufs=2, space="PSUM") as ps_tp,
            tc.tile_pool(name="ps_rst", bufs=2, space="PSUM") as ps_rst,
            tc.tile_pool(name="ps_pool", bufs=1, space="PSUM") as ps_pool,
        ):
            ident = res.tile([P, P], FP, tag="ident")
            make_identity(nc, ident[:])

            src32_sb = res.tile([P, T_total], I32, tag="src32")
            nc.sync.dma_start(src32_sb[:], src32_d[:])
            scat32_sb = res.tile([P, ncalls * PPC], I32, tag="scat32")
            nc.sync.dma_start(scat32_sb[:], scat32_d[:])
            feat32_sb = res.tile([P, nblk], I32, tag="feat32")
            nc.sync.dma_start(feat32_sb[:], feat32_d[:])
            d2s_sb = res.tile([P, nblk], I32, tag="d2s")
            nc.sync.dma_start(d2s_sb[:], d2s_d[:])
            emb_sb = res.tile([P, NVC * H], FP, tag="emb")
            for c in range(NVC):
                nc.sync.dma_start(emb_sb[:, c * H:(c + 1) * H],
                                  emb_d[c * P:(c + 1) * P, :])
            ws_sb = res.tile([P, L * H], FP, tag="ws")
            wn_sb = res.tile([P, L * H], FP, tag="wn")
            for l in range(L):
                nc.sync.dma_start(ws_sb[:, l * H:(l + 1) * H], ws_d[l])
                nc.sync.dma_start(wn_sb[:, l * H:(l + 1) * H], wn_d[l])
            bcol_sb = res.tile([P, L], FP, tag="bcol")
            nc.sync.dma_start(bcol_sb[:], bcol_d[:])
            gcol_sb = res.tile([P, L], FP, tag="gcol")
            nc.sync.dma_start(gcol_sb[:], gcol_d[:])
            becol_sb = res.tile([P, L], FP, tag="becol")
            nc.sync.dma_start(becol_sb[:], becol_d[:])
            acol_sb = res.tile([P, L], FP, tag="acol")
            nc.sync.dma_start(acol_sb[:], acol_d[:])

            h_stage = res.tile([P, nblk * P], FP, tag="hstage")
            stats_sum = res.tile([P, nblk], FP, tag="ssum")
            stats_sq = res.tile([P, nblk], FP, tag="ssq")
            scratch = res.tile([P, P], FP, tag="scratch")
            eps_col = res.tile([P, 1], FP, tag="eps")
            nc.vector.memset(eps_col[:], float(EPS))

            # S staging: partitions [TPP*J:128) of each chunk feed zeros
            S_bufs = []
            for i in range(4):
                Sb = res.tile([P, PPC * H], FP, tag=f"S{i}")
                nc.vector.memset(Sb[TPP * J:, :], 0.0)
                S_bufs.append(Sb)

            # zero the trailing slotbuf block once (deg-0 rows point here)
            nc.sync.dma_start(slotbuf[ncalls * NSLOT:, :], zeros_d[:P, :])

            for l in range(L):
                # ---------------- aggregation ----------------
                if l > 0:
                    for c in range(ncalls):
                        gt = gat.tile([P, TPC * H], FP, tag="g")
                        if "gather" not in ablate:
                            for ti in range(TPC):
                                t_glob = c * TPC + ti
                                nc.gpsimd.indirect_dma_start(
                                    out=gt[:, ti * H:(ti + 1) * H],
                                    out_offset=None, in_=h_full[:],
                                    in_offset=bass.IndirectOffsetOnAxis(
                                        ap=src32_sb[:, t_glob:t_glob + 1],
                                        axis=0))
                        it = wrk.tile([P, TPC * J], FP, tag="indblk")
                        nc.sync.dma_start(
                            it[:], ind_d[:, c * TPC * J:(c + 1) * TPC * J])
                        S = S_bufs[c % 4]
                        for q in range(PPC):
                            ps = ps_slot.tile([P, H], FP, tag="slot")
                            if "aggmm" not in ablate:
                                for ti in range(TPP):
                                    t_loc = q * TPP + ti
                                    nc.tensor.matmul(
                                        ps[ti * J:(ti + 1) * J, :],
                                        lhsT=it[:, t_loc * J:(t_loc + 1) * J],
                                        rhs=gt[:, t_loc * H:(t_loc + 1) * H],
                                        start=True, stop=True)
                                nc.vector.tensor_copy(
                                    S[:TPP * J, q * H:(q + 1) * H],
                                    ps[:TPP * J, :])
                        if "scatter" not in ablate:
                            # one direct HWDGE DMA flushes all 1024 slots
                            nc.sync.dma_start(
                                slotbuf[c * NSLOT:(c + 1) * NSLOT, :]
                                .rearrange("(q p) f -> p q f", p=P),
                                S[:].rearrange("p (q f) -> p q f", f=H))

                # ---------------- main (pass A) ----------------
                for bI in range(nblk):
                    nn = last if bI == nblk - 1 else P
                    ab = wrk.tile([P, H], FP, tag="mablk")
                    if l == 0:
                        cnt_sb = wrk.tile([P, NVC * H], FP, tag="cntblk")
                        nc.sync.dma_start(cnt_sb[:], cnt_d[bI])
                        ps_a = ps_rst.tile([P, H], FP, tag="rst")
                        for cv in range(NVC):
                            nc.tensor.matmul(
                                ps_a[:],
                                lhsT=cnt_sb[:, cv * H:(cv + 1) * H],
                                rhs=emb_sb[:, cv * H:(cv + 1) * H],
                                start=(cv == 0), stop=(cv == NVC - 1))
                        nc.vector.tensor_copy(ab[:], ps_a[:])
                    else:
                        nc.gpsimd.indirect_dma_start(
                            out=ab[:], out_offset=None, in_=slotbuf[:],
                            in_offset=bass.IndirectOffsetOnAxis(
                                ap=d2s_sb[:, bI:bI + 1], axis=0))
                    ps_t = ps_tp.tile([P, P], FP, tag="tp")
                    nc.tensor.transpose(out=ps_t[:], in_=ab[:], identity=ident[:])
                    aT = wrk.tile([P, P], FP, tag="aT")
                    nc.scalar.copy(aT[:], ps_t[:])

                    if l == 0:
                        g0 = wrk.tile([P, H], FP, tag="g0")
                        nc.gpsimd.indirect_dma_start(
                            out=g0[:], out_offset=None, in_=emb_d[:],
                            in_offset=bass.IndirectOffsetOnAxis(
                                ap=feat32_sb[:, bI:bI + 1], axis=0))
                        ps_t0 = ps_tp.tile([P, P], FP, tag="tp")
                        nc.tensor.transpose(out=ps_t0[:], in_=g0[:],
                                            identity=ident[:])
                        hT = wrk.tile([P, P], FP, tag="hT")
                        nc.scalar.copy(hT[:], ps_t0[:])
                        rhs_self = hT[:]
                    else:
                        rhs_self = h_stage[:, bI * P:(bI + 1) * P]

                    ps_r = ps_rst.tile([P, H], FP, tag="rst")
                    nc.tensor.matmul(ps_r[:], lhsT=ws_sb[:, l * H:(l + 1) * H],
                                     rhs=rhs_self, start=True, stop=False)
                    nc.tensor.matmul(ps_r[:], lhsT=wn_sb[:, l * H:(l + 1) * H],
                                     rhs=aT[:], start=False, stop=True)

                    bc = bcol_sb[:, l:l + 1]
                    t1 = wrk.tile([P, P], FP, tag="t1")
                    nc.scalar.activation(t1[:], ps_r[:],
                                         mybir.ActivationFunctionType.Relu,
                                         bias=bc)
                    neg = wrk.tile([P, P], FP, tag="neg")
                    nc.vector.tensor_scalar(
                        neg[:], ps_r[:], bc, 0.0,
                        op0=mybir.AluOpType.add, op1=mybir.AluOpType.min)
                    if nn == P:
                        nc.vector.scalar_tensor_tensor(
                            h_stage[:, bI * P:(bI + 1) * P],
                            neg[:], acol_sb[:, l:l + 1], t1[:],
                            op0=mybir.AluOpType.mult, op1=mybir.AluOpType.add,
                            accum_out=stats_sum[:, bI:bI + 1])
                        nc.scalar.activation(scratch[:],
                                             h_stage[:, bI * P:(bI + 1) * P],
                                             mybir.ActivationFunctionType.Square,
                                             accum_out=stats_sq[:, bI:bI + 1])
                    else:
                        nc.vector.scalar_tensor_tensor(
                            h_stage[:, bI * P:bI * P + nn],
                            neg[:, :nn], acol_sb[:, l:l + 1], t1[:, :nn],
                            op0=mybir.AluOpType.mult, op1=mybir.AluOpType.add,
                            accum_out=stats_sum[:, bI:bI + 1])
                        nc.vector.scalar_tensor_tensor(
                            h_stage[:, bI * P + nn:(bI + 1) * P],
                            neg[:, nn:], acol_sb[:, l:l + 1], t1[:, nn:],
                            op0=mybir.AluOpType.mult, op1=mybir.AluOpType.add)
                        nc.scalar.activation(
                            scratch[:, :nn], h_stage[:, bI * P:bI * P + nn],
                            mybir.ActivationFunctionType.Square,
                            accum_out=stats_sq[:, bI:bI + 1])

                # ---------------- BN stats + allreduce ----------------
                sx = wrk.tile([P, 1], FP, tag="sx")
                nc.vector.tensor_reduce(sx[:], stats_sum[:],
                                        axis=mybir.AxisListType.X,
                                        op=mybir.AluOpType.add)
                sq = wrk.tile([P, 1], FP, tag="sq")
                nc.vector.tensor_reduce(sq[:], stats_sq[:],
                                        axis=mybir.AxisListType.X,
                                        op=mybir.AluOpType.add)
                nc.sync.dma_start(stats_loc[0:1, :], sx[:, 0:1])
                nc.sync.dma_start(stats_loc[1:2, :], sq[:, 0:1])
                nc.gpsimd.collective_compute(
                    "AllReduce", mybir.AluOpType.add, replica_groups=groups,
                    ins=[stats_loc[:]], outs=[stats_red[:]])
                sxr = wrk.tile([P, 1], FP, tag="sxr")
                nc.sync.dma_start(sxr[:, 0:1], stats_red[0:1, :])
                sqr = wrk.tile([P, 1], FP, tag="sqr")
                nc.sync.dma_start(sqr[:, 0:1], stats_red[1:2, :])

                mu = wrk.tile([P, 1], FP, tag="mu")
                nc.scalar.mul(mu[:], sxr[:], 1.0 / N)
                ex2 = wrk.tile([P, 1], FP, tag="ex2")
                nc.scalar.mul(ex2[:], sqr[:], 1.0 / N)
                mu2 = wrk.tile([P, 1], FP, tag="mu2")
                nc.scalar.square(mu2[:], mu[:])
                var = wrk.tile([P, 1], FP, tag="var")
                nc.vector.tensor_sub(var[:], ex2[:], mu2[:])
                sd = wrk.tile([P, 1], FP, tag="sd")
                nc.scalar.activation(sd[:], var[:],
                                     mybir.ActivationFunctionType.Sqrt,
                                     bias=eps_col[:])
                rstd = wrk.tile([P, 1], FP, tag="rstd")
                nc.vector.reciprocal(rstd[:], sd[:])
                scale = wrk.tile([P, 1], FP, tag="scale")
                nc.vector.tensor_mul(scale[:], rstd[:], gcol_sb[:, l:l + 1])
                msc = wrk.tile([P, 1], FP, tag="msc")
                nc.vector.tensor_mul(msc[:], mu[:], scale[:])
                shift = wrk.tile([P, 1], FP, tag="shift")
                nc.vector.tensor_sub(shift[:], becol_sb[:, l:l + 1], msc[:])

                # ---------------- pass B ----------------
                ps_p = ps_pool.tile([P, H], FP, tag="pool")
                for bI in range(nblk):
                    nn = last if bI == nblk - 1 else P
                    sl = h_stage[:, bI * P:(bI + 1) * P]
                    nc.vector.scalar_tensor_tensor(
                        sl, sl, scale[:], shift[:].to_broadcast([P, P]),
                        op0=mybir.AluOpType.mult, op1=mybir.AluOpType.add)
                    ps_t = ps_tp.tile([P, P], FP, tag="tp")
                    nc.tensor.transpose(out=ps_t[:], in_=sl, identity=ident[:])
                    hnm = wrk.tile([P, P], FP, tag="hnm")
                    nc.scalar.copy(hnm[:], ps_t[:])
                    if l < L - 1:
                        nc.sync.dma_start(
                            h_shard[bI * P:bI * P + nn, :], hnm[:nn, :])
                    gb = wrk.tile([P, G], FP, tag="gblk")
                    nc.sync.dma_start(gb[:], gind_d[bI])
                    nc.tensor.matmul(ps_p[:G, :], lhsT=gb[:], rhs=hnm[:],
                                     start=(bI == 0), stop=(bI == nblk - 1))
                pl = wrk.tile([P, H], FP, tag="pl")
                nc.vector.tensor_copy(pl[:G, :], ps_p[:G, :])
                nc.sync.dma_start(pool_loc[l], pl[:G, :])

                if l < L - 1:
                    nc.gpsimd.collective_compute(
                        "AllGather", mybir.AluOpType.bypass,
                        replica_groups=groups,
                        ins=[h_shard[:]], outs=[h_full[:]])

            nc.gpsimd.collective_compute(
                "AllReduce", mybir.AluOpType.add, replica_groups=groups,
                ins=[pool_loc[:]], outs=[pool_red[:]])
            for l in range(L):
                ob = wrk.tile([P, H], FP, tag="ob")
                nc.sync.dma_start(ob[:G, :], pool_red[l])
                nc.sync.dma_start(out_d[:, l * H:(l + 1) * H], ob[:G, :])

    nc.compile()
    return nc


# ---------------------------------------------------------------------------
# entry point
# ---------------------------------------------------------------------------

_CACHE = {}


def _run(cfg, inputs, trace=False):
    from concourse.bass_utils import run_bass_kernel_spmd
    in_maps, ncalls, seg_bounds, ncalls0 = _prep(cfg, **inputs)
    key = (cfg["N"], cfg["G"], cfg["H"], ncalls, tuple(seg_bounds))
    if key not in _CACHE:
        _CACHE[key] = build_program(cfg, ncalls, seg_bounds, ncalls0)
    nc = _CACHE[key]
    last_exc = None
    for attempt in range(3):
        try:
            return run_bass_kernel_spmd(nc, in_maps, list(range(N_CORES)),
                                        trace=trace)
        except Exception as e:  # rare transient device-unrecoverable errors
            last_exc = e
            try:
                import jax
                import jax.extend.backend
                jax.clear_caches()
                jax.extend.backend.clear_backends()
            except Exception:
                pass
    raise last_exc


def kernel(in_feat, src, dst, graph_ids, emb, W_self, W_neigh, b,
           gamma, beta, prelu_w):
    cfg = _mkcfg(**CFG_FULL)
    res = _run(cfg, dict(
        in_feat=in_feat, src=src, dst=dst, graph_ids=graph_ids, emb=emb,
        W_self=W_self, W_neigh=W_neigh, b=b, gamma=gamma, beta=beta,
        prelu_w=prelu_w))
    return np.asarray(res.results[0]["out"], np.float32)



# revision 10
# speedup vs baseline: 1.5722x; 1.5722x over previous
"""GCN (4x SAGEConv mean-agg + PReLU + BatchNorm, graph mean-pool) on 8 TRN2 NeuronCores.

Contract: kernel(**inputs) takes FULL inputs (as produced by setup_inputs) and
returns the FULL [G, 4H] output. Self-contained: all shapes/sharding hardcoded.

Sharding: nodes (and their in-edges, i.e. edges bucketed by dst) are
partitioned contiguously across 8 cores. Weights replicated. h is replicated
in HBM per layer via AllGather. BatchNorm stats and the final pooled output
use small AllReduces.

Aggregation: edges sorted by (dst-range, dst) and packed into 128-edge tiles
of <=32 whole dst runs. Per tile one indirect-DMA gather pulls h[src] rows
(128 x 512B descriptors); a (1/deg-weighted) indicator matmul reduces the tile
to its dst slots in PSUM (3 tiles per PSUM tile at base partitions 0/32/64);
all 1024 slots of a 24-tile call are flushed with ONE direct HWDGE DMA into a
slot-space DRAM buffer, and the main pass reads each 128-node block back with
a single 128-row indirect gather through the host-built inverse slot map
(deg-0 nodes point at a zeroed trailing block). Layer 1 needs no gather or
DRAM roundtrip at all: agg0 is a count-matrix matmul against the 257-row
embedding table, and the self term gathers from the tiny table directly.

The device-time floor on this hardware is the Pool engine's SWDGE fixed cost
(~1us per indirect DMA instruction, max 128 descriptors each); the batched
GPSIMD dma_gather/dma_scatter_add ucode that would lift it is not present in
this (bedrock) image.
"""

import numpy as np

import concourse.bass as bass
import concourse.tile as tile
from concourse import bacc, mybir
from concourse.masks import make_identity

FP = mybir.dt.float32
I16 = mybir.dt.int16
I32 = mybir.dt.int32

N_CORES = 8
P = 128          # partitions
J = 32           # dst slots (runs) per edge-tile
TPP = 3          # edge-tiles per PSUM tile (matmul out base partition 0/32/64)
PPC = 8          # PSUM tiles per call
TPC = TPP * PPC  # 24 edge-tiles per gather/scatter call
NIDX = TPC * P   # 3072 gather indices per call
NSLOT = PPC * P  # 1024 scatter slots per call
L = 4
EPS = 1e-5

# SEG = dst rows per agg range (per core, multiple of 128). Each range gets
# its own agg tensor so Tile can overlap main-pass blocks of completed ranges
# with the remaining ranges' gathers/scatters.
CFG_FULL = dict(N=100_000, G=128, H=128, NV=257, SEG=3_200)


def _mkcfg(N, G, H, NV, SEG):
    assert N % N_CORES == 0
    npc = N // N_CORES
    nblk = (npc + P - 1) // P
    last = npc - (nblk - 1) * P
    assert SEG % P == 0
    return dict(
        N=N, G=G, H=H, NV=NV, SEG=SEG, NPC=npc, NBLK=nblk, LAST=last,
        NSEG=(npc + SEG - 1) // SEG,
        NVC=(NV + P - 1) // P,
        AGG_ROWS=nblk * P,
    )


# ---------------------------------------------------------------------------
# host-side preprocessing
# ---------------------------------------------------------------------------

def _pack_tiles(run_len):
    """Pack runs (all of one src-segment, dst-sorted) into tiles of <=128
    edges and <=J runs, runs kept whole."""
    K = len(run_len)
    tile_of_run = np.empty(K, np.int64)
    slot_of_run = np.empty(K, np.int64)
    t = 0
    edges = 0
    runs = 0
    for k in range(K):
        r = int(run_len[k])
        if r > P:
            raise ValueError(f"in-degree run {r} exceeds {P}")
        if edges + r > P or runs >= J:
            t += 1
            edges = 0
            runs = 0
        tile_of_run[k] = t
        slot_of_run[k] = runs
        edges += r
        runs += 1
    return tile_of_run, slot_of_run, t + 1


def _prep_core(cfg, cc, src, dst, in_feat, invdeg):
    npc, nblk = cfg["NPC"], cfg["NBLK"]
    SEG, NSEG = cfg["SEG"], cfg["NSEG"]
    lo = cc * npc
    sel = (dst >= lo) & (dst < lo + npc)
    e_src = src[sel]
    e_dstl = (dst[sel] - lo).astype(np.int64)
    e_seg = e_dstl // SEG
    order = np.lexsort((e_dstl, e_seg))
    e_src = e_src[order].astype(np.int64)
    e_dstl = e_dstl[order]
    e_seg = e_seg[order]
    Ec = len(e_src)

    # runs of equal (seg, dst)
    key = e_seg * npc + e_dstl
    change = np.empty(Ec, bool)
    if Ec:
        change[0] = True
        change[1:] = key[1:] != key[:-1]
    run_starts = np.nonzero(change)[0]
    run_len = np.diff(np.concatenate([run_starts, [Ec]]))
    run_dst = e_dstl[run_starts]
    run_seg = e_seg[run_starts]

    # pack per segment
    seg_tiles = []          # per segment: (tile_of_run idx arrays etc.)
    tiles_per_seg = np.zeros(NSEG, np.int64)
    seg_run_sel = [np.nonzero(run_seg == s)[0] for s in range(NSEG)]
    packs = []
    for s in range(NSEG):
        rl = run_len[seg_run_sel[s]]
        if len(rl):
            tr, sr, T = _pack_tiles(rl)
        else:
            tr = np.empty(0, np.int64)
            sr = np.empty(0, np.int64)
            T = 0
        packs.append((tr, sr))
        tiles_per_seg[s] = T
    return dict(
        Ec=Ec, e_src=e_src, e_dstl=e_dstl, run_starts=run_starts,
        run_len=run_len, run_dst=run_dst, run_seg=run_seg,
        seg_run_sel=seg_run_sel, packs=packs, tiles_per_seg=tiles_per_seg,
        in_feat=in_feat, invdeg=invdeg, lo=lo,
    )


def _finish_core(cfg, core, calls_per_seg):
    npc, nblk = cfg["NPC"], cfg["NBLK"]
    SEG, NSEG, NVC = cfg["SEG"], cfg["NSEG"], cfg["NVC"]
    ncalls = int(calls_per_seg.sum())
    T_total = ncalls * TPC
    SCR = cfg["SEG"]  # scratch row, local to each range's agg tensor

    src_idx = np.zeros((T_total, P), np.int64)   # within-segment row idx
    ind = np.zeros((T_total, P, J), np.float32)
    slotrow = np.full((T_total, J), SCR, np.int64)

    call_base_of_seg = np.concatenate([[0], np.cumsum(calls_per_seg)])[:-1]
    invdeg = core["invdeg"]
    for s in range(NSEG):
        rsel = core["seg_run_sel"][s]
        if not len(rsel):
            continue
        tr, sr = core["packs"][s]
        tile_base = call_base_of_seg[s] * TPC
        run_starts = core["run_starts"][rsel]
        run_len = core["run_len"][rsel]
        run_dst = core["run_dst"][rsel]
        # per-edge position info (vectorized)
        n_e = int(run_len.sum())
        roe = np.repeat(np.arange(len(rsel)), run_len)     # run-of-edge (local)
        t_e = tr[roe] + tile_base
        # first edge index (global, within e_src) of each tile
        e_idx = np.repeat(run_starts, run_len) + (
            np.arange(n_e) - np.repeat(np.cumsum(run_len) - run_len, run_len))
        tfe = np.full(tr.max() + 1 + tile_base, 1 << 60, np.int64)
        np.minimum.at(tfe, t_e, e_idx)
        pos_e = e_idx - tfe[t_e]
        j_e = sr[roe]
        src_idx[t_e, pos_e] = core["e_src"][e_idx]
        dst_e = core["e_dstl"][e_idx]
        ind[t_e, pos_e, j_e] = invdeg[core["lo"] + dst_e]
        slotrow[tr + tile_base, sr] = run_dst - s * SEG

    # ---- device layouts (walrus indirect DMA: one int32 offset column per
    # instruction; gather per tile, scatter per 128-slot PSUM chunk) ----
    src32 = np.ascontiguousarray(src_idx.T.astype(np.int32))  # [128, T_total]
    # scatter chunk (call c, psum q): row p holds slot (tile c*TPC+q*TPP+p//J,
    # run p%J) for p < TPP*J, scratch otherwise
    scat_tok = np.full((ncalls, NSLOT), SCR, np.int64)
    sr_view = slotrow.reshape(ncalls, TPC, J)
    for q in range(PPC):
        blk = sr_view[:, q * TPP:(q + 1) * TPP, :].reshape(ncalls, TPP * J)
        scat_tok[:, q * P:q * P + TPP * J] = blk
    scat32 = np.ascontiguousarray(
        scat_tok.reshape(ncalls * PPC, P).T.astype(np.int32))  # [128, ncalls*PPC]

    ind_dev = np.ascontiguousarray(
        ind.transpose(1, 0, 2).reshape(P, T_total * J))

    # inverse slot map: own node row -> slot-space index (or the zero row)
    seg_of_call = np.repeat(np.arange(NSEG), calls_per_seg)
    ZROW = ncalls * NSLOT
    d2s = np.full(nblk * P, ZROW, np.int64)
    tt, jj = np.nonzero(slotrow != SCR)
    loc = slotrow[tt, jj] + seg_of_call[tt // TPC] * SEG
    q_ = (tt % TPC) // TPP
    p_ = ((tt % TPC) % TPP) * J + jj
    d2s[loc] = (tt // TPC) * NSLOT + q_ * P + p_
    d2s_dev = np.ascontiguousarray(d2s.reshape(nblk, P).T.astype(np.int32))

    # layer-1 count matrix (1/deg folded)
    NV = cfg["NV"]
    v_e = core["in_feat"][core["e_src"]]
    cntm = np.zeros(npc * NVC * P, np.float32)
    np.add.at(cntm, core["e_dstl"] * (NVC * P) + v_e, 1.0)
    cntm = cntm.reshape(npc, NVC * P) * invdeg[core["lo"]:core["lo"] + npc,
                                               None].astype(np.float32)
    cnt_pad = np.zeros((nblk * P, NVC * P), np.float32)
    cnt_pad[:npc] = cntm
    cb = cnt_pad.reshape(nblk, P, NVC, P)
    cnt_dev = np.ascontiguousarray(
        cb.transpose(0, 3, 2, 1).reshape(nblk, P, NVC * P))

    # layer-1 self gather indices (emb rows per own node) [128, nblk]
    feat = np.zeros(nblk * P, np.int64)
    feat[:npc] = core["in_feat"][core["lo"]:core["lo"] + npc]
    feat32 = np.ascontiguousarray(feat.reshape(nblk, P).T.astype(np.int32))

    return dict(src32=src32, scat32=scat32, ind=ind_dev, cnt=cnt_dev,
                feat32=feat32, d2s=d2s_dev, ncalls0=0)


def _prep(cfg, in_feat, src, dst, graph_ids, emb, W_self, W_neigh, b,
          gamma, beta, prelu_w):
    N, G, H = cfg["N"], cfg["G"], cfg["H"]
    npc, nblk = cfg["NPC"], cfg["NBLK"]
    NV, NVC, NSEG = cfg["NV"], cfg["NVC"], cfg["NSEG"]
    in_feat = np.asarray(in_feat).astype(np.int64)
    src = np.asarray(src).astype(np.int64)
    dst = np.asarray(dst).astype(np.int64)
    graph_ids = np.asarray(graph_ids).astype(np.int64)

    deg = np.bincount(dst, minlength=N)
    invdeg = (1.0 / np.clip(deg, 1, None)).astype(np.float64)

    cores = [_prep_core(cfg, cc, src, dst, in_feat, invdeg)
             for cc in range(N_CORES)]
    calls_per_seg = np.zeros(NSEG, np.int64)
    for s in range(NSEG):
        mx = max(int(c["tiles_per_seg"][s]) for c in cores)
        calls_per_seg[s] = (mx + TPC - 1) // TPC
    ncalls = int(calls_per_seg.sum())

    fins = [_finish_core(cfg, c, calls_per_seg) for c in cores]

    cnt_g = np.clip(np.bincount(graph_ids, minlength=G), 1, None)
    emb_pad = np.zeros((NVC * P, H), np.float32)
    emb_pad[:NV] = np.asarray(emb, np.float32)

    # per-call dst-range index (same on every core)
    seg_bounds = []
    for s in range(NSEG):
        seg_bounds += [s] * int(calls_per_seg[s])

    in_maps = []
    for cc, fin in enumerate(fins):
        lo = cc * npc
        gown = np.zeros(nblk * P, np.int64)
        gown[:npc] = graph_ids[lo:lo + npc]
        gind = np.zeros((nblk * P, G), np.float32)
        gind[np.arange(npc), gown[:npc]] = 1.0 / cnt_g[gown[:npc]]
        gind = np.ascontiguousarray(gind.reshape(nblk, P, G))

        tflag = np.full((P, 1), 1.0 if cc == 0 else 0.0, np.float32)
        in_maps.append(dict(
            src32=fin["src32"], scat32=fin["scat32"], ind=fin["ind"],
            cnt=fin["cnt"], feat32=fin["feat32"], d2s=fin["d2s"],
            gind=gind, emb=emb_pad,
            zeros=np.zeros((cfg["SEG"] + P, H), np.float32),
            W_self=np.ascontiguousarray(np.asarray(W_self, np.float32)),
            W_neigh=np.ascontiguousarray(np.asarray(W_neigh, np.float32)),
            W_sum=np.ascontiguousarray(
                np.asarray(W_self, np.float32) + np.asarray(W_neigh, np.float32)),
            tflag=tflag,
            b_cols=np.ascontiguousarray(np.asarray(b, np.float32).T),
            gam_cols=np.ascontiguousarray(np.asarray(gamma, np.float32).T),
            bet_cols=np.ascontiguousarray(np.asarray(beta, np.float32).T),
            alp_cols=np.ascontiguousarray(np.asarray(prelu_w, np.float32).T),
        ))
    return in_maps, ncalls, seg_bounds, fins[0]["ncalls0"]


# ---------------------------------------------------------------------------
# device program
# ---------------------------------------------------------------------------

def build_program(cfg, ncalls, seg_bounds, ncalls0, ablate=()):
    """BN-folded pipeline: the exchanged/gathered per-node state is z = the
    pre-BatchNorm PReLU output. h = z*s + t (per-channel affine from batch
    stats) is folded into the next layer's weights on device:
      rst_pre[l+1] = z @ (diag(s_l) W_self) + agg(z) @ (diag(s_l) W_neigh)
                     + (t_l @ (W_self+W_neigh) + b)          [min in-deg >= 1]
    so the BN stats AllReduce and the affine are OFF the critical path
    (computed while the next layer's gather stream runs), and the old pass B
    (normalize + transpose + pool) is fused into pass A. Graph pooling
    accumulates raw z; the affine is applied to the [H, G] pooled tile
    (t added on core 0 only) before the final AllReduce."""
    N, G, H = cfg["N"], cfg["G"], cfg["H"]
    npc, nblk, last = cfg["NPC"], cfg["NBLK"], cfg["LAST"]
    NVC = cfg["NVC"]
    agg_rows = cfg["AGG_ROWS"]
    T_total = ncalls * TPC

    nc = bacc.Bacc("TRN2", target_bir_lowering=False, debug=False,
                   num_devices=N_CORES)

    src32_d = nc.declare_dram_parameter("src32", [P, T_total], I32,
                                        isOutput=False)
    scat32_d = nc.declare_dram_parameter("scat32", [P, ncalls * PPC], I32,
                                         isOutput=False)
    ind_d = nc.declare_dram_parameter("ind", [P, T_total * J], FP, isOutput=False)
    cnt_d = nc.declare_dram_parameter("cnt", [nblk, P, NVC * P], FP, isOutput=False)
    feat32_d = nc.declare_dram_parameter("feat32", [P, nblk], I32,
                                         isOutput=False)
    d2s_d = nc.declare_dram_parameter("d2s", [P, nblk], I32, isOutput=False)
    gind_d = nc.declare_dram_parameter("gind", [nblk, P, G], FP, isOutput=False)
    emb_d = nc.declare_dram_parameter("emb", [NVC * P, H], FP, isOutput=False)
    zeros_d = nc.declare_dram_parameter("zeros", [cfg["SEG"] + P, H], FP,
                                        isOutput=False)
    ws_d = nc.declare_dram_parameter("W_self", [L, H, H], FP, isOutput=False)
    wn_d = nc.declare_dram_parameter("W_neigh", [L, H, H], FP, isOutput=False)
    wsum_d = nc.declare_dram_parameter("W_sum", [L, H, H], FP, isOutput=False)
    tflag_d = nc.declare_dram_parameter("tflag", [P, 1], FP, isOutput=False)
    bcol_d = nc.declare_dram_parameter("b_cols", [H, L], FP, isOutput=False)
    gcol_d = nc.declare_dram_parameter("gam_cols", [H, L], FP, isOutput=False)
    becol_d = nc.declare_dram_parameter("bet_cols", [H, L], FP, isOutput=False)
    acol_d = nc.declare_dram_parameter("alp_cols", [H, L], FP, isOutput=False)
    out_d = nc.declare_dram_parameter("out", [G, L * H], FP, isOutput=True)

    NSEG = cfg["NSEG"]
    SEG = cfg["SEG"]
    h_shard = nc.dram_tensor("h_shard", [npc, H], FP)
    h_full = nc.dram_tensor("h_full", [N, H], FP, addr_space="Shared")
    # slot-space aggregation buffer + one trailing zero block for deg-0 rows
    slotbuf = nc.dram_tensor("slotbuf", [ncalls * NSLOT + P, H], FP)
    stats_loc = nc.dram_tensor("stats_loc", [2, H], FP)
    stats_red = nc.dram_tensor("stats_red", [2, H], FP, addr_space="Shared")
    # pooled z held TRANSPOSED [H, G] so the channel affine uses per-partition
    # scalars; transposed back to [G, H] only at the very end
    pool_loc = nc.dram_tensor("pool_loc", [L, H, G], FP)
    pool_red = nc.dram_tensor("pool_red", [L, H, G], FP, addr_space="Shared")

    groups = [list(range(N_CORES))]

    with tile.TileContext(nc) as tc:
        with (
            tc.tile_pool(name="res", bufs=1) as res,
            tc.tile_pool(name="wrk", bufs=3) as wrk,
            tc.tile_pool(name="gat", bufs=3) as gat,
            tc.tile_pool(name="ps_slot", bufs=3, space="PSUM") as ps_slot,
            tc.tile_pool(name="ps_tp", bufs=2, space="PSUM") as ps_tp,
            tc.tile_pool(name="ps_rst", bufs=2, space="PSUM") as ps_rst,
            tc.tile_pool(name="ps_pool", bufs=1, space="PSUM") as ps_pool,
        ):
            ident = res.tile([P, P], FP, tag="ident")
            make_identity(nc, ident[:])

            src32_sb = res.tile([P, T_total], I32, tag="src32")
            nc.sync.dma_start(src32_sb[:], src32_d[:])
            scat32_sb = res.tile([P, ncalls * PPC], I32, tag="scat32")
            nc.sync.dma_start(scat32_sb[:], scat32_d[:])
            feat32_sb = res.tile([P, nblk], I32, tag="feat32")
            nc.sync.dma_start(feat32_sb[:], feat32_d[:])
            d2s_sb = res.tile([P, nblk], I32, tag="d2s")
            nc.sync.dma_start(d2s_sb[:], d2s_d[:])
            emb_sb = res.tile([P, NVC * H], FP, tag="emb")
            for c in range(NVC):
                nc.sync.dma_start(emb_sb[:, c * H:(c + 1) * H],
                                  emb_d[c * P:(c + 1) * P, :])
            ws_sb = res.tile([P, L * H], FP, tag="ws")
            wn_sb = res.tile([P, L * H], FP, tag="wn")
            wsum_sb = res.tile([P, L * H], FP, tag="wsum")
            for l in range(L):
                nc.sync.dma_start(ws_sb[:, l * H:(l + 1) * H], ws_d[l])
                nc.sync.dma_start(wn_sb[:, l * H:(l + 1) * H], wn_d[l])
                nc.sync.dma_start(wsum_sb[:, l * H:(l + 1) * H], wsum_d[l])
            tflag_sb = res.tile([P, 1], FP, tag="tflag")
            nc.sync.dma_start(tflag_sb[:], tflag_d[:])
            bcol_sb = res.tile([P, L], FP, tag="bcol")
            nc.sync.dma_start(bcol_sb[:], bcol_d[:])
            gcol_sb = res.tile([P, L], FP, tag="gcol")
            nc.sync.dma_start(gcol_sb[:], gcol_d[:])
            becol_sb = res.tile([P, L], FP, tag="becol")
            nc.sync.dma_start(becol_sb[:], becol_d[:])
            acol_sb = res.tile([P, L], FP, tag="acol")
            nc.sync.dma_start(acol_sb[:], acol_d[:])

            h_stage = res.tile([P, nblk * P], FP, tag="hstage")
            stats_sum = res.tile([P, nblk], FP, tag="ssum")
            stats_sq = res.tile([P, nblk], FP, tag="ssq")
            scratch = res.tile([P, P], FP, tag="scratch")
            eps_col = res.tile([P, 1], FP, tag="eps")
            nc.vector.memset(eps_col[:], float(EPS))
            # per-layer BN affine (s, t), folded weights and bias columns
            s_all = res.tile([P, L], FP, tag="sall")
            t_all = res.tile([P, L], FP, tag="tall")
            wsf_sb = res.tile([P, L * H], FP, tag="wsf")
            wnf_sb = res.tile([P, L * H], FP, tag="wnf")
            biasf_sb = res.tile([P, L], FP, tag="biasf")
            pl_all = res.tile([P, L * H], FP, tag="plall")

            # S staging: partitions [TPP*J:128) of each chunk feed zeros
            S_bufs = []
            for i in range(4):
                Sb = res.tile([P, PPC * H], FP, tag=f"S{i}")
                nc.vector.memset(Sb[TPP * J:, :], 0.0)
                S_bufs.append(Sb)

            # zero the trailing slotbuf block once (deg-0 rows point here)
            nc.sync.dma_start(slotbuf[ncalls * NSLOT:, :], zeros_d[:P, :])

            def emit_agg_call(c):
                gt = gat.tile([P, TPC * H], FP, tag="g")
                if "gather" not in ablate:
                    for ti in range(TPC):
                        t_glob = c * TPC + ti
                        nc.gpsimd.indirect_dma_start(
                            out=gt[:, ti * H:(ti + 1) * H],
                            out_offset=None, in_=h_full[:],
                            in_offset=bass.IndirectOffsetOnAxis(
                                ap=src32_sb[:, t_glob:t_glob + 1],
                                axis=0))
                it = wrk.tile([P, TPC * J], FP, tag="indblk")
                nc.sync.dma_start(
                    it[:], ind_d[:, c * TPC * J:(c + 1) * TPC * J])
                S = S_bufs[c % 4]
                for q in range(PPC):
                    ps = ps_slot.tile([P, H], FP, tag="slot")
                    if "aggmm" not in ablate:
                        for ti in range(TPP):
                            t_loc = q * TPP + ti
                            nc.tensor.matmul(
                                ps[ti * J:(ti + 1) * J, :],
                                lhsT=it[:, t_loc * J:(t_loc + 1) * J],
                                rhs=gt[:, t_loc * H:(t_loc + 1) * H],
                                start=True, stop=True)
                        nc.vector.tensor_copy(
                            S[:TPP * J, q * H:(q + 1) * H],
                            ps[:TPP * J, :])
                if "scatter" not in ablate:
                    # one direct HWDGE DMA flushes all 1024 slots
                    nc.sync.dma_start(
                        slotbuf[c * NSLOT:(c + 1) * NSLOT, :]
                        .rearrange("(q p) f -> p q f", p=P),
                        S[:].rearrange("p (q f) -> p q f", f=H))

            def emit_stats_post(j):
                """s_j, t_j from the (already AllReduced) stats of z^j; fold
                layer j+1's weights/bias; apply the pool affine for layer j."""
                sxr = wrk.tile([P, 1], FP, tag="sxr")
                nc.sync.dma_start(sxr[:, 0:1], stats_red[0:1, :])
                sqr = wrk.tile([P, 1], FP, tag="sqr")
                nc.sync.dma_start(sqr[:, 0:1], stats_red[1:2, :])
                mu = wrk.tile([P, 1], FP, tag="mu")
                nc.scalar.mul(mu[:], sxr[:], 1.0 / N)
                ex2 = wrk.tile([P, 1], FP, tag="ex2")
                nc.scalar.mul(ex2[:], sqr[:], 1.0 / N)
                mu2 = wrk.tile([P, 1], FP, tag="mu2")
                nc.scalar.square(mu2[:], mu[:])
                var = wrk.tile([P, 1], FP, tag="var")
                nc.vector.tensor_sub(var[:], ex2[:], mu2[:])
                sd = wrk.tile([P, 1], FP, tag="sd")
                nc.scalar.activation(sd[:], var[:],
                                     mybir.ActivationFunctionType.Sqrt,
                                     bias=eps_col[:])
                rstd = wrk.tile([P, 1], FP, tag="rstd")
                nc.vector.reciprocal(rstd[:], sd[:])
                s_col = s_all[:, j:j + 1]
                t_col = t_all[:, j:j + 1]
                nc.vector.tensor_mul(s_col, rstd[:], gcol_sb[:, j:j + 1])
                msc = wrk.tile([P, 1], FP, tag="msc")
                nc.vector.tensor_mul(msc[:], mu[:], s_col)
                nc.vector.tensor_sub(t_col, becol_sb[:, j:j + 1], msc[:])
                if j < L - 1:
                    ln = j + 1
                    nc.vector.tensor_scalar_mul(
                        wsf_sb[:, ln * H:(ln + 1) * H],
                        ws_sb[:, ln * H:(ln + 1) * H], s_col)
                    nc.vector.tensor_scalar_mul(
                        wnf_sb[:, ln * H:(ln + 1) * H],
                        wn_sb[:, ln * H:(ln + 1) * H], s_col)
                    ps_b = ps_rst.tile([P, H], FP, tag="rst")
                    nc.tensor.matmul(ps_b[:, 0:1],
                                     lhsT=wsum_sb[:, ln * H:(ln + 1) * H],
                                     rhs=t_col, start=True, stop=True)
                    nc.vector.tensor_add(biasf_sb[:, ln:ln + 1], ps_b[:, 0:1],
                                         bcol_sb[:, ln:ln + 1])
                # pool affine for layer j: [H, G] = s*poolT + t (core 0 only)
                ps_t = ps_tp.tile([P, P], FP, tag="tp")
                nc.tensor.transpose(out=ps_t[:],
                                    in_=pl_all[:, j * H:(j + 1) * H],
                                    identity=ident[:])
                poolT = wrk.tile([P, P], FP, tag="poolT")
                nc.scalar.copy(poolT[:], ps_t[:])
                tf = wrk.tile([P, 1], FP, tag="tf")
                nc.vector.tensor_mul(tf[:], t_col, tflag_sb[:])
                pla = wrk.tile([P, G], FP, tag="pla")
                nc.vector.scalar_tensor_tensor(
                    pla[:], poolT[:, :G], s_col, tf[:].to_broadcast([P, G]),
                    op0=mybir.AluOpType.mult, op1=mybir.AluOpType.add)
                nc.sync.dma_start(pool_loc[j], pla[:])

            def emit_mainpass(l):
                ps_p = ps_pool.tile([P, H], FP, tag="pool")
                for bI in range(nblk):
                    nn = last if bI == nblk - 1 else P
                    ab = wrk.tile([P, H], FP, tag="mablk")
                    if l == 0:
                        cnt_sb = wrk.tile([P, NVC * H], FP, tag="cntblk")
                        nc.sync.dma_start(cnt_sb[:], cnt_d[bI])
                        ps_a = ps_rst.tile([P, H], FP, tag="rst")
                        for cv in range(NVC):
                            nc.tensor.matmul(
                                ps_a[:],
                                lhsT=cnt_sb[:, cv * H:(cv + 1) * H],
                                rhs=emb_sb[:, cv * H:(cv + 1) * H],
                                start=(cv == 0), stop=(cv == NVC - 1))
                        nc.vector.tensor_copy(ab[:], ps_a[:])
                    else:
                        nc.gpsimd.indirect_dma_start(
                            out=ab[:], out_offset=None, in_=slotbuf[:],
                            in_offset=bass.IndirectOffsetOnAxis(
                                ap=d2s_sb[:, bI:bI + 1], axis=0))
                    ps_t = ps_tp.tile([P, P], FP, tag="tp")
                    nc.tensor.transpose(out=ps_t[:], in_=ab[:],
                                        identity=ident[:])
                    aT = wrk.tile([P, P], FP, tag="aT")
                    nc.scalar.copy(aT[:], ps_t[:])

                    if l == 0:
                        g0 = wrk.tile([P, H], FP, tag="g0")
                        nc.gpsimd.indirect_dma_start(
                            out=g0[:], out_offset=None, in_=emb_d[:],
                            in_offset=bass.IndirectOffsetOnAxis(
                                ap=feat32_sb[:, bI:bI + 1], axis=0))
                        ps_t0 = ps_tp.tile([P, P], FP, tag="tp")
                        nc.tensor.transpose(out=ps_t0[:], in_=g0[:],
                                            identity=ident[:])
                        hT = wrk.tile([P, P], FP, tag="hT")
                        nc.scalar.copy(hT[:], ps_t0[:])
                        rhs_self = hT[:]
                        lhs_s = ws_sb[:, 0:H]
                        lhs_n = wn_sb[:, 0:H]
                        bc = bcol_sb[:, 0:1]
                    else:
                        rhs_self = h_stage[:, bI * P:(bI + 1) * P]
                        lhs_s = wsf_sb[:, l * H:(l + 1) * H]
                        lhs_n = wnf_sb[:, l * H:(l + 1) * H]
                        bc = biasf_sb[:, l:l + 1]

                    ps_r = ps_rst.tile([P, H], FP, tag="rst")
                    nc.tensor.matmul(ps_r[:], lhsT=lhs_s, rhs=rhs_self,
                                     start=True, stop=False)
                    nc.tensor.matmul(ps_r[:], lhsT=lhs_n, rhs=aT[:],
                                     start=False, stop=True)

                    t1 = wrk.tile([P, P], FP, tag="t1")
                    nc.scalar.activation(t1[:], ps_r[:],
                                         mybir.ActivationFunctionType.Relu,
                                         bias=bc)
                    neg = wrk.tile([P, P], FP, tag="neg")
                    nc.vector.tensor_scalar(
                        neg[:], ps_r[:], bc, 0.0,
                        op0=mybir.AluOpType.add, op1=mybir.AluOpType.min)
                    zb = h_stage[:, bI * P:(bI + 1) * P]
                    if nn == P:
                        nc.vector.scalar_tensor_tensor(
                            zb, neg[:], acol_sb[:, l:l + 1], t1[:],
                            op0=mybir.AluOpType.mult, op1=mybir.AluOpType.add,
                            accum_out=stats_sum[:, bI:bI + 1])
                        nc.scalar.activation(scratch[:], zb,
                                             mybir.ActivationFunctionType.Square,
                                             accum_out=stats_sq[:, bI:bI + 1])
                    else:
                        nc.vector.scalar_tensor_tensor(
                            h_stage[:, bI * P:bI * P + nn],
                            neg[:, :nn], acol_sb[:, l:l + 1], t1[:, :nn],
                            op0=mybir.AluOpType.mult, op1=mybir.AluOpType.add,
                            accum_out=stats_sum[:, bI:bI + 1])
                        nc.vector.scalar_tensor_tensor(
                            h_stage[:, bI * P + nn:(bI + 1) * P],
                            neg[:, nn:], acol_sb[:, l:l + 1], t1[:, nn:],
                            op0=mybir.AluOpType.mult, op1=mybir.AluOpType.add)
                        nc.scalar.activation(
                            scratch[:, :nn], h_stage[:, bI * P:bI * P + nn],
                            mybir.ActivationFunctionType.Square,
                            accum_out=stats_sq[:, bI:bI + 1])

                    # fused tail (old pass B): transpose z to node rows,
                    # write the shard, accumulate the raw-z pool
                    ps_t2 = ps_tp.tile([P, P], FP, tag="tp")
                    nc.tensor.transpose(out=ps_t2[:], in_=zb,
                                        identity=ident[:])
                    hnm = wrk.tile([P, P], FP, tag="hnm")
                    nc.scalar.copy(hnm[:], ps_t2[:])
                    if l < L - 1:
                        nc.sync.dma_start(
                            h_shard[bI * P:bI * P + nn, :], hnm[:nn, :])
                    gb = wrk.tile([P, G], FP, tag="gblk")
                    nc.sync.dma_start(gb[:], gind_d[bI])
                    nc.tensor.matmul(ps_p[:G, :], lhsT=gb[:], rhs=hnm[:],
                                     start=(bI == 0), stop=(bI == nblk - 1))
                nc.vector.tensor_copy(pl_all[:G, l * H:(l + 1) * H],
                                      ps_p[:G, :])
                # per-channel z sums for this layer's BN stats
                sx = wrk.tile([P, 1], FP, tag="sx")
                nc.vector.tensor_reduce(sx[:], stats_sum[:],
                                        axis=mybir.AxisListType.X,
                                        op=mybir.AluOpType.add)
                sq = wrk.tile([P, 1], FP, tag="sq")
                nc.vector.tensor_reduce(sq[:], stats_sq[:],
                                        axis=mybir.AxisListType.X,
                                        op=mybir.AluOpType.add)
                nc.sync.dma_start(stats_loc[0:1, :], sx[:, 0:1])
                nc.sync.dma_start(stats_loc[1:2, :], sq[:, 0:1])

            # ---------------- schedule ----------------
            emit_mainpass(0)
            nc.gpsimd.collective_compute(
                "AllGather", mybir.AluOpType.bypass, replica_groups=groups,
                ins=[h_shard[:]], outs=[h_full[:]])
            nc.gpsimd.collective_compute(
                "AllReduce", mybir.AluOpType.add, replica_groups=groups,
                ins=[stats_loc[:]], outs=[stats_red[:]])
            for l in range(1, L):
                for c in range(ncalls):
                    emit_agg_call(c)
                    if c == 1:
                        # overlap the previous layer's stats postprocessing
                        # with this layer's gather stream
                        emit_stats_post(l - 1)
                emit_mainpass(l)
                if l < L - 1:
                    nc.gpsimd.collective_compute(
                        "AllGather", mybir.AluOpType.bypass,
                        replica_groups=groups,
                        ins=[h_shard[:]], outs=[h_full[:]])
                nc.gpsimd.collective_compute(
                    "AllReduce", mybir.AluOpType.add, replica_groups=groups,
                    ins=[stats_loc[:]], outs=[stats_red[:]])
            emit_stats_post(L - 1)

            nc.gpsimd.collective_compute(
                "AllReduce", mybir.AluOpType.add, replica_groups=groups,
                ins=[pool_loc[:]], outs=[pool_red[:]])
            for l in range(L):
                pr = wrk.tile([P, G], FP, tag="pr")
                nc.sync.dma_start(pr[:], pool_red[l])
                ps_o = ps_tp.tile([P, P], FP, tag="tp")
                nc.tensor.transpose(out=ps_o[:], in_=pr[:], identity=ident[:])
                ob = wrk.tile([P, H], FP, tag="ob")
                nc.scalar.copy(ob[:], ps_o[:])
                nc.sync.dma_start(out_d[:, l * H:(l + 1) * H], ob[:G, :])

    nc.compile()
    return nc


# ---------------------------------------------------------------------------
# entry point
# ---------------------------------------------------------------------------

_CACHE = {}


def _run(cfg, inputs, trace=False):
    from concourse.bass_utils import run_bass_kernel_spmd
    in_maps, ncalls, seg_bounds, ncalls0 = _prep(cfg, **inputs)
    key = (cfg["N"], cfg["G"], cfg["H"], ncalls, tuple(seg_bounds))
    if key not in _CACHE:
        _CACHE[key] = build_program(cfg, ncalls, seg_bounds, ncalls0)
    nc = _CACHE[key]
    last_exc = None
    for attempt in range(3):
        try:
            return run_bass_kernel_spmd(nc, in_maps, list(range(N_CORES)),
                                        trace=trace)
        except Exception as e:  # rare transient device-unrecoverable errors
            last_exc = e
            try:
                import jax
                import jax.extend.backend
                jax.clear_caches()
                jax.extend.backend.clear_backends()
            except Exception:
                pass
    raise last_exc


def kernel(in_feat, src, dst, graph_ids, emb, W_self, W_neigh, b,
           gamma, beta, prelu_w):
    cfg = _mkcfg(**CFG_FULL)
    res = _run(cfg, dict(
        in_feat=in_feat, src=src, dst=dst, graph_ids=graph_ids, emb=emb,
        W_self=W_self, W_neigh=W_neigh, b=b, gamma=gamma, beta=beta,
        prelu_w=prelu_w))
    return np.asarray(res.results[0]["out"], np.float32)



# revision 19
# speedup vs baseline: 1.6633x; 1.0580x over previous
"""GCN (4x SAGEConv mean-agg + PReLU + BatchNorm, graph mean-pool) on 8 TRN2 NeuronCores.

Contract: kernel(**inputs) takes FULL inputs (as produced by setup_inputs) and
returns the FULL [G, 4H] output. Self-contained: all shapes/sharding hardcoded.

Sharding: nodes (and their in-edges, i.e. edges bucketed by dst) are
partitioned contiguously across 8 cores. Weights replicated. h is replicated
in HBM per layer via AllGather. BatchNorm stats and the final pooled output
use small AllReduces.

Aggregation: edges sorted by (dst-range, dst) and packed into 128-edge tiles
of <=32 whole dst runs. Per tile one indirect-DMA gather pulls h[src] rows
(128 x 512B descriptors); a (1/deg-weighted) indicator matmul reduces the tile
to its dst slots in PSUM (3 tiles per PSUM tile at base partitions 0/32/64);
all 1024 slots of a 24-tile call are flushed with ONE direct HWDGE DMA into a
slot-space DRAM buffer, and the main pass reads each 128-node block back with
a single 128-row indirect gather through the host-built inverse slot map
(deg-0 nodes point at a zeroed trailing block). Layer 1 needs no gather or
DRAM roundtrip at all: agg0 is a count-matrix matmul against the 257-row
embedding table, and the self term gathers from the tiny table directly.

The device-time floor on this hardware is the Pool engine's SWDGE fixed cost
(~1us per indirect DMA instruction, max 128 descriptors each); the batched
GPSIMD dma_gather/dma_scatter_add ucode that would lift it is not present in
this (bedrock) image.
"""

import numpy as np

import concourse.bass as bass
import concourse.tile as tile
from concourse import bacc, mybir
from concourse.masks import make_identity

FP = mybir.dt.float32
I16 = mybir.dt.int16
I32 = mybir.dt.int32

N_CORES = 8
P = 128          # partitions
J = 32           # dst slots (runs) per edge-tile
TPP = 3          # edge-tiles per PSUM tile (matmul out base partition 0/32/64)
PPC = 8          # PSUM tiles per call
TPC = TPP * PPC  # 24 edge-tiles per gather/scatter call
NIDX = TPC * P   # 3072 gather indices per call
NSLOT = PPC * P  # 1024 scatter slots per call
L = 4
EPS = 1e-5

# SEG = dst rows per agg range (per core, multiple of 128). Each range gets
# its own agg tensor so Tile can overlap main-pass blocks of completed ranges
# with the remaining ranges' gathers/scatters.
CFG_FULL = dict(N=100_000, G=128, H=128, NV=257, SEG=3_200)


def _mkcfg(N, G, H, NV, SEG):
    assert N % N_CORES == 0
    npc = N // N_CORES
    nblk = (npc + P - 1) // P
    last = npc - (nblk - 1) * P
    assert SEG % P == 0
    return dict(
        N=N, G=G, H=H, NV=NV, SEG=SEG, NPC=npc, NBLK=nblk, LAST=last,
        NSEG=(npc + SEG - 1) // SEG,
        NVC=(NV + P - 1) // P,
        AGG_ROWS=nblk * P,
    )


# ---------------------------------------------------------------------------
# host-side preprocessing
# ---------------------------------------------------------------------------

def _pack_tiles(run_len):
    """Pack runs (all of one src-segment, dst-sorted) into tiles of <=128
    edges and <=J runs, runs kept whole."""
    K = len(run_len)
    tile_of_run = np.empty(K, np.int64)
    slot_of_run = np.empty(K, np.int64)
    t = 0
    edges = 0
    runs = 0
    for k in range(K):
        r = int(run_len[k])
        if r > P:
            raise ValueError(f"in-degree run {r} exceeds {P}")
        if edges + r > P or runs >= J:
            t += 1
            edges = 0
            runs = 0
        tile_of_run[k] = t
        slot_of_run[k] = runs
        edges += r
        runs += 1
    return tile_of_run, slot_of_run, t + 1


def _prep_core(cfg, cc, src, dst, in_feat, invdeg):
    npc, nblk = cfg["NPC"], cfg["NBLK"]
    SEG, NSEG = cfg["SEG"], cfg["NSEG"]
    lo = cc * npc
    sel = (dst >= lo) & (dst < lo + npc)
    e_src = src[sel]
    e_dstl = (dst[sel] - lo).astype(np.int64)
    e_seg = e_dstl // SEG
    order = np.lexsort((e_dstl, e_seg))
    e_src = e_src[order].astype(np.int64)
    e_dstl = e_dstl[order]
    e_seg = e_seg[order]
    Ec = len(e_src)

    # runs of equal (seg, dst)
    key = e_seg * npc + e_dstl
    change = np.empty(Ec, bool)
    if Ec:
        change[0] = True
        change[1:] = key[1:] != key[:-1]
    run_starts = np.nonzero(change)[0]
    run_len = np.diff(np.concatenate([run_starts, [Ec]]))
    run_dst = e_dstl[run_starts]
    run_seg = e_seg[run_starts]

    # pack per segment
    seg_tiles = []          # per segment: (tile_of_run idx arrays etc.)
    tiles_per_seg = np.zeros(NSEG, np.int64)
    seg_run_sel = [np.nonzero(run_seg == s)[0] for s in range(NSEG)]
    packs = []
    for s in range(NSEG):
        rl = run_len[seg_run_sel[s]]
        if len(rl):
            tr, sr, T = _pack_tiles(rl)
        else:
            tr = np.empty(0, np.int64)
            sr = np.empty(0, np.int64)
            T = 0
        packs.append((tr, sr))
        tiles_per_seg[s] = T
    return dict(
        Ec=Ec, e_src=e_src, e_dstl=e_dstl, run_starts=run_starts,
        run_len=run_len, run_dst=run_dst, run_seg=run_seg,
        seg_run_sel=seg_run_sel, packs=packs, tiles_per_seg=tiles_per_seg,
        in_feat=in_feat, invdeg=invdeg, lo=lo,
    )


def _finish_core(cfg, core, calls_per_seg):
    npc, nblk = cfg["NPC"], cfg["NBLK"]
    SEG, NSEG, NVC = cfg["SEG"], cfg["NSEG"], cfg["NVC"]
    ncalls = int(calls_per_seg.sum())
    T_total = ncalls * TPC
    SCR = cfg["SEG"]  # scratch row, local to each range's agg tensor

    src_idx = np.zeros((T_total, P), np.int64)   # within-segment row idx
    ind = np.zeros((T_total, P, J), np.float32)
    slotrow = np.full((T_total, J), SCR, np.int64)

    call_base_of_seg = np.concatenate([[0], np.cumsum(calls_per_seg)])[:-1]
    invdeg = core["invdeg"]
    for s in range(NSEG):
        rsel = core["seg_run_sel"][s]
        if not len(rsel):
            continue
        tr, sr = core["packs"][s]
        tile_base = call_base_of_seg[s] * TPC
        run_starts = core["run_starts"][rsel]
        run_len = core["run_len"][rsel]
        run_dst = core["run_dst"][rsel]
        # per-edge position info (vectorized)
        n_e = int(run_len.sum())
        roe = np.repeat(np.arange(len(rsel)), run_len)     # run-of-edge (local)
        t_e = tr[roe] + tile_base
        # first edge index (global, within e_src) of each tile
        e_idx = np.repeat(run_starts, run_len) + (
            np.arange(n_e) - np.repeat(np.cumsum(run_len) - run_len, run_len))
        tfe = np.full(tr.max() + 1 + tile_base, 1 << 60, np.int64)
        np.minimum.at(tfe, t_e, e_idx)
        pos_e = e_idx - tfe[t_e]
        j_e = sr[roe]
        src_idx[t_e, pos_e] = core["e_src"][e_idx]
        dst_e = core["e_dstl"][e_idx]
        ind[t_e, pos_e, j_e] = invdeg[core["lo"] + dst_e]
        slotrow[tr + tile_base, sr] = run_dst - s * SEG

    # ---- device layouts (walrus indirect DMA: one int32 offset column per
    # instruction; gather per tile, scatter per 128-slot PSUM chunk) ----
    src32 = np.ascontiguousarray(src_idx.T.astype(np.int32))  # [128, T_total]
    # scatter chunk (call c, psum q): row p holds slot (tile c*TPC+q*TPP+p//J,
    # run p%J) for p < TPP*J, scratch otherwise
    scat_tok = np.full((ncalls, NSLOT), SCR, np.int64)
    sr_view = slotrow.reshape(ncalls, TPC, J)
    for q in range(PPC):
        blk = sr_view[:, q * TPP:(q + 1) * TPP, :].reshape(ncalls, TPP * J)
        scat_tok[:, q * P:q * P + TPP * J] = blk
    scat32 = np.ascontiguousarray(
        scat_tok.reshape(ncalls * PPC, P).T.astype(np.int32))  # [128, ncalls*PPC]

    ind_dev = np.ascontiguousarray(
        ind.transpose(1, 0, 2).reshape(P, T_total * J))

    # inverse slot map: own node row -> slot-space index (or the zero row)
    seg_of_call = np.repeat(np.arange(NSEG), calls_per_seg)
    ZROW = ncalls * NSLOT
    d2s = np.full(nblk * P, ZROW, np.int64)
    tt, jj = np.nonzero(slotrow != SCR)
    loc = slotrow[tt, jj] + seg_of_call[tt // TPC] * SEG
    q_ = (tt % TPC) // TPP
    p_ = ((tt % TPC) % TPP) * J + jj
    d2s[loc] = (tt // TPC) * NSLOT + q_ * P + p_
    d2s_dev = np.ascontiguousarray(d2s.reshape(nblk, P).T.astype(np.int32))

    # layer-1 count matrix (1/deg folded)
    NV = cfg["NV"]
    v_e = core["in_feat"][core["e_src"]]
    cntm = np.zeros(npc * NVC * P, np.float32)
    np.add.at(cntm, core["e_dstl"] * (NVC * P) + v_e, 1.0)
    cntm = cntm.reshape(npc, NVC * P) * invdeg[core["lo"]:core["lo"] + npc,
                                               None].astype(np.float32)
    cnt_pad = np.zeros((nblk * P, NVC * P), np.float32)
    cnt_pad[:npc] = cntm
    cb = cnt_pad.reshape(nblk, P, NVC, P)
    cnt_dev = np.ascontiguousarray(
        cb.transpose(0, 3, 2, 1).reshape(nblk, P, NVC * P))

    # layer-1 self gather indices (emb rows per own node) [128, nblk]
    feat = np.zeros(nblk * P, np.int64)
    feat[:npc] = core["in_feat"][core["lo"]:core["lo"] + npc]
    feat32 = np.ascontiguousarray(feat.reshape(nblk, P).T.astype(np.int32))

    # earliest call index after which each 128-dst block's d2s gather can run
    call_of = d2s // NSLOT          # ZROW rows map to ncalls (ready at 0)
    blk_ready = np.where(d2s == ZROW, 0, call_of).reshape(nblk, P).max(1)

    return dict(src32=src32, scat32=scat32, ind=ind_dev, cnt=cnt_dev,
                feat32=feat32, d2s=d2s_dev, ncalls0=0, blk_ready=blk_ready)


def _prep(cfg, in_feat, src, dst, graph_ids, emb, W_self, W_neigh, b,
          gamma, beta, prelu_w):
    N, G, H = cfg["N"], cfg["G"], cfg["H"]
    npc, nblk = cfg["NPC"], cfg["NBLK"]
    NV, NVC, NSEG = cfg["NV"], cfg["NVC"], cfg["NSEG"]
    in_feat = np.asarray(in_feat).astype(np.int64)
    src = np.asarray(src).astype(np.int64)
    dst = np.asarray(dst).astype(np.int64)
    graph_ids = np.asarray(graph_ids).astype(np.int64)

    deg = np.bincount(dst, minlength=N)
    invdeg = (1.0 / np.clip(deg, 1, None)).astype(np.float64)

    cores = [_prep_core(cfg, cc, src, dst, in_feat, invdeg)
             for cc in range(N_CORES)]
    calls_per_seg = np.zeros(NSEG, np.int64)
    for s in range(NSEG):
        mx = max(int(c["tiles_per_seg"][s]) for c in cores)
        calls_per_seg[s] = (mx + TPC - 1) // TPC
    ncalls = int(calls_per_seg.sum())

    fins = [_finish_core(cfg, c, calls_per_seg) for c in cores]

    blk_ready = np.maximum.reduce([f["blk_ready"] for f in fins])

    cnt_g = np.clip(np.bincount(graph_ids, minlength=G), 1, None)
    emb_pad = np.zeros((NVC * P, H), np.float32)
    emb_pad[:NV] = np.asarray(emb, np.float32)

    # per-call dst-range index (same on every core)
    seg_bounds = []
    for s in range(NSEG):
        seg_bounds += [s] * int(calls_per_seg[s])

    in_maps = []
    for cc, fin in enumerate(fins):
        lo = cc * npc
        gown = np.zeros(nblk * P, np.int64)
        gown[:npc] = graph_ids[lo:lo + npc]
        gind = np.zeros((nblk * P, G), np.float32)
        gind[np.arange(npc), gown[:npc]] = 1.0 / cnt_g[gown[:npc]]
        gind = np.ascontiguousarray(gind.reshape(nblk, P, G))

        tflag = np.full((P, 1), 1.0 if cc == 0 else 0.0, np.float32)
        in_maps.append(dict(
            src32=fin["src32"], scat32=fin["scat32"], ind=fin["ind"],
            cnt=fin["cnt"], feat32=fin["feat32"], d2s=fin["d2s"],
            gind=gind, emb=emb_pad,
            zeros=np.zeros((cfg["SEG"] + P, H), np.float32),
            W_self=np.ascontiguousarray(np.asarray(W_self, np.float32)),
            W_neigh=np.ascontiguousarray(np.asarray(W_neigh, np.float32)),
            W_sum=np.ascontiguousarray(
                np.asarray(W_self, np.float32) + np.asarray(W_neigh, np.float32)),
            tflag=tflag,
            b_cols=np.ascontiguousarray(np.asarray(b, np.float32).T),
            gam_cols=np.ascontiguousarray(np.asarray(gamma, np.float32).T),
            bet_cols=np.ascontiguousarray(np.asarray(beta, np.float32).T),
            alp_cols=np.ascontiguousarray(np.asarray(prelu_w, np.float32).T),
        ))
    return in_maps, ncalls, seg_bounds, tuple(int(x) for x in blk_ready)


# ---------------------------------------------------------------------------
# device program
# ---------------------------------------------------------------------------

def build_program(cfg, ncalls, seg_bounds, blk_ready, ablate=()):
    """BN-folded pipeline: the exchanged/gathered per-node state is z = the
    pre-BatchNorm PReLU output. h = z*s + t (per-channel affine from batch
    stats) is folded into the next layer's weights on device:
      rst_pre[l+1] = z @ (diag(s_l) W_self) + agg(z) @ (diag(s_l) W_neigh)
                     + (t_l @ (W_self+W_neigh) + b)          [min in-deg >= 1]
    so the BN stats AllReduce and the affine are OFF the critical path
    (computed while the next layer's gather stream runs), and the old pass B
    (normalize + transpose + pool) is fused into pass A. Graph pooling
    accumulates raw z; the affine is applied to the [H, G] pooled tile
    (t added on core 0 only) before the final AllReduce."""
    N, G, H = cfg["N"], cfg["G"], cfg["H"]
    npc, nblk, last = cfg["NPC"], cfg["NBLK"], cfg["LAST"]
    NVC = cfg["NVC"]
    agg_rows = cfg["AGG_ROWS"]
    T_total = ncalls * TPC

    nc = bacc.Bacc("TRN2", target_bir_lowering=False, debug=False,
                   num_devices=N_CORES)

    src32_d = nc.declare_dram_parameter("src32", [P, T_total], I32,
                                        isOutput=False)
    scat32_d = nc.declare_dram_parameter("scat32", [P, ncalls * PPC], I32,
                                         isOutput=False)
    ind_d = nc.declare_dram_parameter("ind", [P, T_total * J], FP, isOutput=False)
    cnt_d = nc.declare_dram_parameter("cnt", [nblk, P, NVC * P], FP, isOutput=False)
    feat32_d = nc.declare_dram_parameter("feat32", [P, nblk], I32,
                                         isOutput=False)
    d2s_d = nc.declare_dram_parameter("d2s", [P, nblk], I32, isOutput=False)
    gind_d = nc.declare_dram_parameter("gind", [nblk, P, G], FP, isOutput=False)
    emb_d = nc.declare_dram_parameter("emb", [NVC * P, H], FP, isOutput=False)
    zeros_d = nc.declare_dram_parameter("zeros", [cfg["SEG"] + P, H], FP,
                                        isOutput=False)
    ws_d = nc.declare_dram_parameter("W_self", [L, H, H], FP, isOutput=False)
    wn_d = nc.declare_dram_parameter("W_neigh", [L, H, H], FP, isOutput=False)
    wsum_d = nc.declare_dram_parameter("W_sum", [L, H, H], FP, isOutput=False)
    tflag_d = nc.declare_dram_parameter("tflag", [P, 1], FP, isOutput=False)
    bcol_d = nc.declare_dram_parameter("b_cols", [H, L], FP, isOutput=False)
    gcol_d = nc.declare_dram_parameter("gam_cols", [H, L], FP, isOutput=False)
    becol_d = nc.declare_dram_parameter("bet_cols", [H, L], FP, isOutput=False)
    acol_d = nc.declare_dram_parameter("alp_cols", [H, L], FP, isOutput=False)
    out_d = nc.declare_dram_parameter("out", [G, L * H], FP, isOutput=True)

    NSEG = cfg["NSEG"]
    SEG = cfg["SEG"]
    h_shard = nc.dram_tensor("h_shard", [npc, H], FP)
    h_full = nc.dram_tensor("h_full", [N, H], FP, addr_space="Shared")
    # slot-space aggregation buffer + one trailing zero block for deg-0 rows
    slotbuf = nc.dram_tensor("slotbuf", [ncalls * NSLOT + P, H], FP)
    stats_loc = nc.dram_tensor("stats_loc", [2, H], FP)
    stats_red = nc.dram_tensor("stats_red", [2, H], FP, addr_space="Shared")
    # pooled z held TRANSPOSED [H, G] so the channel affine uses per-partition
    # scalars; transposed back to [G, H] only at the very end
    pool_loc = nc.dram_tensor("pool_loc", [L, H, G], FP)
    pool_red = nc.dram_tensor("pool_red", [L, H, G], FP, addr_space="Shared")

    groups = [list(range(N_CORES))]

    with tile.TileContext(nc) as tc:
        with (
            tc.tile_pool(name="res", bufs=1) as res,
            tc.tile_pool(name="wrk", bufs=3) as wrk,
            tc.tile_pool(name="gat", bufs=3) as gat,
            tc.tile_pool(name="ps_slot", bufs=3, space="PSUM") as ps_slot,
            tc.tile_pool(name="ps_tp", bufs=2, space="PSUM") as ps_tp,
            tc.tile_pool(name="ps_rst", bufs=2, space="PSUM") as ps_rst,
            tc.tile_pool(name="ps_pool", bufs=1, space="PSUM") as ps_pool,
        ):
            ident = res.tile([P, P], FP, tag="ident")
            make_identity(nc, ident[:])

            src32_sb = res.tile([P, T_total], I32, tag="src32")
            nc.sync.dma_start(src32_sb[:], src32_d[:])
            scat32_sb = res.tile([P, ncalls * PPC], I32, tag="scat32")
            nc.sync.dma_start(scat32_sb[:], scat32_d[:])
            feat32_sb = res.tile([P, nblk], I32, tag="feat32")
            nc.sync.dma_start(feat32_sb[:], feat32_d[:])
            d2s_sb = res.tile([P, nblk], I32, tag="d2s")
            nc.sync.dma_start(d2s_sb[:], d2s_d[:])
            emb_sb = res.tile([P, NVC * H], FP, tag="emb")
            for c in range(NVC):
                nc.sync.dma_start(emb_sb[:, c * H:(c + 1) * H],
                                  emb_d[c * P:(c + 1) * P, :])
            ws_sb = res.tile([P, L * H], FP, tag="ws")
            wn_sb = res.tile([P, L * H], FP, tag="wn")
            wsum_sb = res.tile([P, L * H], FP, tag="wsum")
            for l in range(L):
                nc.sync.dma_start(ws_sb[:, l * H:(l + 1) * H], ws_d[l])
                nc.sync.dma_start(wn_sb[:, l * H:(l + 1) * H], wn_d[l])
                nc.sync.dma_start(wsum_sb[:, l * H:(l + 1) * H], wsum_d[l])
            tflag_sb = res.tile([P, 1], FP, tag="tflag")
            nc.sync.dma_start(tflag_sb[:], tflag_d[:])
            bcol_sb = res.tile([P, L], FP, tag="bcol")
            nc.sync.dma_start(bcol_sb[:], bcol_d[:])
            gcol_sb = res.tile([P, L], FP, tag="gcol")
            nc.sync.dma_start(gcol_sb[:], gcol_d[:])
            becol_sb = res.tile([P, L], FP, tag="becol")
            nc.sync.dma_start(becol_sb[:], becol_d[:])
            acol_sb = res.tile([P, L], FP, tag="acol")
            nc.sync.dma_start(acol_sb[:], acol_d[:])

            h_stage = res.tile([P, nblk * P], FP, tag="hstage")
            stats_sum = res.tile([P, nblk], FP, tag="ssum")
            stats_sq = res.tile([P, nblk], FP, tag="ssq")
            scratch = res.tile([P, P], FP, tag="scratch")
            eps_col = res.tile([P, 1], FP, tag="eps")
            nc.vector.memset(eps_col[:], float(EPS))
            # per-layer BN affine (s, t), folded weights and bias columns
            s_all = res.tile([P, L], FP, tag="sall")
            t_all = res.tile([P, L], FP, tag="tall")
            wsf_sb = res.tile([P, L * H], FP, tag="wsf")
            wnf_sb = res.tile([P, L * H], FP, tag="wnf")
            biasf_sb = res.tile([P, L], FP, tag="biasf")
            pl_all = res.tile([P, L * H], FP, tag="plall")

            # S staging: partitions [TPP*J:128) of each chunk feed zeros
            S_bufs = []
            for i in range(4):
                Sb = res.tile([P, PPC * H], FP, tag=f"S{i}")
                nc.vector.memset(Sb[TPP * J:, :], 0.0)
                S_bufs.append(Sb)

            # zero the trailing slotbuf block once (deg-0 rows point here)
            nc.sync.dma_start(slotbuf[ncalls * NSLOT:, :], zeros_d[:P, :])

            def emit_agg_call(c):
                gt = gat.tile([P, TPC * H], FP, tag="g")
                if "gather" not in ablate:
                    for ti in range(TPC):
                        t_glob = c * TPC + ti
                        nc.gpsimd.indirect_dma_start(
                            out=gt[:, ti * H:(ti + 1) * H],
                            out_offset=None, in_=h_full[:],
                            in_offset=bass.IndirectOffsetOnAxis(
                                ap=src32_sb[:, t_glob:t_glob + 1],
                                axis=0))
                it = wrk.tile([P, TPC * J], FP, tag="indblk")
                nc.sync.dma_start(
                    it[:], ind_d[:, c * TPC * J:(c + 1) * TPC * J])
                S = S_bufs[c % 4]
                for q in range(PPC):
                    ps = ps_slot.tile([P, H], FP, tag="slot")
                    if "aggmm" not in ablate:
                        for ti in range(TPP):
                            t_loc = q * TPP + ti
                            nc.tensor.matmul(
                                ps[ti * J:(ti + 1) * J, :],
                                lhsT=it[:, t_loc * J:(t_loc + 1) * J],
                                rhs=gt[:, t_loc * H:(t_loc + 1) * H],
                                start=True, stop=True)
                        nc.vector.tensor_copy(
                            S[:TPP * J, q * H:(q + 1) * H],
                            ps[:TPP * J, :])
                if "scatter" not in ablate:
                    # one direct HWDGE DMA flushes all 1024 slots
                    nc.sync.dma_start(
                        slotbuf[c * NSLOT:(c + 1) * NSLOT, :]
                        .rearrange("(q p) f -> p q f", p=P),
                        S[:].rearrange("p (q f) -> p q f", f=H))

            def emit_stats_post(j):
                """s_j, t_j from the (already AllReduced) stats of z^j; fold
                layer j+1's weights/bias; apply the pool affine for layer j."""
                sxr = wrk.tile([P, 1], FP, tag="sxr")
                nc.sync.dma_start(sxr[:, 0:1], stats_red[0:1, :])
                sqr = wrk.tile([P, 1], FP, tag="sqr")
                nc.sync.dma_start(sqr[:, 0:1], stats_red[1:2, :])
                mu = wrk.tile([P, 1], FP, tag="mu")
                nc.scalar.mul(mu[:], sxr[:], 1.0 / N)
                ex2 = wrk.tile([P, 1], FP, tag="ex2")
                nc.scalar.mul(ex2[:], sqr[:], 1.0 / N)
                mu2 = wrk.tile([P, 1], FP, tag="mu2")
                nc.scalar.square(mu2[:], mu[:])
                var = wrk.tile([P, 1], FP, tag="var")
                nc.vector.tensor_sub(var[:], ex2[:], mu2[:])
                sd = wrk.tile([P, 1], FP, tag="sd")
                nc.scalar.activation(sd[:], var[:],
                                     mybir.ActivationFunctionType.Sqrt,
                                     bias=eps_col[:])
                rstd = wrk.tile([P, 1], FP, tag="rstd")
                nc.vector.reciprocal(rstd[:], sd[:])
                s_col = s_all[:, j:j + 1]
                t_col = t_all[:, j:j + 1]
                nc.vector.tensor_mul(s_col, rstd[:], gcol_sb[:, j:j + 1])
                msc = wrk.tile([P, 1], FP, tag="msc")
                nc.vector.tensor_mul(msc[:], mu[:], s_col)
                nc.vector.tensor_sub(t_col, becol_sb[:, j:j + 1], msc[:])
                if j < L - 1:
                    ln = j + 1
                    nc.vector.tensor_scalar_mul(
                        wsf_sb[:, ln * H:(ln + 1) * H],
                        ws_sb[:, ln * H:(ln + 1) * H], s_col)
                    nc.vector.tensor_scalar_mul(
                        wnf_sb[:, ln * H:(ln + 1) * H],
                        wn_sb[:, ln * H:(ln + 1) * H], s_col)
                    ps_b = ps_rst.tile([P, H], FP, tag="rst")
                    nc.tensor.matmul(ps_b[:, 0:1],
                                     lhsT=wsum_sb[:, ln * H:(ln + 1) * H],
                                     rhs=t_col, start=True, stop=True)
                    nc.vector.tensor_add(biasf_sb[:, ln:ln + 1], ps_b[:, 0:1],
                                         bcol_sb[:, ln:ln + 1])
                # pool affine for layer j: [H, G] = s*poolT + t (core 0 only)
                ps_t = ps_tp.tile([P, P], FP, tag="tp")
                nc.tensor.transpose(out=ps_t[:],
                                    in_=pl_all[:, j * H:(j + 1) * H],
                                    identity=ident[:])
                poolT = wrk.tile([P, P], FP, tag="poolT")
                nc.scalar.copy(poolT[:], ps_t[:])
                tf = wrk.tile([P, 1], FP, tag="tf")
                nc.vector.tensor_mul(tf[:], t_col, tflag_sb[:])
                pla = wrk.tile([P, G], FP, tag="pla")
                nc.vector.scalar_tensor_tensor(
                    pla[:], poolT[:, :G], s_col, tf[:].to_broadcast([P, G]),
                    op0=mybir.AluOpType.mult, op1=mybir.AluOpType.add)
                nc.sync.dma_start(pool_loc[j], pla[:])

            def emit_block(l, bI, ps_p):
                    nn = last if bI == nblk - 1 else P
                    ab = wrk.tile([P, H], FP, tag="mablk")
                    if l == 0:
                        cnt_sb = wrk.tile([P, NVC * H], FP, tag="cntblk")
                        nc.sync.dma_start(cnt_sb[:], cnt_d[bI])
                        ps_a = ps_rst.tile([P, H], FP, tag="rst")
                        for cv in range(NVC):
                            nc.tensor.matmul(
                                ps_a[:],
                                lhsT=cnt_sb[:, cv * H:(cv + 1) * H],
                                rhs=emb_sb[:, cv * H:(cv + 1) * H],
                                start=(cv == 0), stop=(cv == NVC - 1))
                        nc.vector.tensor_copy(ab[:], ps_a[:])
                    else:
                        nc.gpsimd.indirect_dma_start(
                            out=ab[:], out_offset=None, in_=slotbuf[:],
                            in_offset=bass.IndirectOffsetOnAxis(
                                ap=d2s_sb[:, bI:bI + 1], axis=0))
                    ps_t = ps_tp.tile([P, P], FP, tag="tp")
                    nc.tensor.transpose(out=ps_t[:], in_=ab[:],
                                        identity=ident[:])
                    aT = wrk.tile([P, P], FP, tag="aT")
                    nc.scalar.copy(aT[:], ps_t[:])

                    if l == 0:
                        g0 = wrk.tile([P, H], FP, tag="g0")
                        nc.gpsimd.indirect_dma_start(
                            out=g0[:], out_offset=None, in_=emb_d[:],
                            in_offset=bass.IndirectOffsetOnAxis(
                                ap=feat32_sb[:, bI:bI + 1], axis=0))
                        ps_t0 = ps_tp.tile([P, P], FP, tag="tp")
                        nc.tensor.transpose(out=ps_t0[:], in_=g0[:],
                                            identity=ident[:])
                        hT = wrk.tile([P, P], FP, tag="hT")
                        nc.scalar.copy(hT[:], ps_t0[:])
                        rhs_self = hT[:]
                        lhs_s = ws_sb[:, 0:H]
                        lhs_n = wn_sb[:, 0:H]
                        bc = bcol_sb[:, 0:1]
                    else:
                        rhs_self = h_stage[:, bI * P:(bI + 1) * P]
                        lhs_s = wsf_sb[:, l * H:(l + 1) * H]
                        lhs_n = wnf_sb[:, l * H:(l + 1) * H]
                        bc = biasf_sb[:, l:l + 1]

                    ps_r = ps_rst.tile([P, H], FP, tag="rst")
                    nc.tensor.matmul(ps_r[:], lhsT=lhs_s, rhs=rhs_self,
                                     start=True, stop=False)
                    nc.tensor.matmul(ps_r[:], lhsT=lhs_n, rhs=aT[:],
                                     start=False, stop=True)

                    t1 = wrk.tile([P, P], FP, tag="t1")
                    nc.scalar.activation(t1[:], ps_r[:],
                                         mybir.ActivationFunctionType.Relu,
                                         bias=bc)
                    neg = wrk.tile([P, P], FP, tag="neg")
                    nc.vector.tensor_scalar(
                        neg[:], ps_r[:], bc, 0.0,
                        op0=mybir.AluOpType.add, op1=mybir.AluOpType.min)
                    zb = h_stage[:, bI * P:(bI + 1) * P]
                    if nn == P:
                        nc.vector.scalar_tensor_tensor(
                            zb, neg[:], acol_sb[:, l:l + 1], t1[:],
                            op0=mybir.AluOpType.mult, op1=mybir.AluOpType.add,
                            accum_out=stats_sum[:, bI:bI + 1])
                        nc.scalar.activation(scratch[:], zb,
                                             mybir.ActivationFunctionType.Square,
                                             accum_out=stats_sq[:, bI:bI + 1])
                    else:
                        nc.vector.scalar_tensor_tensor(
                            h_stage[:, bI * P:bI * P + nn],
                            neg[:, :nn], acol_sb[:, l:l + 1], t1[:, :nn],
                            op0=mybir.AluOpType.mult, op1=mybir.AluOpType.add,
                            accum_out=stats_sum[:, bI:bI + 1])
                        nc.vector.scalar_tensor_tensor(
                            h_stage[:, bI * P + nn:(bI + 1) * P],
                            neg[:, nn:], acol_sb[:, l:l + 1], t1[:, nn:],
                            op0=mybir.AluOpType.mult, op1=mybir.AluOpType.add)
                        nc.scalar.activation(
                            scratch[:, :nn], h_stage[:, bI * P:bI * P + nn],
                            mybir.ActivationFunctionType.Square,
                            accum_out=stats_sq[:, bI:bI + 1])

                    # fused tail (old pass B): transpose z to node rows,
                    # write the shard, accumulate the raw-z pool
                    ps_t2 = ps_tp.tile([P, P], FP, tag="tp")
                    nc.tensor.transpose(out=ps_t2[:], in_=zb,
                                        identity=ident[:])
                    hnm = wrk.tile([P, P], FP, tag="hnm")
                    nc.scalar.copy(hnm[:], ps_t2[:])
                    if l < L - 1:
                        nc.sync.dma_start(
                            h_shard[bI * P:bI * P + nn, :], hnm[:nn, :])
                    gb = wrk.tile([P, G], FP, tag="gblk")
                    nc.sync.dma_start(gb[:], gind_d[bI])
                    nc.tensor.matmul(ps_p[:G, :], lhsT=gb[:], rhs=hnm[:],
                                     start=(bI == 0), stop=(bI == nblk - 1))

            def emit_layer_tail(l, ps_p):
                nc.vector.tensor_copy(pl_all[:G, l * H:(l + 1) * H],
                                      ps_p[:G, :])
                # per-channel z sums for this layer's BN stats
                sx = wrk.tile([P, 1], FP, tag="sx")
                nc.vector.tensor_reduce(sx[:], stats_sum[:],
                                        axis=mybir.AxisListType.X,
                                        op=mybir.AluOpType.add)
                sq = wrk.tile([P, 1], FP, tag="sq")
                nc.vector.tensor_reduce(sq[:], stats_sq[:],
                                        axis=mybir.AxisListType.X,
                                        op=mybir.AluOpType.add)
                nc.sync.dma_start(stats_loc[0:1, :], sx[:, 0:1])
                nc.sync.dma_start(stats_loc[1:2, :], sq[:, 0:1])

            # ---------------- schedule ----------------
            ps_p = ps_pool.tile([P, H], FP, tag="pool")
            for bI in range(nblk):
                emit_block(0, bI, ps_p)
            emit_layer_tail(0, ps_p)
            nc.gpsimd.collective_compute(
                "AllGather", mybir.AluOpType.bypass, replica_groups=groups,
                ins=[h_shard[:]], outs=[h_full[:]])
            nc.gpsimd.collective_compute(
                "AllReduce", mybir.AluOpType.add, replica_groups=groups,
                ins=[stats_loc[:]], outs=[stats_red[:]])
            for l in range(1, L):
                ps_p = ps_pool.tile([P, H], FP, tag="pool")
                nxt = 0
                for c in range(ncalls):
                    emit_agg_call(c)
                    if c == 1:
                        # overlap the previous layer's stats postprocessing
                        # with this layer's gather stream
                        emit_stats_post(l - 1)
                    # interleave ready blocks' pass A into the gather stream.
                    # Tile does NOT track the indirect d2s gather's read of
                    # slotbuf, so correctness is by queue order: the d2s for a
                    # block issues >= LAG calls (~30us of pool work each) of
                    # gather stream after the flush covering its slots was
                    # issued, far exceeding the flush's ~10us completion.
                    while nxt < nblk and blk_ready[nxt] + 4 <= c:
                        emit_block(l, nxt, ps_p)
                        nxt += 1
                while nxt < nblk:
                    emit_block(l, nxt, ps_p)
                    nxt += 1
                emit_layer_tail(l, ps_p)
                if l < L - 1:
                    nc.gpsimd.collective_compute(
                        "AllGather", mybir.AluOpType.bypass,
                        replica_groups=groups,
                        ins=[h_shard[:]], outs=[h_full[:]])
                nc.gpsimd.collective_compute(
                    "AllReduce", mybir.AluOpType.add, replica_groups=groups,
                    ins=[stats_loc[:]], outs=[stats_red[:]])
            emit_stats_post(L - 1)

            nc.gpsimd.collective_compute(
                "AllReduce", mybir.AluOpType.add, replica_groups=groups,
                ins=[pool_loc[:]], outs=[pool_red[:]])
            for l in range(L):
                pr = wrk.tile([P, G], FP, tag="pr")
                nc.sync.dma_start(pr[:], pool_red[l])
                ps_o = ps_tp.tile([P, P], FP, tag="tp")
                nc.tensor.transpose(out=ps_o[:], in_=pr[:], identity=ident[:])
                ob = wrk.tile([P, H], FP, tag="ob")
                nc.scalar.copy(ob[:], ps_o[:])
                nc.sync.dma_start(out_d[:, l * H:(l + 1) * H], ob[:G, :])

    nc.compile()
    return nc


# ---------------------------------------------------------------------------
# entry point
# ---------------------------------------------------------------------------

_CACHE = {}


def _run(cfg, inputs, trace=False):
    from concourse.bass_utils import run_bass_kernel_spmd
    in_maps, ncalls, seg_bounds, blk_ready = _prep(cfg, **inputs)
    key = (cfg["N"], cfg["G"], cfg["H"], ncalls, tuple(seg_bounds), blk_ready)
    if key not in _CACHE:
        _CACHE[key] = build_program(cfg, ncalls, seg_bounds, blk_ready)
    nc = _CACHE[key]
    last_exc = None
    for attempt in range(3):
        try:
            return run_bass_kernel_spmd(nc, in_maps, list(range(N_CORES)),
                                        trace=trace)
        except Exception as e:  # rare transient device-unrecoverable errors
            last_exc = e
            try:
                import jax
                import jax.extend.backend
                jax.clear_caches()
                jax.extend.backend.clear_backends()
            except Exception:
                pass
    raise last_exc


def kernel(in_feat, src, dst, graph_ids, emb, W_self, W_neigh, b,
           gamma, beta, prelu_w):
    cfg = _mkcfg(**CFG_FULL)
    res = _run(cfg, dict(
        in_feat=in_feat, src=src, dst=dst, graph_ids=graph_ids, emb=emb,
        W_self=W_self, W_neigh=W_neigh, b=b, gamma=gamma, beta=beta,
        prelu_w=prelu_w))
    return np.asarray(res.results[0]["out"], np.float32)



# revision 29
# speedup vs baseline: 1.7448x; 1.0490x over previous
"""GCN (4x SAGEConv mean-agg + PReLU + BatchNorm, graph mean-pool) on 8 TRN2 NeuronCores.

Contract: kernel(**inputs) takes FULL inputs (as produced by setup_inputs) and
returns the FULL [G, 4H] output. Self-contained: all shapes/sharding hardcoded.

Sharding: nodes (and their in-edges, i.e. edges bucketed by dst) are
partitioned contiguously across 8 cores. Weights replicated. h is replicated
in HBM per layer via AllGather. BatchNorm stats and the final pooled output
use small AllReduces.

Aggregation: edges sorted by (dst-range, dst) and packed into 128-edge tiles
of <=32 whole dst runs. Per tile one indirect-DMA gather pulls h[src] rows
(128 x 512B descriptors); a (1/deg-weighted) indicator matmul reduces the tile
to its dst slots in PSUM (3 tiles per PSUM tile at base partitions 0/32/64);
all 1024 slots of a 24-tile call are flushed with ONE direct HWDGE DMA into a
slot-space DRAM buffer, and the main pass reads each 128-node block back with
a single 128-row indirect gather through the host-built inverse slot map
(deg-0 nodes point at a zeroed trailing block). Layer 1 needs no gather or
DRAM roundtrip at all: agg0 is a count-matrix matmul against the 257-row
embedding table, and the self term gathers from the tiny table directly.

The device-time floor on this hardware is the Pool engine's SWDGE fixed cost
(~1us per indirect DMA instruction, max 128 descriptors each); the batched
GPSIMD dma_gather/dma_scatter_add ucode that would lift it is not present in
this (bedrock) image.
"""

import numpy as np

import concourse.bass as bass
import concourse.tile as tile
from concourse import bacc, mybir
from concourse.masks import make_identity

FP = mybir.dt.float32
I16 = mybir.dt.int16
I32 = mybir.dt.int32

N_CORES = 8
P = 128          # partitions
J = 32           # dst slots (runs) per edge-tile
TPP = 3          # edge-tiles per PSUM tile (matmul out base partition 0/32/64)
PPC = 8          # PSUM tiles per call
TPC = TPP * PPC  # 24 edge-tiles per gather/scatter call
NIDX = TPC * P   # 3072 gather indices per call
NSLOT = PPC * J  # 256 slot rows per call (32 shared per PSUM group)
L = 4
EPS = 1e-5

# SEG = dst rows per agg range (per core, multiple of 128). Each range gets
# its own agg tensor so Tile can overlap main-pass blocks of completed ranges
# with the remaining ranges' gathers/scatters.
CFG_FULL = dict(N=100_000, G=128, H=128, NV=257, SEG=3_200)


def _mkcfg(N, G, H, NV, SEG):
    assert N % N_CORES == 0
    npc = N // N_CORES
    nblk = (npc + P - 1) // P
    last = npc - (nblk - 1) * P
    assert SEG % P == 0
    return dict(
        N=N, G=G, H=H, NV=NV, SEG=SEG, NPC=npc, NBLK=nblk, LAST=last,
        NSEG=(npc + SEG - 1) // SEG,
        NVC=(NV + P - 1) // P,
        AGG_ROWS=nblk * P,
    )


# ---------------------------------------------------------------------------
# host-side preprocessing
# ---------------------------------------------------------------------------

def _prep_core(cfg, cc, src, dst, in_feat, invdeg):
    """Sort the core's in-edges by dst and greedily pack dst runs into
    384-edge PSUM groups (3 gather tiles sharing 32 slot rows, accumulated
    into one PSUM tile). Runs may split across a group's 3 tiles; a group
    closes when 384 edges or 32 runs are reached."""
    npc = cfg["NPC"]
    lo = cc * npc
    sel = (dst >= lo) & (dst < lo + npc)
    e_src = src[sel].astype(np.int64)
    e_dstl = (dst[sel] - lo).astype(np.int64)
    order = np.argsort(e_dstl, kind="stable")
    e_src = e_src[order]
    e_dstl = e_dstl[order]
    Ec = len(e_src)

    change = np.empty(Ec, bool)
    if Ec:
        change[0] = True
        change[1:] = e_dstl[1:] != e_dstl[:-1]
    run_starts = np.nonzero(change)[0]
    run_len = np.diff(np.concatenate([run_starts, [Ec]]))
    run_dst = e_dstl[run_starts]

    GEDGE = TPP * P  # 384 edges per group
    K = len(run_len)
    grp_of_run = np.empty(K, np.int64)
    slot_of_run = np.empty(K, np.int64)
    pos_of_run = np.empty(K, np.int64)   # first edge position within group
    g = 0
    fill = 0
    nruns = 0
    for k in range(K):
        r = int(run_len[k])
        if r > GEDGE:
            raise ValueError(f"in-degree run {r} exceeds {GEDGE}")
        if fill + r > GEDGE or nruns >= J:
            g += 1
            fill = 0
            nruns = 0
        grp_of_run[k] = g
        slot_of_run[k] = nruns
        pos_of_run[k] = fill
        fill += r
        nruns += 1
    return dict(
        Ec=Ec, e_src=e_src, e_dstl=e_dstl, run_starts=run_starts,
        run_len=run_len, run_dst=run_dst, grp_of_run=grp_of_run,
        slot_of_run=slot_of_run, pos_of_run=pos_of_run, n_groups=g + 1,
        in_feat=in_feat, invdeg=invdeg, lo=lo,
    )


def _finish_core(cfg, core, ncalls):
    npc, nblk = cfg["NPC"], cfg["NBLK"]
    NVC = cfg["NVC"]
    T_total = ncalls * TPC
    invdeg = core["invdeg"]

    src_idx = np.zeros((T_total, P), np.int64)
    ind = np.zeros((T_total, P, J), np.float32)

    run_len = core["run_len"]
    K = len(run_len)
    n_e = int(run_len.sum())
    roe = np.repeat(np.arange(K), run_len)            # run of edge
    off = np.arange(n_e) - np.repeat(core["run_starts"], run_len)
    gpos = core["pos_of_run"][roe] + off              # position in group
    grp = core["grp_of_run"][roe]
    t_e = grp * TPP + gpos // P
    pos_e = gpos % P
    src_idx[t_e, pos_e] = core["e_src"]
    ind[t_e, pos_e, core["slot_of_run"][roe]] = \
        invdeg[core["lo"] + core["e_dstl"]]

    src32 = np.ascontiguousarray(src_idx.T.astype(np.int32))  # [128, T_total]
    ind_dev = np.ascontiguousarray(
        ind.transpose(1, 0, 2).reshape(P, T_total * J))

    # inverse slot map: own node row -> slot-space index (or the zero row)
    ZROW = ncalls * NSLOT
    d2s = np.full(nblk * P, ZROW, np.int64)
    d2s[core["run_dst"]] = (
        (core["grp_of_run"] // PPC) * NSLOT
        + (core["grp_of_run"] % PPC) * J + core["slot_of_run"])
    d2s_dev = np.ascontiguousarray(d2s.reshape(nblk, P).T.astype(np.int32))

    # layer-1 count matrix (1/deg folded)
    NV = cfg["NV"]
    v_e = core["in_feat"][core["e_src"]]
    cntm = np.zeros(npc * NVC * P, np.float32)
    np.add.at(cntm, core["e_dstl"] * (NVC * P) + v_e, 1.0)
    cntm = cntm.reshape(npc, NVC * P) * invdeg[core["lo"]:core["lo"] + npc,
                                               None].astype(np.float32)
    cnt_pad = np.zeros((nblk * P, NVC * P), np.float32)
    cnt_pad[:npc] = cntm
    cb = cnt_pad.reshape(nblk, P, NVC, P)
    cnt_dev = np.ascontiguousarray(
        cb.transpose(0, 3, 2, 1).reshape(nblk, P, NVC * P))

    # layer-1 self gather indices (emb rows per own node) [128, nblk]
    feat = np.zeros(nblk * P, np.int64)
    feat[:npc] = core["in_feat"][core["lo"]:core["lo"] + npc]
    feat32 = np.ascontiguousarray(feat.reshape(nblk, P).T.astype(np.int32))

    # earliest call index after which each 128-dst block's d2s gather can run
    call_of = d2s // NSLOT          # ZROW rows map to ncalls (ready at 0)
    blk_ready = np.where(d2s == ZROW, 0, call_of).reshape(nblk, P).max(1)

    return dict(src32=src32, ind=ind_dev, cnt=cnt_dev,
                feat32=feat32, d2s=d2s_dev, blk_ready=blk_ready)


def _prep(cfg, in_feat, src, dst, graph_ids, emb, W_self, W_neigh, b,
          gamma, beta, prelu_w):
    N, G, H = cfg["N"], cfg["G"], cfg["H"]
    npc, nblk = cfg["NPC"], cfg["NBLK"]
    NV, NVC, NSEG = cfg["NV"], cfg["NVC"], cfg["NSEG"]
    in_feat = np.asarray(in_feat).astype(np.int64)
    src = np.asarray(src).astype(np.int64)
    dst = np.asarray(dst).astype(np.int64)
    graph_ids = np.asarray(graph_ids).astype(np.int64)

    deg = np.bincount(dst, minlength=N)
    invdeg = (1.0 / np.clip(deg, 1, None)).astype(np.float64)

    cores = [_prep_core(cfg, cc, src, dst, in_feat, invdeg)
             for cc in range(N_CORES)]
    ncalls = (max(c["n_groups"] for c in cores) + PPC - 1) // PPC

    fins = [_finish_core(cfg, c, ncalls) for c in cores]

    blk_ready = np.maximum.reduce([f["blk_ready"] for f in fins])

    cnt_g = np.clip(np.bincount(graph_ids, minlength=G), 1, None)
    emb_pad = np.zeros((NVC * P, H), np.float32)
    emb_pad[:NV] = np.asarray(emb, np.float32)

    seg_bounds = []
    in_maps = []
    for cc, fin in enumerate(fins):
        lo = cc * npc
        gown = np.zeros(nblk * P, np.int64)
        gown[:npc] = graph_ids[lo:lo + npc]
        gind = np.zeros((nblk * P, G), np.float32)
        gind[np.arange(npc), gown[:npc]] = 1.0 / cnt_g[gown[:npc]]
        gind = np.ascontiguousarray(gind.reshape(nblk, P, G))

        tflag = np.full((P, 1), 1.0 if cc == 0 else 0.0, np.float32)
        in_maps.append(dict(
            src32=fin["src32"], ind=fin["ind"],
            cnt=fin["cnt"], feat32=fin["feat32"], d2s=fin["d2s"],
            gind=gind, emb=emb_pad,
            zeros=np.zeros((cfg["SEG"] + P, H), np.float32),
            W_self=np.ascontiguousarray(np.asarray(W_self, np.float32)),
            W_neigh=np.ascontiguousarray(np.asarray(W_neigh, np.float32)),
            W_sum=np.ascontiguousarray(
                np.asarray(W_self, np.float32) + np.asarray(W_neigh, np.float32)),
            tflag=tflag,
            b_cols=np.ascontiguousarray(np.asarray(b, np.float32).T),
            gam_cols=np.ascontiguousarray(np.asarray(gamma, np.float32).T),
            bet_cols=np.ascontiguousarray(np.asarray(beta, np.float32).T),
            alp_cols=np.ascontiguousarray(np.asarray(prelu_w, np.float32).T),
        ))
    return in_maps, ncalls, seg_bounds, tuple(int(x) for x in blk_ready)


# ---------------------------------------------------------------------------
# device program
# ---------------------------------------------------------------------------

def build_program(cfg, ncalls, seg_bounds, blk_ready, ablate=()):
    """BN-folded pipeline: the exchanged/gathered per-node state is z = the
    pre-BatchNorm PReLU output. h = z*s + t (per-channel affine from batch
    stats) is folded into the next layer's weights on device:
      rst_pre[l+1] = z @ (diag(s_l) W_self) + agg(z) @ (diag(s_l) W_neigh)
                     + (t_l @ (W_self+W_neigh) + b)          [min in-deg >= 1]
    so the BN stats AllReduce and the affine are OFF the critical path
    (computed while the next layer's gather stream runs), and the old pass B
    (normalize + transpose + pool) is fused into pass A. Graph pooling
    accumulates raw z; the affine is applied to the [H, G] pooled tile
    (t added on core 0 only) before the final AllReduce."""
    N, G, H = cfg["N"], cfg["G"], cfg["H"]
    npc, nblk, last = cfg["NPC"], cfg["NBLK"], cfg["LAST"]
    NVC = cfg["NVC"]
    agg_rows = cfg["AGG_ROWS"]
    T_total = ncalls * TPC

    nc = bacc.Bacc("TRN2", target_bir_lowering=False, debug=False,
                   num_devices=N_CORES)

    src32_d = nc.declare_dram_parameter("src32", [P, T_total], I32,
                                        isOutput=False)
    ind_d = nc.declare_dram_parameter("ind", [P, T_total * J], FP, isOutput=False)
    cnt_d = nc.declare_dram_parameter("cnt", [nblk, P, NVC * P], FP, isOutput=False)
    feat32_d = nc.declare_dram_parameter("feat32", [P, nblk], I32,
                                         isOutput=False)
    d2s_d = nc.declare_dram_parameter("d2s", [P, nblk], I32, isOutput=False)
    gind_d = nc.declare_dram_parameter("gind", [nblk, P, G], FP, isOutput=False)
    emb_d = nc.declare_dram_parameter("emb", [NVC * P, H], FP, isOutput=False)
    zeros_d = nc.declare_dram_parameter("zeros", [cfg["SEG"] + P, H], FP,
                                        isOutput=False)
    ws_d = nc.declare_dram_parameter("W_self", [L, H, H], FP, isOutput=False)
    wn_d = nc.declare_dram_parameter("W_neigh", [L, H, H], FP, isOutput=False)
    wsum_d = nc.declare_dram_parameter("W_sum", [L, H, H], FP, isOutput=False)
    tflag_d = nc.declare_dram_parameter("tflag", [P, 1], FP, isOutput=False)
    bcol_d = nc.declare_dram_parameter("b_cols", [H, L], FP, isOutput=False)
    gcol_d = nc.declare_dram_parameter("gam_cols", [H, L], FP, isOutput=False)
    becol_d = nc.declare_dram_parameter("bet_cols", [H, L], FP, isOutput=False)
    acol_d = nc.declare_dram_parameter("alp_cols", [H, L], FP, isOutput=False)
    out_d = nc.declare_dram_parameter("out", [G, L * H], FP, isOutput=True)

    NSEG = cfg["NSEG"]
    SEG = cfg["SEG"]
    h_shard = nc.dram_tensor("h_shard", [npc, H], FP)
    h_full = nc.dram_tensor("h_full", [N, H], FP, addr_space="Shared")
    # slot-space aggregation buffer + one trailing zero block for deg-0 rows
    slotbuf = nc.dram_tensor("slotbuf", [ncalls * NSLOT + P, H], FP)
    stats_loc = nc.dram_tensor("stats_loc", [2, H], FP)
    stats_red = nc.dram_tensor("stats_red", [2, H], FP, addr_space="Shared")
    # pooled z held TRANSPOSED [H, G] so the channel affine uses per-partition
    # scalars; transposed back to [G, H] only at the very end
    pool_loc = nc.dram_tensor("pool_loc", [L, H, G], FP)
    pool_red = nc.dram_tensor("pool_red", [L, H, G], FP, addr_space="Shared")

    groups = [list(range(N_CORES))]

    with tile.TileContext(nc) as tc:
        with (
            tc.tile_pool(name="res", bufs=1) as res,
            tc.tile_pool(name="wrk", bufs=3) as wrk,
            tc.tile_pool(name="gat", bufs=3) as gat,
            tc.tile_pool(name="ps_slot", bufs=3, space="PSUM") as ps_slot,
            tc.tile_pool(name="ps_tp", bufs=2, space="PSUM") as ps_tp,
            tc.tile_pool(name="ps_rst", bufs=2, space="PSUM") as ps_rst,
            tc.tile_pool(name="ps_pool", bufs=1, space="PSUM") as ps_pool,
        ):
            ident = res.tile([P, P], FP, tag="ident")
            make_identity(nc, ident[:])

            src32_sb = res.tile([P, T_total], I32, tag="src32")
            nc.sync.dma_start(src32_sb[:], src32_d[:])
            feat32_sb = res.tile([P, nblk], I32, tag="feat32")
            nc.sync.dma_start(feat32_sb[:], feat32_d[:])
            d2s_sb = res.tile([P, nblk], I32, tag="d2s")
            nc.sync.dma_start(d2s_sb[:], d2s_d[:])
            emb_sb = res.tile([P, NVC * H], FP, tag="emb")
            for c in range(NVC):
                nc.sync.dma_start(emb_sb[:, c * H:(c + 1) * H],
                                  emb_d[c * P:(c + 1) * P, :])
            ws_sb = res.tile([P, L * H], FP, tag="ws")
            wn_sb = res.tile([P, L * H], FP, tag="wn")
            wsum_sb = res.tile([P, L * H], FP, tag="wsum")
            for l in range(L):
                nc.sync.dma_start(ws_sb[:, l * H:(l + 1) * H], ws_d[l])
                nc.sync.dma_start(wn_sb[:, l * H:(l + 1) * H], wn_d[l])
                nc.sync.dma_start(wsum_sb[:, l * H:(l + 1) * H], wsum_d[l])
            tflag_sb = res.tile([P, 1], FP, tag="tflag")
            nc.sync.dma_start(tflag_sb[:], tflag_d[:])
            bcol_sb = res.tile([P, L], FP, tag="bcol")
            nc.sync.dma_start(bcol_sb[:], bcol_d[:])
            gcol_sb = res.tile([P, L], FP, tag="gcol")
            nc.sync.dma_start(gcol_sb[:], gcol_d[:])
            becol_sb = res.tile([P, L], FP, tag="becol")
            nc.sync.dma_start(becol_sb[:], becol_d[:])
            acol_sb = res.tile([P, L], FP, tag="acol")
            nc.sync.dma_start(acol_sb[:], acol_d[:])

            h_stage = res.tile([P, nblk * P], FP, tag="hstage")
            stats_sum = res.tile([P, nblk], FP, tag="ssum")
            stats_sq = res.tile([P, nblk], FP, tag="ssq")
            scratch = res.tile([P, P], FP, tag="scratch")
            eps_col = res.tile([P, 1], FP, tag="eps")
            nc.vector.memset(eps_col[:], float(EPS))
            # per-layer BN affine (s, t), folded weights and bias columns
            s_all = res.tile([P, L], FP, tag="sall")
            t_all = res.tile([P, L], FP, tag="tall")
            wsf_sb = res.tile([P, L * H], FP, tag="wsf")
            wnf_sb = res.tile([P, L * H], FP, tag="wnf")
            biasf_sb = res.tile([P, L], FP, tag="biasf")
            pl_all = res.tile([P, L * H], FP, tag="plall")

            # S staging: 32 slot rows per PSUM group, 8 groups per call
            S_bufs = []
            for i in range(4):
                Sb = res.tile([J, PPC * H], FP, tag=f"S{i}")
                S_bufs.append(Sb)

            # zero the trailing slotbuf block once (deg-0 rows point here)
            nc.sync.dma_start(slotbuf[ncalls * NSLOT:, :], zeros_d[:P, :])

            def emit_agg_call(c):
                gt = gat.tile([P, TPC * H], FP, tag="g")
                if "gather" not in ablate:
                    for ti in range(TPC):
                        t_glob = c * TPC + ti
                        nc.gpsimd.indirect_dma_start(
                            out=gt[:, ti * H:(ti + 1) * H],
                            out_offset=None, in_=h_full[:],
                            in_offset=bass.IndirectOffsetOnAxis(
                                ap=src32_sb[:, t_glob:t_glob + 1],
                                axis=0))
                it = wrk.tile([P, TPC * J], FP, tag="indblk")
                nc.sync.dma_start(
                    it[:], ind_d[:, c * TPC * J:(c + 1) * TPC * J])
                S = S_bufs[c % 4]
                for q in range(PPC):
                    ps = ps_slot.tile([P, H], FP, tag="slot")
                    if "aggmm" not in ablate:
                        # 3 tiles accumulate into the group's 32 shared slots
                        for ti in range(TPP):
                            t_loc = q * TPP + ti
                            nc.tensor.matmul(
                                ps[:J, :],
                                lhsT=it[:, t_loc * J:(t_loc + 1) * J],
                                rhs=gt[:, t_loc * H:(t_loc + 1) * H],
                                start=(ti == 0), stop=(ti == TPP - 1))
                        nc.vector.tensor_copy(
                            S[:, q * H:(q + 1) * H], ps[:J, :])
                if "scatter" not in ablate:
                    # one direct HWDGE DMA flushes all 256 slots
                    nc.sync.dma_start(
                        slotbuf[c * NSLOT:(c + 1) * NSLOT, :]
                        .rearrange("(q p) f -> p q f", p=J),
                        S[:].rearrange("p (q f) -> p q f", f=H))

            def emit_stats_post(j):
                """s_j, t_j from the (already AllReduced) stats of z^j; fold
                layer j+1's weights/bias; apply the pool affine for layer j."""
                sxr = wrk.tile([P, 1], FP, tag="sxr")
                nc.sync.dma_start(sxr[:, 0:1], stats_red[0:1, :])
                sqr = wrk.tile([P, 1], FP, tag="sqr")
                nc.sync.dma_start(sqr[:, 0:1], stats_red[1:2, :])
                mu = wrk.tile([P, 1], FP, tag="mu")
                nc.scalar.mul(mu[:], sxr[:], 1.0 / N)
                ex2 = wrk.tile([P, 1], FP, tag="ex2")
                nc.scalar.mul(ex2[:], sqr[:], 1.0 / N)
                mu2 = wrk.tile([P, 1], FP, tag="mu2")
                nc.scalar.square(mu2[:], mu[:])
                var = wrk.tile([P, 1], FP, tag="var")
                nc.vector.tensor_sub(var[:], ex2[:], mu2[:])
                sd = wrk.tile([P, 1], FP, tag="sd")
                nc.scalar.activation(sd[:], var[:],
                                     mybir.ActivationFunctionType.Sqrt,
                                     bias=eps_col[:])
                rstd = wrk.tile([P, 1], FP, tag="rstd")
                nc.vector.reciprocal(rstd[:], sd[:])
                s_col = s_all[:, j:j + 1]
                t_col = t_all[:, j:j + 1]
                nc.vector.tensor_mul(s_col, rstd[:], gcol_sb[:, j:j + 1])
                msc = wrk.tile([P, 1], FP, tag="msc")
                nc.vector.tensor_mul(msc[:], mu[:], s_col)
                nc.vector.tensor_sub(t_col, becol_sb[:, j:j + 1], msc[:])
                if j < L - 1:
                    ln = j + 1
                    nc.vector.tensor_scalar_mul(
                        wsf_sb[:, ln * H:(ln + 1) * H],
                        ws_sb[:, ln * H:(ln + 1) * H], s_col)
                    nc.vector.tensor_scalar_mul(
                        wnf_sb[:, ln * H:(ln + 1) * H],
                        wn_sb[:, ln * H:(ln + 1) * H], s_col)
                    ps_b = ps_rst.tile([P, H], FP, tag="rst")
                    nc.tensor.matmul(ps_b[:, 0:1],
                                     lhsT=wsum_sb[:, ln * H:(ln + 1) * H],
                                     rhs=t_col, start=True, stop=True)
                    nc.vector.tensor_add(biasf_sb[:, ln:ln + 1], ps_b[:, 0:1],
                                         bcol_sb[:, ln:ln + 1])
                # pool affine for layer j: [H, G] = s*poolT + t (core 0 only)
                ps_t = ps_tp.tile([P, P], FP, tag="tp")
                nc.tensor.transpose(out=ps_t[:],
                                    in_=pl_all[:, j * H:(j + 1) * H],
                                    identity=ident[:])
                poolT = wrk.tile([P, P], FP, tag="poolT")
                nc.scalar.copy(poolT[:], ps_t[:])
                tf = wrk.tile([P, 1], FP, tag="tf")
                nc.vector.tensor_mul(tf[:], t_col, tflag_sb[:])
                pla = wrk.tile([P, G], FP, tag="pla")
                nc.vector.scalar_tensor_tensor(
                    pla[:], poolT[:, :G], s_col, tf[:].to_broadcast([P, G]),
                    op0=mybir.AluOpType.mult, op1=mybir.AluOpType.add)
                nc.sync.dma_start(pool_loc[j], pla[:])

            def emit_block(l, bI, ps_p):
                    nn = last if bI == nblk - 1 else P
                    ab = wrk.tile([P, H], FP, tag="mablk")
                    if l == 0:
                        cnt_sb = wrk.tile([P, NVC * H], FP, tag="cntblk")
                        nc.sync.dma_start(cnt_sb[:], cnt_d[bI])
                        ps_a = ps_rst.tile([P, H], FP, tag="rst")
                        for cv in range(NVC):
                            nc.tensor.matmul(
                                ps_a[:],
                                lhsT=cnt_sb[:, cv * H:(cv + 1) * H],
                                rhs=emb_sb[:, cv * H:(cv + 1) * H],
                                start=(cv == 0), stop=(cv == NVC - 1))
                        nc.vector.tensor_copy(ab[:], ps_a[:])
                    else:
                        nc.gpsimd.indirect_dma_start(
                            out=ab[:], out_offset=None, in_=slotbuf[:],
                            in_offset=bass.IndirectOffsetOnAxis(
                                ap=d2s_sb[:, bI:bI + 1], axis=0))
                    ps_t = ps_tp.tile([P, P], FP, tag="tp")
                    nc.tensor.transpose(out=ps_t[:], in_=ab[:],
                                        identity=ident[:])
                    aT = wrk.tile([P, P], FP, tag="aT")
                    nc.scalar.copy(aT[:], ps_t[:])

                    if l == 0:
                        g0 = wrk.tile([P, H], FP, tag="g0")
                        nc.gpsimd.indirect_dma_start(
                            out=g0[:], out_offset=None, in_=emb_d[:],
                            in_offset=bass.IndirectOffsetOnAxis(
                                ap=feat32_sb[:, bI:bI + 1], axis=0))
                        ps_t0 = ps_tp.tile([P, P], FP, tag="tp")
                        nc.tensor.transpose(out=ps_t0[:], in_=g0[:],
                                            identity=ident[:])
                        hT = wrk.tile([P, P], FP, tag="hT")
                        nc.scalar.copy(hT[:], ps_t0[:])
                        rhs_self = hT[:]
                        lhs_s = ws_sb[:, 0:H]
                        lhs_n = wn_sb[:, 0:H]
                        bc = bcol_sb[:, 0:1]
                    else:
                        rhs_self = h_stage[:, bI * P:(bI + 1) * P]
                        lhs_s = wsf_sb[:, l * H:(l + 1) * H]
                        lhs_n = wnf_sb[:, l * H:(l + 1) * H]
                        bc = biasf_sb[:, l:l + 1]

                    ps_r = ps_rst.tile([P, H], FP, tag="rst")
                    nc.tensor.matmul(ps_r[:], lhsT=lhs_s, rhs=rhs_self,
                                     start=True, stop=False)
                    nc.tensor.matmul(ps_r[:], lhsT=lhs_n, rhs=aT[:],
                                     start=False, stop=True)

                    t1 = wrk.tile([P, P], FP, tag="t1")
                    nc.scalar.activation(t1[:], ps_r[:],
                                         mybir.ActivationFunctionType.Relu,
                                         bias=bc)
                    neg = wrk.tile([P, P], FP, tag="neg")
                    nc.vector.tensor_scalar(
                        neg[:], ps_r[:], bc, 0.0,
                        op0=mybir.AluOpType.add, op1=mybir.AluOpType.min)
                    zb = h_stage[:, bI * P:(bI + 1) * P]
                    if nn == P:
                        nc.vector.scalar_tensor_tensor(
                            zb, neg[:], acol_sb[:, l:l + 1], t1[:],
                            op0=mybir.AluOpType.mult, op1=mybir.AluOpType.add,
                            accum_out=stats_sum[:, bI:bI + 1])
                        nc.scalar.activation(scratch[:], zb,
                                             mybir.ActivationFunctionType.Square,
                                             accum_out=stats_sq[:, bI:bI + 1])
                    else:
                        nc.vector.scalar_tensor_tensor(
                            h_stage[:, bI * P:bI * P + nn],
                            neg[:, :nn], acol_sb[:, l:l + 1], t1[:, :nn],
                            op0=mybir.AluOpType.mult, op1=mybir.AluOpType.add,
                            accum_out=stats_sum[:, bI:bI + 1])
                        nc.vector.scalar_tensor_tensor(
                            h_stage[:, bI * P + nn:(bI + 1) * P],
                            neg[:, nn:], acol_sb[:, l:l + 1], t1[:, nn:],
                            op0=mybir.AluOpType.mult, op1=mybir.AluOpType.add)
                        nc.scalar.activation(
                            scratch[:, :nn], h_stage[:, bI * P:bI * P + nn],
                            mybir.ActivationFunctionType.Square,
                            accum_out=stats_sq[:, bI:bI + 1])

                    # fused tail (old pass B): transpose z to node rows,
                    # write the shard, accumulate the raw-z pool
                    ps_t2 = ps_tp.tile([P, P], FP, tag="tp")
                    nc.tensor.transpose(out=ps_t2[:], in_=zb,
                                        identity=ident[:])
                    hnm = wrk.tile([P, P], FP, tag="hnm")
                    nc.scalar.copy(hnm[:], ps_t2[:])
                    if l < L - 1:
                        nc.sync.dma_start(
                            h_shard[bI * P:bI * P + nn, :], hnm[:nn, :])
                    gb = wrk.tile([P, G], FP, tag="gblk")
                    nc.sync.dma_start(gb[:], gind_d[bI])
                    nc.tensor.matmul(ps_p[:G, :], lhsT=gb[:], rhs=hnm[:],
                                     start=(bI == 0), stop=(bI == nblk - 1))

            def emit_layer_tail(l, ps_p):
                nc.vector.tensor_copy(pl_all[:G, l * H:(l + 1) * H],
                                      ps_p[:G, :])
                # per-channel z sums for this layer's BN stats
                sx = wrk.tile([P, 1], FP, tag="sx")
                nc.vector.tensor_reduce(sx[:], stats_sum[:],
                                        axis=mybir.AxisListType.X,
                                        op=mybir.AluOpType.add)
                sq = wrk.tile([P, 1], FP, tag="sq")
                nc.vector.tensor_reduce(sq[:], stats_sq[:],
                                        axis=mybir.AxisListType.X,
                                        op=mybir.AluOpType.add)
                nc.sync.dma_start(stats_loc[0:1, :], sx[:, 0:1])
                nc.sync.dma_start(stats_loc[1:2, :], sq[:, 0:1])

            # ---------------- schedule ----------------
            ps_p = ps_pool.tile([P, H], FP, tag="pool")
            for bI in range(nblk):
                emit_block(0, bI, ps_p)
            emit_layer_tail(0, ps_p)
            nc.gpsimd.collective_compute(
                "AllGather", mybir.AluOpType.bypass, replica_groups=groups,
                ins=[h_shard[:]], outs=[h_full[:]])
            nc.gpsimd.collective_compute(
                "AllReduce", mybir.AluOpType.add, replica_groups=groups,
                ins=[stats_loc[:]], outs=[stats_red[:]])
            for l in range(1, L):
                ps_p = ps_pool.tile([P, H], FP, tag="pool")
                nxt = 0
                for c in range(ncalls):
                    emit_agg_call(c)
                    if c == 1:
                        # overlap the previous layer's stats postprocessing
                        # with this layer's gather stream
                        emit_stats_post(l - 1)
                    # interleave ready blocks' pass A into the gather stream.
                    # Tile does NOT track the indirect d2s gather's read of
                    # slotbuf, so correctness is by queue order: the d2s for a
                    # block issues >= LAG calls (~30us of pool work each) of
                    # gather stream after the flush covering its slots was
                    # issued, far exceeding the flush's ~10us completion.
                    while nxt < nblk and blk_ready[nxt] + 4 <= c:
                        emit_block(l, nxt, ps_p)
                        nxt += 1
                while nxt < nblk:
                    emit_block(l, nxt, ps_p)
                    nxt += 1
                emit_layer_tail(l, ps_p)
                if l < L - 1:
                    nc.gpsimd.collective_compute(
                        "AllGather", mybir.AluOpType.bypass,
                        replica_groups=groups,
                        ins=[h_shard[:]], outs=[h_full[:]])
                nc.gpsimd.collective_compute(
                    "AllReduce", mybir.AluOpType.add, replica_groups=groups,
                    ins=[stats_loc[:]], outs=[stats_red[:]])
            emit_stats_post(L - 1)

            nc.gpsimd.collective_compute(
                "AllReduce", mybir.AluOpType.add, replica_groups=groups,
                ins=[pool_loc[:]], outs=[pool_red[:]])
            for l in range(L):
                pr = wrk.tile([P, G], FP, tag="pr")
                nc.sync.dma_start(pr[:], pool_red[l])
                ps_o = ps_tp.tile([P, P], FP, tag="tp")
                nc.tensor.transpose(out=ps_o[:], in_=pr[:], identity=ident[:])
                ob = wrk.tile([P, H], FP, tag="ob")
                nc.scalar.copy(ob[:], ps_o[:])
                nc.sync.dma_start(out_d[:, l * H:(l + 1) * H], ob[:G, :])

    nc.compile()
    return nc


# ---------------------------------------------------------------------------
# entry point
# ---------------------------------------------------------------------------

_CACHE = {}


def _run(cfg, inputs, trace=False):
    from concourse.bass_utils import run_bass_kernel_spmd
    in_maps, ncalls, seg_bounds, blk_ready = _prep(cfg, **inputs)
    key = (cfg["N"], cfg["G"], cfg["H"], ncalls, tuple(seg_bounds), blk_ready)
    if key not in _CACHE:
        _CACHE[key] = build_program(cfg, ncalls, seg_bounds, blk_ready)
    nc = _CACHE[key]
    last_exc = None
    for attempt in range(3):
        try:
            return run_bass_kernel_spmd(nc, in_maps, list(range(N_CORES)),
                                        trace=trace)
        except Exception as e:  # rare transient device-unrecoverable errors
            last_exc = e
            try:
                import jax
                import jax.extend.backend
                jax.clear_caches()
                jax.extend.backend.clear_backends()
            except Exception:
                pass
    raise last_exc


def kernel(in_feat, src, dst, graph_ids, emb, W_self, W_neigh, b,
           gamma, beta, prelu_w):
    cfg = _mkcfg(**CFG_FULL)
    res = _run(cfg, dict(
        in_feat=in_feat, src=src, dst=dst, graph_ids=graph_ids, emb=emb,
        W_self=W_self, W_neigh=W_neigh, b=b, gamma=gamma, beta=beta,
        prelu_w=prelu_w))
    return np.asarray(res.results[0]["out"], np.float32)



# revision 37
# speedup vs baseline: 1.8325x; 1.0502x over previous
"""GCN (4x SAGEConv mean-agg + PReLU + BatchNorm, graph mean-pool) on 8 TRN2 NeuronCores.

Contract: kernel(**inputs) takes FULL inputs (as produced by setup_inputs) and
returns the FULL [G, 4H] output. Self-contained: all shapes/sharding hardcoded.

Sharding: nodes (and their in-edges, i.e. edges bucketed by dst) are
partitioned contiguously across 8 cores. Weights replicated. h is replicated
in HBM per layer via AllGather. BatchNorm stats and the final pooled output
use small AllReduces.

Aggregation: edges sorted by (dst-range, dst) and packed into 128-edge tiles
of <=32 whole dst runs. Per tile one indirect-DMA gather pulls h[src] rows
(128 x 512B descriptors); a (1/deg-weighted) indicator matmul reduces the tile
to its dst slots in PSUM (3 tiles per PSUM tile at base partitions 0/32/64);
all 1024 slots of a 24-tile call are flushed with ONE direct HWDGE DMA into a
slot-space DRAM buffer, and the main pass reads each 128-node block back with
a single 128-row indirect gather through the host-built inverse slot map
(deg-0 nodes point at a zeroed trailing block). Layer 1 needs no gather or
DRAM roundtrip at all: agg0 is a count-matrix matmul against the 257-row
embedding table, and the self term gathers from the tiny table directly.

The device-time floor on this hardware is the Pool engine's SWDGE fixed cost
(~1us per indirect DMA instruction, max 128 descriptors each); the batched
GPSIMD dma_gather/dma_scatter_add ucode that would lift it is not present in
this (bedrock) image.
"""

import ml_dtypes
import numpy as np

import concourse.bass as bass
import concourse.tile as tile
from concourse import bacc, mybir
from concourse.masks import make_identity

FP = mybir.dt.float32
BF = mybir.dt.bfloat16
I16 = mybir.dt.int16
I32 = mybir.dt.int32

N_CORES = 8
P = 128          # partitions
J = 32           # dst slots (runs) per edge-tile
TPP = 3          # edge-tiles per PSUM tile (matmul out base partition 0/32/64)
PPC = 8          # PSUM tiles per call
TPC = TPP * PPC  # 24 edge-tiles per gather/scatter call
NIDX = TPC * P   # 3072 gather indices per call
NSLOT = PPC * J  # 256 slot rows per call (32 shared per PSUM group)
L = 4
EPS = 1e-5

# SEG = dst rows per agg range (per core, multiple of 128). Each range gets
# its own agg tensor so Tile can overlap main-pass blocks of completed ranges
# with the remaining ranges' gathers/scatters.
CFG_FULL = dict(N=100_000, G=128, H=128, NV=257, SEG=3_200)


def _mkcfg(N, G, H, NV, SEG):
    assert N % N_CORES == 0
    npc = N // N_CORES
    nblk = (npc + P - 1) // P
    last = npc - (nblk - 1) * P
    assert SEG % P == 0
    return dict(
        N=N, G=G, H=H, NV=NV, SEG=SEG, NPC=npc, NBLK=nblk, LAST=last,
        NSEG=(npc + SEG - 1) // SEG,
        NVC=(NV + P - 1) // P,
        AGG_ROWS=nblk * P,
    )


# ---------------------------------------------------------------------------
# host-side preprocessing
# ---------------------------------------------------------------------------

def _prep_core(cfg, cc, src, dst, in_feat, invdeg):
    """Sort the core's in-edges by dst and greedily pack dst runs into
    384-edge PSUM groups (3 gather tiles sharing 32 slot rows, accumulated
    into one PSUM tile). Runs may split across a group's 3 tiles; a group
    closes when 384 edges or 32 runs are reached."""
    npc = cfg["NPC"]
    lo = cc * npc
    sel = (dst >= lo) & (dst < lo + npc)
    e_src = src[sel].astype(np.int64)
    e_dstl = (dst[sel] - lo).astype(np.int64)
    order = np.argsort(e_dstl, kind="stable")
    e_src = e_src[order]
    e_dstl = e_dstl[order]
    Ec = len(e_src)

    change = np.empty(Ec, bool)
    if Ec:
        change[0] = True
        change[1:] = e_dstl[1:] != e_dstl[:-1]
    run_starts = np.nonzero(change)[0]
    run_len = np.diff(np.concatenate([run_starts, [Ec]]))
    run_dst = e_dstl[run_starts]

    GEDGE = TPP * P  # 384 edges per group
    K = len(run_len)
    grp_of_run = np.empty(K, np.int64)
    slot_of_run = np.empty(K, np.int64)
    pos_of_run = np.empty(K, np.int64)   # first edge position within group
    g = 0
    fill = 0
    nruns = 0
    for k in range(K):
        r = int(run_len[k])
        if r > GEDGE:
            raise ValueError(f"in-degree run {r} exceeds {GEDGE}")
        if fill + r > GEDGE or nruns >= J:
            g += 1
            fill = 0
            nruns = 0
        grp_of_run[k] = g
        slot_of_run[k] = nruns
        pos_of_run[k] = fill
        fill += r
        nruns += 1
    return dict(
        Ec=Ec, e_src=e_src, e_dstl=e_dstl, run_starts=run_starts,
        run_len=run_len, run_dst=run_dst, grp_of_run=grp_of_run,
        slot_of_run=slot_of_run, pos_of_run=pos_of_run, n_groups=g + 1,
        in_feat=in_feat, invdeg=invdeg, lo=lo,
    )


def _finish_core(cfg, core, ncalls):
    npc, nblk = cfg["NPC"], cfg["NBLK"]
    NVC = cfg["NVC"]
    T_total = ncalls * TPC
    invdeg = core["invdeg"]

    src_idx = np.zeros((T_total, P), np.int64)
    ind = np.zeros((T_total, P, J), np.float32)

    run_len = core["run_len"]
    K = len(run_len)
    n_e = int(run_len.sum())
    roe = np.repeat(np.arange(K), run_len)            # run of edge
    off = np.arange(n_e) - np.repeat(core["run_starts"], run_len)
    gpos = core["pos_of_run"][roe] + off              # position in group
    grp = core["grp_of_run"][roe]
    t_e = grp * TPP + gpos // P
    pos_e = gpos % P
    src_idx[t_e, pos_e] = core["e_src"]
    ind[t_e, pos_e, core["slot_of_run"][roe]] = \
        invdeg[core["lo"] + core["e_dstl"]]

    src32 = np.ascontiguousarray(src_idx.T.astype(np.int32))  # [128, T_total]
    ind_dev = np.ascontiguousarray(
        ind.transpose(1, 0, 2).reshape(P, T_total * J)
        .astype(ml_dtypes.bfloat16))

    # inverse slot map: own node row -> slot-space index (or the zero row)
    ZROW = ncalls * NSLOT
    d2s = np.full(nblk * P, ZROW, np.int64)
    d2s[core["run_dst"]] = (
        (core["grp_of_run"] // PPC) * NSLOT
        + (core["grp_of_run"] % PPC) * J + core["slot_of_run"])
    d2s_dev = np.ascontiguousarray(d2s.reshape(nblk, P).T.astype(np.int32))

    # layer-1 count matrix (1/deg folded)
    NV = cfg["NV"]
    v_e = core["in_feat"][core["e_src"]]
    cntm = np.zeros(npc * NVC * P, np.float32)
    np.add.at(cntm, core["e_dstl"] * (NVC * P) + v_e, 1.0)
    cntm = cntm.reshape(npc, NVC * P) * invdeg[core["lo"]:core["lo"] + npc,
                                               None].astype(np.float32)
    cnt_pad = np.zeros((nblk * P, NVC * P), np.float32)
    cnt_pad[:npc] = cntm
    cb = cnt_pad.reshape(nblk, P, NVC, P)
    cnt_dev = np.ascontiguousarray(
        cb.transpose(0, 3, 2, 1).reshape(nblk, P, NVC * P))

    # layer-1 self gather indices (emb rows per own node) [128, nblk]
    feat = np.zeros(nblk * P, np.int64)
    feat[:npc] = core["in_feat"][core["lo"]:core["lo"] + npc]
    feat32 = np.ascontiguousarray(feat.reshape(nblk, P).T.astype(np.int32))

    # earliest call index after which each 128-dst block's d2s gather can run
    call_of = d2s // NSLOT          # ZROW rows map to ncalls (ready at 0)
    blk_ready = np.where(d2s == ZROW, 0, call_of).reshape(nblk, P).max(1)

    return dict(src32=src32, ind=ind_dev, cnt=cnt_dev,
                feat32=feat32, d2s=d2s_dev, blk_ready=blk_ready)


def _prep(cfg, in_feat, src, dst, graph_ids, emb, W_self, W_neigh, b,
          gamma, beta, prelu_w):
    N, G, H = cfg["N"], cfg["G"], cfg["H"]
    npc, nblk = cfg["NPC"], cfg["NBLK"]
    NV, NVC, NSEG = cfg["NV"], cfg["NVC"], cfg["NSEG"]
    in_feat = np.asarray(in_feat).astype(np.int64)
    src = np.asarray(src).astype(np.int64)
    dst = np.asarray(dst).astype(np.int64)
    graph_ids = np.asarray(graph_ids).astype(np.int64)

    deg = np.bincount(dst, minlength=N)
    invdeg = (1.0 / np.clip(deg, 1, None)).astype(np.float64)

    cores = [_prep_core(cfg, cc, src, dst, in_feat, invdeg)
             for cc in range(N_CORES)]
    ncalls = (max(c["n_groups"] for c in cores) + PPC - 1) // PPC

    fins = [_finish_core(cfg, c, ncalls) for c in cores]

    blk_ready = np.maximum.reduce([f["blk_ready"] for f in fins])

    cnt_g = np.clip(np.bincount(graph_ids, minlength=G), 1, None)
    emb_pad = np.zeros((NVC * P, H), np.float32)
    emb_pad[:NV] = np.asarray(emb, np.float32)

    seg_bounds = []
    in_maps = []
    for cc, fin in enumerate(fins):
        lo = cc * npc
        gown = np.zeros(nblk * P, np.int64)
        gown[:npc] = graph_ids[lo:lo + npc]
        gind = np.zeros((nblk * P, G), np.float32)
        gind[np.arange(npc), gown[:npc]] = 1.0 / cnt_g[gown[:npc]]
        gind = np.ascontiguousarray(gind.reshape(nblk, P, G))

        tflag = np.full((P, 1), 1.0 if cc == 0 else 0.0, np.float32)
        in_maps.append(dict(
            src32=fin["src32"], ind=fin["ind"],
            cnt=fin["cnt"], feat32=fin["feat32"], d2s=fin["d2s"],
            gind=gind, emb=emb_pad,
            zeros=np.zeros((cfg["SEG"] + P, H), np.float32),
            W_self=np.ascontiguousarray(np.asarray(W_self, np.float32)),
            W_neigh=np.ascontiguousarray(np.asarray(W_neigh, np.float32)),
            W_sum=np.ascontiguousarray(
                np.asarray(W_self, np.float32) + np.asarray(W_neigh, np.float32)),
            tflag=tflag,
            b_cols=np.ascontiguousarray(np.asarray(b, np.float32).T),
            gam_cols=np.ascontiguousarray(np.asarray(gamma, np.float32).T),
            bet_cols=np.ascontiguousarray(np.asarray(beta, np.float32).T),
            alp_cols=np.ascontiguousarray(np.asarray(prelu_w, np.float32).T),
        ))
    return in_maps, ncalls, seg_bounds, tuple(int(x) for x in blk_ready)


# ---------------------------------------------------------------------------
# device program
# ---------------------------------------------------------------------------

def build_program(cfg, ncalls, seg_bounds, blk_ready, ablate=()):
    """BN-folded pipeline: the exchanged/gathered per-node state is z = the
    pre-BatchNorm PReLU output. h = z*s + t (per-channel affine from batch
    stats) is folded into the next layer's weights on device:
      rst_pre[l+1] = z @ (diag(s_l) W_self) + agg(z) @ (diag(s_l) W_neigh)
                     + (t_l @ (W_self+W_neigh) + b)          [min in-deg >= 1]
    so the BN stats AllReduce and the affine are OFF the critical path
    (computed while the next layer's gather stream runs), and the old pass B
    (normalize + transpose + pool) is fused into pass A. Graph pooling
    accumulates raw z; the affine is applied to the [H, G] pooled tile
    (t added on core 0 only) before the final AllReduce."""
    N, G, H = cfg["N"], cfg["G"], cfg["H"]
    npc, nblk, last = cfg["NPC"], cfg["NBLK"], cfg["LAST"]
    NVC = cfg["NVC"]
    agg_rows = cfg["AGG_ROWS"]
    T_total = ncalls * TPC

    nc = bacc.Bacc("TRN2", target_bir_lowering=False, debug=False,
                   num_devices=N_CORES)

    src32_d = nc.declare_dram_parameter("src32", [P, T_total], I32,
                                        isOutput=False)
    ind_d = nc.declare_dram_parameter("ind", [P, T_total * J], BF, isOutput=False)
    cnt_d = nc.declare_dram_parameter("cnt", [nblk, P, NVC * P], FP, isOutput=False)
    feat32_d = nc.declare_dram_parameter("feat32", [P, nblk], I32,
                                         isOutput=False)
    d2s_d = nc.declare_dram_parameter("d2s", [P, nblk], I32, isOutput=False)
    gind_d = nc.declare_dram_parameter("gind", [nblk, P, G], FP, isOutput=False)
    emb_d = nc.declare_dram_parameter("emb", [NVC * P, H], FP, isOutput=False)
    zeros_d = nc.declare_dram_parameter("zeros", [cfg["SEG"] + P, H], FP,
                                        isOutput=False)
    ws_d = nc.declare_dram_parameter("W_self", [L, H, H], FP, isOutput=False)
    wn_d = nc.declare_dram_parameter("W_neigh", [L, H, H], FP, isOutput=False)
    wsum_d = nc.declare_dram_parameter("W_sum", [L, H, H], FP, isOutput=False)
    tflag_d = nc.declare_dram_parameter("tflag", [P, 1], FP, isOutput=False)
    bcol_d = nc.declare_dram_parameter("b_cols", [H, L], FP, isOutput=False)
    gcol_d = nc.declare_dram_parameter("gam_cols", [H, L], FP, isOutput=False)
    becol_d = nc.declare_dram_parameter("bet_cols", [H, L], FP, isOutput=False)
    acol_d = nc.declare_dram_parameter("alp_cols", [H, L], FP, isOutput=False)
    out_d = nc.declare_dram_parameter("out", [G, L * H], FP, isOutput=True)

    NSEG = cfg["NSEG"]
    SEG = cfg["SEG"]
    # z exchanged/gathered in bf16: halves the AllGather on the critical path
    # and the per-edge gather bytes; everything downstream accumulates fp32
    h_shard = nc.dram_tensor("h_shard", [npc, H], BF)
    h_full = nc.dram_tensor("h_full", [N, H], BF, addr_space="Shared")
    # slot-space aggregation buffer + one trailing zero block for deg-0 rows
    slotbuf = nc.dram_tensor("slotbuf", [ncalls * NSLOT + P, H], FP)
    stats_loc = nc.dram_tensor("stats_loc", [2, H], FP)
    stats_red = nc.dram_tensor("stats_red", [2, H], FP, addr_space="Shared")
    # pooled z held TRANSPOSED [H, G] so the channel affine uses per-partition
    # scalars; transposed back to [G, H] only at the very end
    pool_loc = nc.dram_tensor("pool_loc", [L, H, G], FP)
    pool_red = nc.dram_tensor("pool_red", [L, H, G], FP, addr_space="Shared")

    groups = [list(range(N_CORES))]

    with tile.TileContext(nc) as tc:
        with (
            tc.tile_pool(name="res", bufs=1) as res,
            tc.tile_pool(name="wrk", bufs=3) as wrk,
            tc.tile_pool(name="gat", bufs=3) as gat,
            tc.tile_pool(name="ps_slot", bufs=3, space="PSUM") as ps_slot,
            tc.tile_pool(name="ps_tp", bufs=2, space="PSUM") as ps_tp,
            tc.tile_pool(name="ps_rst", bufs=2, space="PSUM") as ps_rst,
            tc.tile_pool(name="ps_pool", bufs=1, space="PSUM") as ps_pool,
        ):
            ident = res.tile([P, P], FP, tag="ident")
            make_identity(nc, ident[:])

            src32_sb = res.tile([P, T_total], I32, tag="src32")
            nc.sync.dma_start(src32_sb[:], src32_d[:])
            feat32_sb = res.tile([P, nblk], I32, tag="feat32")
            nc.sync.dma_start(feat32_sb[:], feat32_d[:])
            d2s_sb = res.tile([P, nblk], I32, tag="d2s")
            nc.sync.dma_start(d2s_sb[:], d2s_d[:])
            emb_sb = res.tile([P, NVC * H], FP, tag="emb")
            for c in range(NVC):
                nc.sync.dma_start(emb_sb[:, c * H:(c + 1) * H],
                                  emb_d[c * P:(c + 1) * P, :])
            ws_sb = res.tile([P, L * H], FP, tag="ws")
            wn_sb = res.tile([P, L * H], FP, tag="wn")
            wsum_sb = res.tile([P, L * H], FP, tag="wsum")
            for l in range(L):
                nc.sync.dma_start(ws_sb[:, l * H:(l + 1) * H], ws_d[l])
                nc.sync.dma_start(wn_sb[:, l * H:(l + 1) * H], wn_d[l])
                nc.sync.dma_start(wsum_sb[:, l * H:(l + 1) * H], wsum_d[l])
            tflag_sb = res.tile([P, 1], FP, tag="tflag")
            nc.sync.dma_start(tflag_sb[:], tflag_d[:])
            bcol_sb = res.tile([P, L], FP, tag="bcol")
            nc.sync.dma_start(bcol_sb[:], bcol_d[:])
            gcol_sb = res.tile([P, L], FP, tag="gcol")
            nc.sync.dma_start(gcol_sb[:], gcol_d[:])
            becol_sb = res.tile([P, L], FP, tag="becol")
            nc.sync.dma_start(becol_sb[:], becol_d[:])
            acol_sb = res.tile([P, L], FP, tag="acol")
            nc.sync.dma_start(acol_sb[:], acol_d[:])

            h_stage = res.tile([P, nblk * P], FP, tag="hstage")
            stats_sum = res.tile([P, nblk], FP, tag="ssum")
            stats_sq = res.tile([P, nblk], FP, tag="ssq")
            scratch = res.tile([P, P], FP, tag="scratch")
            eps_col = res.tile([P, 1], FP, tag="eps")
            nc.vector.memset(eps_col[:], float(EPS))
            # per-layer BN affine (s, t), folded weights and bias columns
            s_all = res.tile([P, L], FP, tag="sall")
            t_all = res.tile([P, L], FP, tag="tall")
            wsf_sb = res.tile([P, L * H], FP, tag="wsf")
            wnf_sb = res.tile([P, L * H], FP, tag="wnf")
            biasf_sb = res.tile([P, L], FP, tag="biasf")
            pl_all = res.tile([P, L * H], FP, tag="plall")

            # S staging: 32 slot rows per PSUM group, 8 groups per call
            S_bufs = []
            for i in range(4):
                Sb = res.tile([J, PPC * H], FP, tag=f"S{i}")
                S_bufs.append(Sb)

            # zero the trailing slotbuf block once (deg-0 rows point here)
            nc.sync.dma_start(slotbuf[ncalls * NSLOT:, :], zeros_d[:P, :])

            def emit_agg_call(c):
                gt = gat.tile([P, TPC * H], BF, tag="g")
                if "gather" not in ablate:
                    for ti in range(TPC):
                        t_glob = c * TPC + ti
                        nc.gpsimd.indirect_dma_start(
                            out=gt[:, ti * H:(ti + 1) * H],
                            out_offset=None, in_=h_full[:],
                            in_offset=bass.IndirectOffsetOnAxis(
                                ap=src32_sb[:, t_glob:t_glob + 1],
                                axis=0))
                it = wrk.tile([P, TPC * J], BF, tag="indblk")
                nc.sync.dma_start(
                    it[:], ind_d[:, c * TPC * J:(c + 1) * TPC * J])
                S = S_bufs[c % 4]
                for q in range(PPC):
                    ps = ps_slot.tile([P, H], FP, tag="slot")
                    if "aggmm" not in ablate:
                        # 3 tiles accumulate into the group's 32 shared slots
                        for ti in range(TPP):
                            t_loc = q * TPP + ti
                            nc.tensor.matmul(
                                ps[:J, :],
                                lhsT=it[:, t_loc * J:(t_loc + 1) * J],
                                rhs=gt[:, t_loc * H:(t_loc + 1) * H],
                                start=(ti == 0), stop=(ti == TPP - 1))
                        nc.vector.tensor_copy(
                            S[:, q * H:(q + 1) * H], ps[:J, :])
                if "scatter" not in ablate:
                    # one direct HWDGE DMA flushes all 256 slots
                    nc.sync.dma_start(
                        slotbuf[c * NSLOT:(c + 1) * NSLOT, :]
                        .rearrange("(q p) f -> p q f", p=J),
                        S[:].rearrange("p (q f) -> p q f", f=H))

            def emit_stats_post(j):
                """s_j, t_j from the (already AllReduced) stats of z^j; fold
                layer j+1's weights/bias; apply the pool affine for layer j."""
                sxr = wrk.tile([P, 1], FP, tag="sxr")
                nc.sync.dma_start(sxr[:, 0:1], stats_red[0:1, :])
                sqr = wrk.tile([P, 1], FP, tag="sqr")
                nc.sync.dma_start(sqr[:, 0:1], stats_red[1:2, :])
                mu = wrk.tile([P, 1], FP, tag="mu")
                nc.scalar.mul(mu[:], sxr[:], 1.0 / N)
                ex2 = wrk.tile([P, 1], FP, tag="ex2")
                nc.scalar.mul(ex2[:], sqr[:], 1.0 / N)
                mu2 = wrk.tile([P, 1], FP, tag="mu2")
                nc.scalar.square(mu2[:], mu[:])
                var = wrk.tile([P, 1], FP, tag="var")
                nc.vector.tensor_sub(var[:], ex2[:], mu2[:])
                sd = wrk.tile([P, 1], FP, tag="sd")
                nc.scalar.activation(sd[:], var[:],
                                     mybir.ActivationFunctionType.Sqrt,
                                     bias=eps_col[:])
                rstd = wrk.tile([P, 1], FP, tag="rstd")
                nc.vector.reciprocal(rstd[:], sd[:])
                s_col = s_all[:, j:j + 1]
                t_col = t_all[:, j:j + 1]
                nc.vector.tensor_mul(s_col, rstd[:], gcol_sb[:, j:j + 1])
                msc = wrk.tile([P, 1], FP, tag="msc")
                nc.vector.tensor_mul(msc[:], mu[:], s_col)
                nc.vector.tensor_sub(t_col, becol_sb[:, j:j + 1], msc[:])
                if j < L - 1:
                    ln = j + 1
                    nc.vector.tensor_scalar_mul(
                        wsf_sb[:, ln * H:(ln + 1) * H],
                        ws_sb[:, ln * H:(ln + 1) * H], s_col)
                    nc.vector.tensor_scalar_mul(
                        wnf_sb[:, ln * H:(ln + 1) * H],
                        wn_sb[:, ln * H:(ln + 1) * H], s_col)
                    ps_b = ps_rst.tile([P, H], FP, tag="rst")
                    nc.tensor.matmul(ps_b[:, 0:1],
                                     lhsT=wsum_sb[:, ln * H:(ln + 1) * H],
                                     rhs=t_col, start=True, stop=True)
                    nc.vector.tensor_add(biasf_sb[:, ln:ln + 1], ps_b[:, 0:1],
                                         bcol_sb[:, ln:ln + 1])
                # pool affine for layer j: [H, G] = s*poolT + t (core 0 only)
                ps_t = ps_tp.tile([P, P], FP, tag="tp")
                nc.tensor.transpose(out=ps_t[:],
                                    in_=pl_all[:, j * H:(j + 1) * H],
                                    identity=ident[:])
                poolT = wrk.tile([P, P], FP, tag="poolT")
                nc.scalar.copy(poolT[:], ps_t[:])
                tf = wrk.tile([P, 1], FP, tag="tf")
                nc.vector.tensor_mul(tf[:], t_col, tflag_sb[:])
                pla = wrk.tile([P, G], FP, tag="pla")
                nc.vector.scalar_tensor_tensor(
                    pla[:], poolT[:, :G], s_col, tf[:].to_broadcast([P, G]),
                    op0=mybir.AluOpType.mult, op1=mybir.AluOpType.add)
                nc.sync.dma_start(pool_loc[j], pla[:])

            def emit_block(l, bI, ps_p):
                    nn = last if bI == nblk - 1 else P
                    ab = wrk.tile([P, H], FP, tag="mablk")
                    if l == 0:
                        cnt_sb = wrk.tile([P, NVC * H], FP, tag="cntblk")
                        nc.sync.dma_start(cnt_sb[:], cnt_d[bI])
                        ps_a = ps_rst.tile([P, H], FP, tag="rst")
                        for cv in range(NVC):
                            nc.tensor.matmul(
                                ps_a[:],
                                lhsT=cnt_sb[:, cv * H:(cv + 1) * H],
                                rhs=emb_sb[:, cv * H:(cv + 1) * H],
                                start=(cv == 0), stop=(cv == NVC - 1))
                        nc.vector.tensor_copy(ab[:], ps_a[:])
                    else:
                        nc.gpsimd.indirect_dma_start(
                            out=ab[:], out_offset=None, in_=slotbuf[:],
                            in_offset=bass.IndirectOffsetOnAxis(
                                ap=d2s_sb[:, bI:bI + 1], axis=0))
                    ps_t = ps_tp.tile([P, P], FP, tag="tp")
                    nc.tensor.transpose(out=ps_t[:], in_=ab[:],
                                        identity=ident[:])
                    aT = wrk.tile([P, P], FP, tag="aT")
                    nc.scalar.copy(aT[:], ps_t[:])

                    if l == 0:
                        g0 = wrk.tile([P, H], FP, tag="g0")
                        nc.gpsimd.indirect_dma_start(
                            out=g0[:], out_offset=None, in_=emb_d[:],
                            in_offset=bass.IndirectOffsetOnAxis(
                                ap=feat32_sb[:, bI:bI + 1], axis=0))
                        ps_t0 = ps_tp.tile([P, P], FP, tag="tp")
                        nc.tensor.transpose(out=ps_t0[:], in_=g0[:],
                                            identity=ident[:])
                        hT = wrk.tile([P, P], FP, tag="hT")
                        nc.scalar.copy(hT[:], ps_t0[:])
                        rhs_self = hT[:]
                        lhs_s = ws_sb[:, 0:H]
                        lhs_n = wn_sb[:, 0:H]
                        bc = bcol_sb[:, 0:1]
                    else:
                        rhs_self = h_stage[:, bI * P:(bI + 1) * P]
                        lhs_s = wsf_sb[:, l * H:(l + 1) * H]
                        lhs_n = wnf_sb[:, l * H:(l + 1) * H]
                        bc = biasf_sb[:, l:l + 1]

                    ps_r = ps_rst.tile([P, H], FP, tag="rst")
                    nc.tensor.matmul(ps_r[:], lhsT=lhs_s, rhs=rhs_self,
                                     start=True, stop=False)
                    nc.tensor.matmul(ps_r[:], lhsT=lhs_n, rhs=aT[:],
                                     start=False, stop=True)

                    t1 = wrk.tile([P, P], FP, tag="t1")
                    nc.scalar.activation(t1[:], ps_r[:],
                                         mybir.ActivationFunctionType.Relu,
                                         bias=bc)
                    neg = wrk.tile([P, P], FP, tag="neg")
                    nc.vector.tensor_scalar(
                        neg[:], ps_r[:], bc, 0.0,
                        op0=mybir.AluOpType.add, op1=mybir.AluOpType.min)
                    zb = h_stage[:, bI * P:(bI + 1) * P]
                    if nn == P:
                        nc.vector.scalar_tensor_tensor(
                            zb, neg[:], acol_sb[:, l:l + 1], t1[:],
                            op0=mybir.AluOpType.mult, op1=mybir.AluOpType.add,
                            accum_out=stats_sum[:, bI:bI + 1])
                        nc.scalar.activation(scratch[:], zb,
                                             mybir.ActivationFunctionType.Square,
                                             accum_out=stats_sq[:, bI:bI + 1])
                    else:
                        nc.vector.scalar_tensor_tensor(
                            h_stage[:, bI * P:bI * P + nn],
                            neg[:, :nn], acol_sb[:, l:l + 1], t1[:, :nn],
                            op0=mybir.AluOpType.mult, op1=mybir.AluOpType.add,
                            accum_out=stats_sum[:, bI:bI + 1])
                        nc.vector.scalar_tensor_tensor(
                            h_stage[:, bI * P + nn:(bI + 1) * P],
                            neg[:, nn:], acol_sb[:, l:l + 1], t1[:, nn:],
                            op0=mybir.AluOpType.mult, op1=mybir.AluOpType.add)
                        nc.scalar.activation(
                            scratch[:, :nn], h_stage[:, bI * P:bI * P + nn],
                            mybir.ActivationFunctionType.Square,
                            accum_out=stats_sq[:, bI:bI + 1])

                    # fused tail (old pass B): transpose z to node rows,
                    # write the shard, accumulate the raw-z pool
                    ps_t2 = ps_tp.tile([P, P], FP, tag="tp")
                    nc.tensor.transpose(out=ps_t2[:], in_=zb,
                                        identity=ident[:])
                    hnm = wrk.tile([P, P], FP, tag="hnm")
                    nc.scalar.copy(hnm[:], ps_t2[:])
                    if l < L - 1:
                        hnm_bf = wrk.tile([P, P], BF, tag="hnmbf")
                        nc.scalar.copy(hnm_bf[:], ps_t2[:])
                        nc.sync.dma_start(
                            h_shard[bI * P:bI * P + nn, :], hnm_bf[:nn, :])
                    gb = wrk.tile([P, G], FP, tag="gblk")
                    nc.sync.dma_start(gb[:], gind_d[bI])
                    nc.tensor.matmul(ps_p[:G, :], lhsT=gb[:], rhs=hnm[:],
                                     start=(bI == 0), stop=(bI == nblk - 1))

            def emit_layer_tail(l, ps_p):
                nc.vector.tensor_copy(pl_all[:G, l * H:(l + 1) * H],
                                      ps_p[:G, :])
                # per-channel z sums for this layer's BN stats
                sx = wrk.tile([P, 1], FP, tag="sx")
                nc.vector.tensor_reduce(sx[:], stats_sum[:],
                                        axis=mybir.AxisListType.X,
                                        op=mybir.AluOpType.add)
                sq = wrk.tile([P, 1], FP, tag="sq")
                nc.vector.tensor_reduce(sq[:], stats_sq[:],
                                        axis=mybir.AxisListType.X,
                                        op=mybir.AluOpType.add)
                nc.sync.dma_start(stats_loc[0:1, :], sx[:, 0:1])
                nc.sync.dma_start(stats_loc[1:2, :], sq[:, 0:1])

            # ---------------- schedule ----------------
            ps_p = ps_pool.tile([P, H], FP, tag="pool")
            for bI in range(nblk):
                emit_block(0, bI, ps_p)
            emit_layer_tail(0, ps_p)
            nc.gpsimd.collective_compute(
                "AllGather", mybir.AluOpType.bypass, replica_groups=groups,
                ins=[h_shard[:]], outs=[h_full[:]])
            nc.gpsimd.collective_compute(
                "AllReduce", mybir.AluOpType.add, replica_groups=groups,
                ins=[stats_loc[:]], outs=[stats_red[:]])
            for l in range(1, L):
                ps_p = ps_pool.tile([P, H], FP, tag="pool")
                nxt = 0
                for c in range(ncalls):
                    emit_agg_call(c)
                    if c == 1:
                        # overlap the previous layer's stats postprocessing
                        # with this layer's gather stream
                        emit_stats_post(l - 1)
                    # interleave ready blocks' pass A into the gather stream.
                    # Tile does NOT track the indirect d2s gather's read of
                    # slotbuf, so correctness is by queue order: the d2s for a
                    # block issues >= LAG calls (~30us of pool work each) of
                    # gather stream after the flush covering its slots was
                    # issued, far exceeding the flush's ~10us completion.
                    while nxt < nblk and blk_ready[nxt] + 4 <= c:
                        emit_block(l, nxt, ps_p)
                        nxt += 1
                while nxt < nblk:
                    emit_block(l, nxt, ps_p)
                    nxt += 1
                emit_layer_tail(l, ps_p)
                if l < L - 1:
                    nc.gpsimd.collective_compute(
                        "AllGather", mybir.AluOpType.bypass,
                        replica_groups=groups,
                        ins=[h_shard[:]], outs=[h_full[:]])
                nc.gpsimd.collective_compute(
                    "AllReduce", mybir.AluOpType.add, replica_groups=groups,
                    ins=[stats_loc[:]], outs=[stats_red[:]])
            emit_stats_post(L - 1)

            nc.gpsimd.collective_compute(
                "AllReduce", mybir.AluOpType.add, replica_groups=groups,
                ins=[pool_loc[:]], outs=[pool_red[:]])
            for l in range(L):
                pr = wrk.tile([P, G], FP, tag="pr")
                nc.sync.dma_start(pr[:], pool_red[l])
                ps_o = ps_tp.tile([P, P], FP, tag="tp")
                nc.tensor.transpose(out=ps_o[:], in_=pr[:], identity=ident[:])
                ob = wrk.tile([P, H], FP, tag="ob")
                nc.scalar.copy(ob[:], ps_o[:])
                nc.sync.dma_start(out_d[:, l * H:(l + 1) * H], ob[:G, :])

    nc.compile()
    return nc


# ---------------------------------------------------------------------------
# entry point
# ---------------------------------------------------------------------------

_CACHE = {}


def _run(cfg, inputs, trace=False):
    from concourse.bass_utils import run_bass_kernel_spmd
    in_maps, ncalls, seg_bounds, blk_ready = _prep(cfg, **inputs)
    key = (cfg["N"], cfg["G"], cfg["H"], ncalls, tuple(seg_bounds), blk_ready)
    if key not in _CACHE:
        _CACHE[key] = build_program(cfg, ncalls, seg_bounds, blk_ready)
    nc = _CACHE[key]
    last_exc = None
    for attempt in range(3):
        try:
            return run_bass_kernel_spmd(nc, in_maps, list(range(N_CORES)),
                                        trace=trace)
        except Exception as e:  # rare transient device-unrecoverable errors
            last_exc = e
            try:
                import jax
                import jax.extend.backend
                jax.clear_caches()
                jax.extend.backend.clear_backends()
            except Exception:
                pass
    raise last_exc


def kernel(in_feat, src, dst, graph_ids, emb, W_self, W_neigh, b,
           gamma, beta, prelu_w):
    cfg = _mkcfg(**CFG_FULL)
    res = _run(cfg, dict(
        in_feat=in_feat, src=src, dst=dst, graph_ids=graph_ids, emb=emb,
        W_self=W_self, W_neigh=W_neigh, b=b, gamma=gamma, beta=beta,
        prelu_w=prelu_w))
    return np.asarray(res.results[0]["out"], np.float32)



# revision 51
# speedup vs baseline: 1.8786x; 1.0251x over previous
"""GCN (4x SAGEConv mean-agg + PReLU + BatchNorm, graph mean-pool) on 8 TRN2 NeuronCores.

Contract: kernel(**inputs) takes FULL inputs (as produced by setup_inputs) and
returns the FULL [G, 4H] output. Self-contained: all shapes/sharding hardcoded.

Sharding: nodes (and their in-edges, i.e. edges bucketed by dst) are
partitioned contiguously across 8 cores. Weights replicated. h is replicated
in HBM per layer via AllGather. BatchNorm stats and the final pooled output
use small AllReduces.

Aggregation: edges sorted by (dst-range, dst) and packed into 128-edge tiles
of <=32 whole dst runs. Per tile one indirect-DMA gather pulls h[src] rows
(128 x 512B descriptors); a (1/deg-weighted) indicator matmul reduces the tile
to its dst slots in PSUM (3 tiles per PSUM tile at base partitions 0/32/64);
all 1024 slots of a 24-tile call are flushed with ONE direct HWDGE DMA into a
slot-space DRAM buffer, and the main pass reads each 128-node block back with
a single 128-row indirect gather through the host-built inverse slot map
(deg-0 nodes point at a zeroed trailing block). Layer 1 needs no gather or
DRAM roundtrip at all: agg0 is a count-matrix matmul against the 257-row
embedding table, and the self term gathers from the tiny table directly.

The device-time floor on this hardware is the Pool engine's SWDGE fixed cost
(~1us per indirect DMA instruction, max 128 descriptors each); the batched
GPSIMD dma_gather/dma_scatter_add ucode that would lift it is not present in
this (bedrock) image.
"""

import ml_dtypes
import numpy as np

import concourse.bass as bass
import concourse.tile as tile
from concourse import bacc, mybir
from concourse.masks import make_identity

FP = mybir.dt.float32
BF = mybir.dt.bfloat16
I16 = mybir.dt.int16
I32 = mybir.dt.int32

N_CORES = 8
P = 128          # partitions
J = 32           # dst slots (runs) per edge-tile
TPP = 3          # edge-tiles per PSUM tile (matmul out base partition 0/32/64)
PPC = 8          # PSUM tiles per call
TPC = TPP * PPC  # 24 edge-tiles per gather/scatter call
NIDX = TPC * P   # 3072 gather indices per call
NSLOT = PPC * J  # 256 slot rows per call (32 shared per PSUM group)
L = 4
EPS = 1e-5

# SEG = dst rows per agg range (per core, multiple of 128). Each range gets
# its own agg tensor so Tile can overlap main-pass blocks of completed ranges
# with the remaining ranges' gathers/scatters.
CFG_FULL = dict(N=100_000, G=128, H=128, NV=257, SEG=3_200)


def _mkcfg(N, G, H, NV, SEG):
    assert N % N_CORES == 0
    npc = N // N_CORES
    nblk = (npc + P - 1) // P
    last = npc - (nblk - 1) * P
    assert SEG % P == 0
    return dict(
        N=N, G=G, H=H, NV=NV, SEG=SEG, NPC=npc, NBLK=nblk, LAST=last,
        NSEG=(npc + SEG - 1) // SEG,
        NVC=(NV + P - 1) // P,
        AGG_ROWS=nblk * P,
    )


# ---------------------------------------------------------------------------
# host-side preprocessing
# ---------------------------------------------------------------------------

def _prep_core(cfg, cc, src, dst, in_feat, invdeg):
    """Sort the core's in-edges by dst. Aggregation is per 128-node block:
    the block's edges are chunked into 128-edge gather tiles; each tile's
    [128, 128] indicator (position -> node offset within block, scaled by
    1/deg) accumulates into one PSUM tile whose 128 slot rows ARE the block's
    nodes in order — no slot buffer, no inverse map."""
    npc, nblk = cfg["NPC"], cfg["NBLK"]
    lo = cc * npc
    sel = (dst >= lo) & (dst < lo + npc)
    e_src = src[sel].astype(np.int64)
    e_dstl = (dst[sel] - lo).astype(np.int64)
    order = np.argsort(e_dstl, kind="stable")
    e_src = e_src[order]
    e_dstl = e_dstl[order]
    eblk = e_dstl // P
    counts = np.bincount(eblk, minlength=nblk)
    tb = (counts + P - 1) // P   # gather tiles needed per node block
    return dict(
        e_src=e_src, e_dstl=e_dstl, eblk=eblk, counts=counts, tb=tb,
        in_feat=in_feat, invdeg=invdeg, lo=lo,
    )


def _finish_core(cfg, core, tb):
    npc, nblk = cfg["NPC"], cfg["NBLK"]
    NVC = cfg["NVC"]
    invdeg = core["invdeg"]
    toff = np.concatenate([[0], np.cumsum(tb)])
    T_total = int(toff[-1])

    src_idx = np.zeros((T_total, P), np.int64)
    ind = np.zeros((T_total, P, P), np.float32)

    counts = core["counts"]
    blk_start = np.concatenate([[0], np.cumsum(counts)])[:-1]
    pos_in_blk = np.arange(len(core["e_src"])) - blk_start[core["eblk"]]
    t_e = toff[core["eblk"]] + pos_in_blk // P
    pos_e = pos_in_blk % P
    src_idx[t_e, pos_e] = core["e_src"]
    ind[t_e, pos_e, core["e_dstl"] % P] = invdeg[core["lo"] + core["e_dstl"]]

    src32 = np.ascontiguousarray(src_idx.T.astype(np.int32))  # [128, T_total]
    ind_dev = np.ascontiguousarray(
        ind.transpose(1, 0, 2).reshape(P, T_total * P)
        .astype(ml_dtypes.bfloat16))

    # layer-1 count matrix (1/deg folded)
    NV = cfg["NV"]
    v_e = core["in_feat"][core["e_src"]]
    cntm = np.zeros(npc * NVC * P, np.float32)
    np.add.at(cntm, core["e_dstl"] * (NVC * P) + v_e, 1.0)
    cntm = cntm.reshape(npc, NVC * P) * invdeg[core["lo"]:core["lo"] + npc,
                                               None].astype(np.float32)
    cnt_pad = np.zeros((nblk * P, NVC * P), np.float32)
    cnt_pad[:npc] = cntm
    cb = cnt_pad.reshape(nblk, P, NVC, P)
    cnt_dev = np.ascontiguousarray(
        cb.transpose(0, 3, 2, 1).reshape(nblk, P, NVC * P))

    # layer-1 self gather indices (emb rows per own node) [128, nblk]
    feat = np.zeros(nblk * P, np.int64)
    feat[:npc] = core["in_feat"][core["lo"]:core["lo"] + npc]
    feat32 = np.ascontiguousarray(feat.reshape(nblk, P).T.astype(np.int32))

    return dict(src32=src32, ind=ind_dev, cnt=cnt_dev, feat32=feat32)


def _prep(cfg, in_feat, src, dst, graph_ids, emb, W_self, W_neigh, b,
          gamma, beta, prelu_w):
    N, G, H = cfg["N"], cfg["G"], cfg["H"]
    npc, nblk = cfg["NPC"], cfg["NBLK"]
    NV, NVC, NSEG = cfg["NV"], cfg["NVC"], cfg["NSEG"]
    in_feat = np.asarray(in_feat).astype(np.int64)
    src = np.asarray(src).astype(np.int64)
    dst = np.asarray(dst).astype(np.int64)
    graph_ids = np.asarray(graph_ids).astype(np.int64)

    deg = np.bincount(dst, minlength=N)
    invdeg = (1.0 / np.clip(deg, 1, None)).astype(np.float64)

    cores = [_prep_core(cfg, cc, src, dst, in_feat, invdeg)
             for cc in range(N_CORES)]
    tb = np.maximum.reduce([c["tb"] for c in cores])  # SPMD: pad to max

    fins = [_finish_core(cfg, c, tb) for c in cores]

    cnt_g = np.clip(np.bincount(graph_ids, minlength=G), 1, None)
    emb_pad = np.zeros((NVC * P, H), np.float32)
    emb_pad[:NV] = np.asarray(emb, np.float32)

    seg_bounds = []
    in_maps = []
    for cc, fin in enumerate(fins):
        lo = cc * npc
        gown = np.zeros(nblk * P, np.int64)
        gown[:npc] = graph_ids[lo:lo + npc]
        gind = np.zeros((nblk * P, G), np.float32)
        gind[np.arange(npc), gown[:npc]] = 1.0 / cnt_g[gown[:npc]]
        gind = np.ascontiguousarray(gind.reshape(nblk, P, G))

        tflag = np.full((P, 1), 1.0 if cc == 0 else 0.0, np.float32)
        in_maps.append(dict(
            src32=fin["src32"], ind=fin["ind"],
            cnt=fin["cnt"], feat32=fin["feat32"],
            gind=gind, emb=emb_pad,
            W_self=np.ascontiguousarray(np.asarray(W_self, np.float32)),
            W_neigh=np.ascontiguousarray(np.asarray(W_neigh, np.float32)),
            W_sum=np.ascontiguousarray(
                np.asarray(W_self, np.float32) + np.asarray(W_neigh, np.float32)),
            tflag=tflag,
            b_cols=np.ascontiguousarray(np.asarray(b, np.float32).T),
            gam_cols=np.ascontiguousarray(np.asarray(gamma, np.float32).T),
            bet_cols=np.ascontiguousarray(np.asarray(beta, np.float32).T),
            alp_cols=np.ascontiguousarray(np.asarray(prelu_w, np.float32).T),
        ))
    return in_maps, tuple(int(x) for x in tb)


# ---------------------------------------------------------------------------
# device program
# ---------------------------------------------------------------------------

def build_program(cfg, tb, ablate=()):
    """BN-folded pipeline: the exchanged/gathered per-node state is z = the
    pre-BatchNorm PReLU output. h = z*s + t (per-channel affine from batch
    stats) is folded into the next layer's weights on device:
      rst_pre[l+1] = z @ (diag(s_l) W_self) + agg(z) @ (diag(s_l) W_neigh)
                     + (t_l @ (W_self+W_neigh) + b)          [min in-deg >= 1]
    so the BN stats AllReduce and the affine are OFF the critical path
    (computed while the next layer's gather stream runs), and the old pass B
    (normalize + transpose + pool) is fused into pass A. Graph pooling
    accumulates raw z; the affine is applied to the [H, G] pooled tile
    (t added on core 0 only) before the final AllReduce."""
    N, G, H = cfg["N"], cfg["G"], cfg["H"]
    npc, nblk, last = cfg["NPC"], cfg["NBLK"], cfg["LAST"]
    NVC = cfg["NVC"]
    agg_rows = cfg["AGG_ROWS"]
    toff = [0]
    for t in tb:
        toff.append(toff[-1] + t)
    T_total = toff[-1]
    TBMAX = max(tb)

    nc = bacc.Bacc("TRN2", target_bir_lowering=False, debug=False,
                   num_devices=N_CORES)

    src32_d = nc.declare_dram_parameter("src32", [P, T_total], I32,
                                        isOutput=False)
    ind_d = nc.declare_dram_parameter("ind", [P, T_total * P], BF, isOutput=False)
    cnt_d = nc.declare_dram_parameter("cnt", [nblk, P, NVC * P], FP, isOutput=False)
    feat32_d = nc.declare_dram_parameter("feat32", [P, nblk], I32,
                                         isOutput=False)
    gind_d = nc.declare_dram_parameter("gind", [nblk, P, G], FP, isOutput=False)
    emb_d = nc.declare_dram_parameter("emb", [NVC * P, H], FP, isOutput=False)
    ws_d = nc.declare_dram_parameter("W_self", [L, H, H], FP, isOutput=False)
    wn_d = nc.declare_dram_parameter("W_neigh", [L, H, H], FP, isOutput=False)
    wsum_d = nc.declare_dram_parameter("W_sum", [L, H, H], FP, isOutput=False)
    tflag_d = nc.declare_dram_parameter("tflag", [P, 1], FP, isOutput=False)
    bcol_d = nc.declare_dram_parameter("b_cols", [H, L], FP, isOutput=False)
    gcol_d = nc.declare_dram_parameter("gam_cols", [H, L], FP, isOutput=False)
    becol_d = nc.declare_dram_parameter("bet_cols", [H, L], FP, isOutput=False)
    acol_d = nc.declare_dram_parameter("alp_cols", [H, L], FP, isOutput=False)
    out_d = nc.declare_dram_parameter("out", [G, L * H], FP, isOutput=True)

    # z exchanged/gathered in bf16: halves the AllGather on the critical path
    # and the per-edge gather bytes; everything downstream accumulates fp32
    h_shard = nc.dram_tensor("h_shard", [npc, H], BF)
    h_full = nc.dram_tensor("h_full", [N, H], BF, addr_space="Shared")
    stats_loc = nc.dram_tensor("stats_loc", [2, H], FP)
    stats_red = nc.dram_tensor("stats_red", [2, H], FP, addr_space="Shared")
    # pooled z held TRANSPOSED [H, G] so the channel affine uses per-partition
    # scalars; transposed back to [G, H] only at the very end
    pool_loc = nc.dram_tensor("pool_loc", [L, H, G], FP)
    pool_red = nc.dram_tensor("pool_red", [L, H, G], FP, addr_space="Shared")

    groups = [list(range(N_CORES))]

    with tile.TileContext(nc) as tc:
        with (
            tc.tile_pool(name="res", bufs=1) as res,
            tc.tile_pool(name="wrk", bufs=3) as wrk,
            tc.tile_pool(name="gat", bufs=3) as gat,
            tc.tile_pool(name="ps_slot", bufs=3, space="PSUM") as ps_slot,
            tc.tile_pool(name="ps_tp", bufs=2, space="PSUM") as ps_tp,
            tc.tile_pool(name="ps_rst", bufs=2, space="PSUM") as ps_rst,
            tc.tile_pool(name="ps_pool", bufs=1, space="PSUM") as ps_pool,
        ):
            ident = res.tile([P, P], FP, tag="ident")
            make_identity(nc, ident[:])

            src32_sb = res.tile([P, T_total], I32, tag="src32")
            nc.sync.dma_start(src32_sb[:], src32_d[:])
            feat32_sb = res.tile([P, nblk], I32, tag="feat32")
            nc.sync.dma_start(feat32_sb[:], feat32_d[:])
            emb_sb = res.tile([P, NVC * H], FP, tag="emb")
            for c in range(NVC):
                nc.sync.dma_start(emb_sb[:, c * H:(c + 1) * H],
                                  emb_d[c * P:(c + 1) * P, :])
            ws_sb = res.tile([P, L * H], FP, tag="ws")
            wn_sb = res.tile([P, L * H], FP, tag="wn")
            wsum_sb = res.tile([P, L * H], FP, tag="wsum")
            for l in range(L):
                nc.sync.dma_start(ws_sb[:, l * H:(l + 1) * H], ws_d[l])
                nc.sync.dma_start(wn_sb[:, l * H:(l + 1) * H], wn_d[l])
                nc.sync.dma_start(wsum_sb[:, l * H:(l + 1) * H], wsum_d[l])
            tflag_sb = res.tile([P, 1], FP, tag="tflag")
            nc.sync.dma_start(tflag_sb[:], tflag_d[:])
            bcol_sb = res.tile([P, L], FP, tag="bcol")
            nc.sync.dma_start(bcol_sb[:], bcol_d[:])
            gcol_sb = res.tile([P, L], FP, tag="gcol")
            nc.sync.dma_start(gcol_sb[:], gcol_d[:])
            becol_sb = res.tile([P, L], FP, tag="becol")
            nc.sync.dma_start(becol_sb[:], becol_d[:])
            acol_sb = res.tile([P, L], FP, tag="acol")
            nc.sync.dma_start(acol_sb[:], acol_d[:])

            h_stage = res.tile([P, nblk * P], FP, tag="hstage")
            stats_sum = res.tile([P, nblk], FP, tag="ssum")
            stats_sq = res.tile([P, nblk], FP, tag="ssq")
            scratch = res.tile([P, P], FP, tag="scratch")
            eps_col = res.tile([P, 1], FP, tag="eps")
            nc.vector.memset(eps_col[:], float(EPS))
            # per-layer BN affine (s, t), folded weights and bias columns
            s_all = res.tile([P, L], FP, tag="sall")
            t_all = res.tile([P, L], FP, tag="tall")
            wsf_sb = res.tile([P, L * H], FP, tag="wsf")
            wnf_sb = res.tile([P, L * H], FP, tag="wnf")
            biasf_sb = res.tile([P, L], FP, tag="biasf")
            pl_all = res.tile([P, L * H], FP, tag="plall")

            def emit_agg_block(bI):
                """Gather + indicator-accumulate the 128-node block's
                aggregation directly in PSUM; returns node-row agg tile."""
                nt = tb[bI]
                t0 = toff[bI]
                gt = gat.tile([P, TBMAX * H], BF, tag="g")
                if "gather" not in ablate:
                    for ti in range(nt):
                        nc.gpsimd.indirect_dma_start(
                            out=gt[:, ti * H:(ti + 1) * H],
                            out_offset=None, in_=h_full[:],
                            in_offset=bass.IndirectOffsetOnAxis(
                                ap=src32_sb[:, t0 + ti:t0 + ti + 1],
                                axis=0))
                it = wrk.tile([P, TBMAX * P], BF, tag="indblk")
                nc.sync.dma_start(
                    it[:, :nt * P], ind_d[:, t0 * P:(t0 + nt) * P])
                ps = ps_slot.tile([P, H], FP, tag="slot")
                for ti in range(nt):
                    nc.tensor.matmul(
                        ps[:],
                        lhsT=it[:, ti * P:(ti + 1) * P],
                        rhs=gt[:, ti * H:(ti + 1) * H],
                        start=(ti == 0), stop=(ti == nt - 1))
                ab = wrk.tile([P, H], FP, tag="mablk")
                nc.vector.tensor_copy(ab[:], ps[:])
                return ab

            def emit_stats_post(j):
                """s_j, t_j from the (already AllReduced) stats of z^j; fold
                layer j+1's weights/bias; apply the pool affine for layer j."""
                sxr = wrk.tile([P, 1], FP, tag="sxr")
                nc.sync.dma_start(sxr[:, 0:1], stats_red[0:1, :])
                sqr = wrk.tile([P, 1], FP, tag="sqr")
                nc.sync.dma_start(sqr[:, 0:1], stats_red[1:2, :])
                mu = wrk.tile([P, 1], FP, tag="mu")
                nc.scalar.mul(mu[:], sxr[:], 1.0 / N)
                ex2 = wrk.tile([P, 1], FP, tag="ex2")
                nc.scalar.mul(ex2[:], sqr[:], 1.0 / N)
                mu2 = wrk.tile([P, 1], FP, tag="mu2")
                nc.scalar.square(mu2[:], mu[:])
                var = wrk.tile([P, 1], FP, tag="var")
                nc.vector.tensor_sub(var[:], ex2[:], mu2[:])
                sd = wrk.tile([P, 1], FP, tag="sd")
                nc.scalar.activation(sd[:], var[:],
                                     mybir.ActivationFunctionType.Sqrt,
                                     bias=eps_col[:])
                rstd = wrk.tile([P, 1], FP, tag="rstd")
                nc.vector.reciprocal(rstd[:], sd[:])
                s_col = s_all[:, j:j + 1]
                t_col = t_all[:, j:j + 1]
                nc.vector.tensor_mul(s_col, rstd[:], gcol_sb[:, j:j + 1])
                msc = wrk.tile([P, 1], FP, tag="msc")
                nc.vector.tensor_mul(msc[:], mu[:], s_col)
                nc.vector.tensor_sub(t_col, becol_sb[:, j:j + 1], msc[:])
                if j < L - 1:
                    ln = j + 1
                    nc.vector.tensor_scalar_mul(
                        wsf_sb[:, ln * H:(ln + 1) * H],
                        ws_sb[:, ln * H:(ln + 1) * H], s_col)
                    nc.vector.tensor_scalar_mul(
                        wnf_sb[:, ln * H:(ln + 1) * H],
                        wn_sb[:, ln * H:(ln + 1) * H], s_col)
                    ps_b = ps_rst.tile([P, H], FP, tag="rst")
                    nc.tensor.matmul(ps_b[:, 0:1],
                                     lhsT=wsum_sb[:, ln * H:(ln + 1) * H],
                                     rhs=t_col, start=True, stop=True)
                    nc.vector.tensor_add(biasf_sb[:, ln:ln + 1], ps_b[:, 0:1],
                                         bcol_sb[:, ln:ln + 1])
                # pool affine for layer j: [H, G] = s*poolT + t (core 0 only)
                ps_t = ps_tp.tile([P, P], FP, tag="tp")
                nc.tensor.transpose(out=ps_t[:],
                                    in_=pl_all[:, j * H:(j + 1) * H],
                                    identity=ident[:])
                poolT = wrk.tile([P, P], FP, tag="poolT")
                nc.scalar.copy(poolT[:], ps_t[:])
                tf = wrk.tile([P, 1], FP, tag="tf")
                nc.vector.tensor_mul(tf[:], t_col, tflag_sb[:])
                pla = wrk.tile([P, G], FP, tag="pla")
                nc.vector.scalar_tensor_tensor(
                    pla[:], poolT[:, :G], s_col, tf[:].to_broadcast([P, G]),
                    op0=mybir.AluOpType.mult, op1=mybir.AluOpType.add)
                nc.sync.dma_start(pool_loc[j], pla[:])

            def emit_block(l, bI, ps_p):
                    nn = last if bI == nblk - 1 else P
                    if l == 0:
                        ab = wrk.tile([P, H], FP, tag="mablk")
                        cnt_sb = wrk.tile([P, NVC * H], FP, tag="cntblk")
                        nc.sync.dma_start(cnt_sb[:], cnt_d[bI])
                        ps_a = ps_rst.tile([P, H], FP, tag="rst")
                        for cv in range(NVC):
                            nc.tensor.matmul(
                                ps_a[:],
                                lhsT=cnt_sb[:, cv * H:(cv + 1) * H],
                                rhs=emb_sb[:, cv * H:(cv + 1) * H],
                                start=(cv == 0), stop=(cv == NVC - 1))
                        nc.vector.tensor_copy(ab[:], ps_a[:])
                    else:
                        ab = emit_agg_block(bI)
                    ps_t = ps_tp.tile([P, P], FP, tag="tp")
                    nc.tensor.transpose(out=ps_t[:], in_=ab[:],
                                        identity=ident[:])
                    aT = wrk.tile([P, P], FP, tag="aT")
                    nc.scalar.copy(aT[:], ps_t[:])

                    if l == 0:
                        g0 = wrk.tile([P, H], FP, tag="g0")
                        nc.gpsimd.indirect_dma_start(
                            out=g0[:], out_offset=None, in_=emb_d[:],
                            in_offset=bass.IndirectOffsetOnAxis(
                                ap=feat32_sb[:, bI:bI + 1], axis=0))
                        ps_t0 = ps_tp.tile([P, P], FP, tag="tp")
                        nc.tensor.transpose(out=ps_t0[:], in_=g0[:],
                                            identity=ident[:])
                        hT = wrk.tile([P, P], FP, tag="hT")
                        nc.scalar.copy(hT[:], ps_t0[:])
                        rhs_self = hT[:]
                        lhs_s = ws_sb[:, 0:H]
                        lhs_n = wn_sb[:, 0:H]
                        bc = bcol_sb[:, 0:1]
                    else:
                        rhs_self = h_stage[:, bI * P:(bI + 1) * P]
                        lhs_s = wsf_sb[:, l * H:(l + 1) * H]
                        lhs_n = wnf_sb[:, l * H:(l + 1) * H]
                        bc = biasf_sb[:, l:l + 1]

                    ps_r = ps_rst.tile([P, H], FP, tag="rst")
                    nc.tensor.matmul(ps_r[:], lhsT=lhs_s, rhs=rhs_self,
                                     start=True, stop=False)
                    nc.tensor.matmul(ps_r[:], lhsT=lhs_n, rhs=aT[:],
                                     start=False, stop=True)

                    t1 = wrk.tile([P, P], FP, tag="t1")
                    nc.scalar.activation(t1[:], ps_r[:],
                                         mybir.ActivationFunctionType.Relu,
                                         bias=bc)
                    neg = wrk.tile([P, P], FP, tag="neg")
                    nc.vector.tensor_scalar(
                        neg[:], ps_r[:], bc, 0.0,
                        op0=mybir.AluOpType.add, op1=mybir.AluOpType.min)
                    zb = h_stage[:, bI * P:(bI + 1) * P]
                    if nn == P:
                        nc.vector.scalar_tensor_tensor(
                            zb, neg[:], acol_sb[:, l:l + 1], t1[:],
                            op0=mybir.AluOpType.mult, op1=mybir.AluOpType.add,
                            accum_out=stats_sum[:, bI:bI + 1])
                        nc.scalar.activation(scratch[:], zb,
                                             mybir.ActivationFunctionType.Square,
                                             accum_out=stats_sq[:, bI:bI + 1])
                    else:
                        nc.vector.scalar_tensor_tensor(
                            h_stage[:, bI * P:bI * P + nn],
                            neg[:, :nn], acol_sb[:, l:l + 1], t1[:, :nn],
                            op0=mybir.AluOpType.mult, op1=mybir.AluOpType.add,
                            accum_out=stats_sum[:, bI:bI + 1])
                        nc.vector.scalar_tensor_tensor(
                            h_stage[:, bI * P + nn:(bI + 1) * P],
                            neg[:, nn:], acol_sb[:, l:l + 1], t1[:, nn:],
                            op0=mybir.AluOpType.mult, op1=mybir.AluOpType.add)
                        nc.scalar.activation(
                            scratch[:, :nn], h_stage[:, bI * P:bI * P + nn],
                            mybir.ActivationFunctionType.Square,
                            accum_out=stats_sq[:, bI:bI + 1])

                    # fused tail (old pass B): transpose z to node rows,
                    # write the shard, accumulate the raw-z pool
                    ps_t2 = ps_tp.tile([P, P], FP, tag="tp")
                    nc.tensor.transpose(out=ps_t2[:], in_=zb,
                                        identity=ident[:])
                    hnm = wrk.tile([P, P], FP, tag="hnm")
                    nc.scalar.copy(hnm[:], ps_t2[:])
                    if l < L - 1:
                        hnm_bf = wrk.tile([P, P], BF, tag="hnmbf")
                        nc.scalar.copy(hnm_bf[:], ps_t2[:])
                        nc.sync.dma_start(
                            h_shard[bI * P:bI * P + nn, :], hnm_bf[:nn, :])
                    gb = wrk.tile([P, G], FP, tag="gblk")
                    nc.sync.dma_start(gb[:], gind_d[bI])
                    nc.tensor.matmul(ps_p[:G, :], lhsT=gb[:], rhs=hnm[:],
                                     start=(bI == 0), stop=(bI == nblk - 1))

            def emit_layer_tail(l, ps_p):
                nc.vector.tensor_copy(pl_all[:G, l * H:(l + 1) * H],
                                      ps_p[:G, :])
                # per-channel z sums for this layer's BN stats
                sx = wrk.tile([P, 1], FP, tag="sx")
                nc.vector.tensor_reduce(sx[:], stats_sum[:],
                                        axis=mybir.AxisListType.X,
                                        op=mybir.AluOpType.add)
                sq = wrk.tile([P, 1], FP, tag="sq")
                nc.vector.tensor_reduce(sq[:], stats_sq[:],
                                        axis=mybir.AxisListType.X,
                                        op=mybir.AluOpType.add)
                nc.sync.dma_start(stats_loc[0:1, :], sx[:, 0:1])
                nc.sync.dma_start(stats_loc[1:2, :], sq[:, 0:1])

            # ---------------- schedule ----------------
            ps_p = ps_pool.tile([P, H], FP, tag="pool")
            for bI in range(nblk):
                emit_block(0, bI, ps_p)
            emit_layer_tail(0, ps_p)
            nc.gpsimd.collective_compute(
                "AllGather", mybir.AluOpType.bypass, replica_groups=groups,
                ins=[h_shard[:]], outs=[h_full[:]])
            nc.gpsimd.collective_compute(
                "AllReduce", mybir.AluOpType.add, replica_groups=groups,
                ins=[stats_loc[:]], outs=[stats_red[:]])
            for l in range(1, L):
                ps_p = ps_pool.tile([P, H], FP, tag="pool")
                # the previous layer's stats postprocessing overlaps this
                # layer's gather stream (its deps resolve early; only the
                # first W-matmuls wait on the folded weights)
                emit_stats_post(l - 1)
                for bI in range(nblk):
                    emit_block(l, bI, ps_p)
                emit_layer_tail(l, ps_p)
                if l < L - 1:
                    nc.gpsimd.collective_compute(
                        "AllGather", mybir.AluOpType.bypass,
                        replica_groups=groups,
                        ins=[h_shard[:]], outs=[h_full[:]])
                nc.gpsimd.collective_compute(
                    "AllReduce", mybir.AluOpType.add, replica_groups=groups,
                    ins=[stats_loc[:]], outs=[stats_red[:]])
            emit_stats_post(L - 1)

            nc.gpsimd.collective_compute(
                "AllReduce", mybir.AluOpType.add, replica_groups=groups,
                ins=[pool_loc[:]], outs=[pool_red[:]])
            for l in range(L):
                pr = wrk.tile([P, G], FP, tag="pr")
                nc.sync.dma_start(pr[:], pool_red[l])
                ps_o = ps_tp.tile([P, P], FP, tag="tp")
                nc.tensor.transpose(out=ps_o[:], in_=pr[:], identity=ident[:])
                ob = wrk.tile([P, H], FP, tag="ob")
                nc.scalar.copy(ob[:], ps_o[:])
                nc.sync.dma_start(out_d[:, l * H:(l + 1) * H], ob[:G, :])

    nc.compile()
    return nc


# ---------------------------------------------------------------------------
# entry point
# ---------------------------------------------------------------------------

_CACHE = {}


def _run(cfg, inputs, trace=False):
    from concourse.bass_utils import run_bass_kernel_spmd
    in_maps, tb = _prep(cfg, **inputs)
    key = (cfg["N"], cfg["G"], cfg["H"], tb)
    if key not in _CACHE:
        _CACHE[key] = build_program(cfg, tb)
    nc = _CACHE[key]
    last_exc = None
    for attempt in range(3):
        try:
            return run_bass_kernel_spmd(nc, in_maps, list(range(N_CORES)),
                                        trace=trace)
        except Exception as e:  # rare transient device-unrecoverable errors
            last_exc = e
            try:
                import jax
                import jax.extend.backend
                jax.clear_caches()
                jax.extend.backend.clear_backends()
            except Exception:
                pass
    raise last_exc


def kernel(in_feat, src, dst, graph_ids, emb, W_self, W_neigh, b,
           gamma, beta, prelu_w):
    cfg = _mkcfg(**CFG_FULL)
    res = _run(cfg, dict(
        in_feat=in_feat, src=src, dst=dst, graph_ids=graph_ids, emb=emb,
        W_self=W_self, W_neigh=W_neigh, b=b, gamma=gamma, beta=beta,
        prelu_w=prelu_w))
    return np.asarray(res.results[0]["out"], np.float32)

